# revision 12
# baseline (speedup 1.0000x reference)
"""Trainium2 Bass kernel for nn_DPoolLSTM (social-pooling LSTM trajectory model).

Sharding: 8 cores x 256 agents (data parallel over agent rows).
Per step: neighbor top-4 search over all 2048 agents (bf16-split score matmul,
DVE max8/max_index, indirect-DMA gather, exact fp32 recheck), pool-LSTM +
main-LSTM in transposed layout (fp32r matmuls). Decode steps exchange
predicted positions across cores with an AllGather collective.
"""

import contextlib
import ctypes
import sys
import types

import numpy as np

N = 2048
RC = 256          # agents per core
NCORES = 8
T_OBS = 9
NE = 8            # encoder steps
POOL_HID = 256
POOL_OUT = 32
EMB = 64
HID = 128
LSTM_IN = 96

_SO_PATH = "/opt/axon/libaxon_pjrt.so"


def _install_ntff_hook():
    """Provide antenv.axon_hooks so run_bass_kernel_spmd(trace=True) works."""
    if "antenv.axon_hooks" in sys.modules:
        return
    state = {"hook": None}

    def set_hook(h):
        state["hook"] = h

    def get_hook():
        return state["hook"]

    mod = types.ModuleType("antenv.axon_hooks")
    mod.set_axon_ntff_profile_hook = set_hook
    mod.get_axon_ntff_profile_hook = get_hook
    sys.modules["antenv.axon_hooks"] = mod

    try:
        lib = ctypes.CDLL(_SO_PATH)
    except OSError:
        return
    if not hasattr(lib, "axon_start_nrt_profile"):
        return
    lib.axon_start_nrt_profile.argtypes = [ctypes.POINTER(ctypes.c_int64), ctypes.c_size_t]
    lib.axon_start_nrt_profile.restype = ctypes.c_int64
    lib.axon_stop_nrt_profile.argtypes = [ctypes.c_char_p]
    lib.axon_stop_nrt_profile.restype = ctypes.c_int64

    @contextlib.contextmanager
    def _hook_cm(output_dir, device_ids):
        import jax

        jax.devices()
        if device_ids:
            ids = (ctypes.c_int64 * len(device_ids))(*device_ids)
            rc = lib.axon_start_nrt_profile(ids, len(device_ids))
        else:
            rc = lib.axon_start_nrt_profile(None, 0)
        if rc != 0:
            raise RuntimeError(f"axon_start_nrt_profile rc={rc}")
        try:
            yield
        finally:
            n = lib.axon_stop_nrt_profile(str(output_dir).encode())
            print(f"ntff profile: {n} file(s) -> {output_dir}", file=sys.stderr)

    set_hook(_hook_cm)


def _split_bf16(x):
    import ml_dtypes

    hi = x.astype(ml_dtypes.bfloat16)
    lo = (x - hi.astype(np.float32)).astype(ml_dtypes.bfloat16)
    return hi, lo


def _build_module(nd):
    import concourse.bass as bass
    import concourse.tile as tile
    from concourse import bacc, mybir
    from concourse.masks import make_identity

    f32 = mybir.dt.float32
    f32r = mybir.dt.float32r
    bf16 = mybir.dt.bfloat16
    u32 = mybir.dt.uint32
    Alu = mybir.AluOpType
    Act = mybir.ActivationFunctionType

    nt = NE + nd  # total steps
    nc = bacc.Bacc("TRN2", target_bir_lowering=False, num_devices=NCORES)

    # ---- external inputs ----
    def din(name, shape, dtype=f32):
        return nc.dram_tensor(name, shape, dtype, kind="ExternalInput")

    lhs_hi_in = din("lhs_hi", [NE + 1, 4, RC], bf16)
    lhs_lo_in = din("lhs_lo", [NE + 1, 4, RC], bf16)
    rhs_hi_in = din("rhs_hi", [NE + 1, 4, N], bf16)
    rhs_lo_in = din("rhs_lo", [NE + 1, 4, N], bf16)
    vrhs_in = din("vrhs", [NE + 1, 3, RC], f32r)
    own4_in = din("own4", [NE + 1, 2, 128, 4])
    table_in = din("tab", [(NE + 1) * N, 4])
    initp_in = din("initp", [2, 2, RC])          # [prev2T, prev1T]
    iota8_in = din("iota8", [128, 8], u32)
    scabc_in = din("scabc", [5, 3])
    ones_in = din("onesr", [1, RC], f32r)
    zeros_in = din("zerosr", [2, RC], f32r)
    wblk_in = din("wblk", [17, POOL_OUT], f32r)
    wihp_in = din("wihp", [33, 4 * POOL_HID], f32r)
    whhp_in = din("whhp", [2, 128, 4 * POOL_HID], f32r)
    wh2p_in = din("wh2p", [2, 128, POOL_OUT], f32r)
    bh2p_in = din("bh2p", [1, POOL_OUT], f32r)
    wie_in = din("wie", [3, EMB - 2], f32r)
    wiha_in = din("wiha", [2, LSTM_IN + 1, 4 * HID], f32r)
    whhm_in = din("whhm", [2, 128, 4 * HID], f32r)
    whn_in = din("whn", [128, 5], f32r)
    bhn_in = din("bhn", [1, 5], f32r)

    out_nrm = nc.dram_tensor("out_nrm", [nt, 5, RC], f32, kind="ExternalOutput")
    out_pos = nc.dram_tensor("out_pos", [max(nd, 1), 2, RC], f32, kind="ExternalOutput")

    with tile.TileContext(nc) as tc:
        ex = contextlib.ExitStack()
        P = ex.enter_context  # pools live until module end

        pers = P(tc.tile_pool(name="pers", bufs=1))
        dram = P(tc.tile_pool(name="dram", bufs=2, space="DRAM"))
        ps_big = P(tc.tile_pool(name="ps_big", bufs=4, space="PSUM"))
        ps_med = P(tc.tile_pool(name="ps_med", bufs=3, space="PSUM"))
        ps_sm = P(tc.tile_pool(name="ps_sm", bufs=1, space="PSUM"))
        sb_big = P(tc.tile_pool(name="sb_big", bufs=3))
        sb_sm = P(tc.tile_pool(name="sb_sm", bufs=4))
        sb_gate = P(tc.tile_pool(name="sb_gate", bufs=3))

        # ---- persistent tiles ----
        ident_r = pers.tile([128, 128], f32r, tag="ident_r")
        ident_f = pers.tile([128, 128], f32, tag="ident_f")
        make_identity(nc, ident_f[:])
        nc.vector.tensor_copy(ident_r[:], ident_f[:])
        iota8 = pers.tile([128, 8], u32, tag="iota8")
        scabc = pers.tile([5, 3], f32, tag="scabc")
        ones_r = pers.tile([1, RC], f32r, tag="ones_r")
        nc.sync.dma_start(iota8[:], iota8_in[:])
        nc.sync.dma_start(scabc[:], scabc_in[:])
        nc.sync.dma_start(ones_r[:], ones_in[:])

        lhs_hi = pers.tile([4, RC], bf16, tag="lhs_hi")
        lhs_lo = pers.tile([4, RC], bf16, tag="lhs_lo")
        vrhs = pers.tile([3, RC], f32r, tag="vrhs")
        nc.sync.dma_start(lhs_hi[:], lhs_hi_in[NE])
        nc.sync.dma_start(lhs_lo[:], lhs_lo_in[NE])
        nc.sync.dma_start(vrhs[:], vrhs_in[NE])
        own4 = [pers.tile([128, 4], f32, tag=f"own4_{m}", name=f"own4_{m}") for m in range(2)]

        xg = pers.tile([17, RC], f32r, tag="xg")
        nc.sync.dma_start(xg[16:17, :], ones_in[:])
        gt_aug = pers.tile([33, RC], f32r, tag="gt_aug")
        nc.sync.dma_start(gt_aug[32:33, :], ones_in[:])
        xT = pers.tile([LSTM_IN + 1, RC], f32r, tag="xT")
        nc.sync.dma_start(xT[62:64, :], zeros_in[:])
        nc.sync.dma_start(xT[96:97, :], ones_in[:])

        wblk = pers.tile([17, POOL_OUT], f32r, tag="wblk")
        wihp = pers.tile([33, 4 * POOL_HID], f32r, tag="wihp")
        whhp = [pers.tile([128, 4 * POOL_HID], f32r, tag=f"whhp{k}", name=f"whhp{k}") for k in range(2)]
        wh2p = [pers.tile([128, POOL_OUT], f32r, tag=f"wh2p{k}", name=f"wh2p{k}") for k in range(2)]
        bh2p = pers.tile([1, POOL_OUT], f32r, tag="bh2p")
        wie = pers.tile([3, EMB - 2], f32r, tag="wie")
        wiha = [pers.tile([LSTM_IN + 1, 4 * HID], f32r, tag=f"wiha{k}", name=f"wiha{k}") for k in range(2)]
        whhm = [pers.tile([128, 4 * HID], f32r, tag=f"whhm{k}", name=f"whhm{k}") for k in range(2)]
        whn = pers.tile([128, 5], f32r, tag="whn")
        bhn = pers.tile([1, 5], f32r, tag="bhn")
        nc.sync.dma_start(wblk[:], wblk_in[:])
        nc.sync.dma_start(wihp[:], wihp_in[:])
        for k in range(2):
            nc.sync.dma_start(whhp[k][:], whhp_in[k])
            nc.sync.dma_start(wh2p[k][:], wh2p_in[k])
            nc.sync.dma_start(wiha[k][:], wiha_in[k])
            nc.sync.dma_start(whhm[k][:], whhm_in[k])
        nc.sync.dma_start(bh2p[:], bh2p_in[:])
        nc.sync.dma_start(wie[:], wie_in[:])
        nc.sync.dma_start(whn[:], whn_in[:])
        nc.sync.dma_start(bhn[:], bhn_in[:])

        hpT = [pers.tile([128, RC], f32r, tag=f"hpT{k}", name=f"hpT{k}") for k in range(2)]
        cpT = [pers.tile([128, RC], f32, tag=f"cpT{k}", name=f"cpT{k}") for k in range(2)]
        hT = pers.tile([128, RC], f32r, tag="hT")
        cT = pers.tile([128, RC], f32, tag="cT")
        for k in range(2):
            nc.vector.memset(cpT[k][:], 0.0)
            nc.vector.tensor_copy(hpT[k][:], cpT[k][:])
        nc.vector.memset(cT[:], 0.0)
        nc.vector.tensor_copy(hT[:], cT[:])

        posT = [pers.tile([2, RC], f32, tag=f"posT{k}", name=f"posT{k}") for k in range(3)]
        nc.sync.dma_start(posT[0][:], initp_in[0])  # prev2T
        nc.sync.dma_start(posT[1][:], initp_in[1])  # prev1T
        prev2, prev1, nxt = posT[0], posT[1], posT[2]

        table_view = table_in[:]  # [(NE+1)*N, 4]

        def neighbor_block(t, lhsh, lhsl, rhsh, rhsl, own4p, tabv, hostfed, fouts):
            for m in range(2):
                s_sb = sb_big.tile([128, N], f32, tag="s_sb", name=f"s_sb_{t}_{m}")
                for b in range(4):
                    sl = slice(512 * b, 512 * b + 512)
                    ps_s = ps_big.tile([128, 512], f32, tag="ps_s", name=f"ps_s_{t}_{m}_{b}")
                    nc.tensor.matmul(ps_s[:], lhsh[:, 128 * m:128 * m + 128],
                                     rhsh[:, sl], start=True, stop=False)
                    nc.tensor.matmul(ps_s[:], lhsh[:, 128 * m:128 * m + 128],
                                     rhsl[:, sl], start=False, stop=False)
                    nc.tensor.matmul(ps_s[:], lhsl[:, 128 * m:128 * m + 128],
                                     rhsh[:, sl], start=False, stop=True)
                    nc.scalar.copy(s_sb[:, sl], ps_s[:])
                mx = sb_sm.tile([128, 8], f32, tag="mx", name=f"mx_{t}_{m}")
                mi = sb_sm.tile([128, 8], u32, tag="mi", name=f"mi_{t}_{m}")
                nc.vector.max(mx[:], s_sb[:])
                nc.vector.max_index(mi[:], mx[:], s_sb[:])
                adj = sb_sm.tile([128, 8], u32, tag="adj", name=f"adj_{t}_{m}")
                if hostfed:
                    nc.vector.tensor_scalar(adj[:], mi[:], N * t, None, op0=Alu.add)
                else:
                    nc.vector.tensor_scalar(adj[:], mi[:], 0xFFFFFF00, None,
                                            op0=Alu.bitwise_and)
                    nc.vector.tensor_tensor(out=adj[:], in0=adj[:], in1=mi[:],
                                            op=Alu.add)
                cand = sb_sm.tile([128, 8, 4], f32, tag="cand", name=f"cand_{t}_{m}")
                for r in range(8):
                    nc.gpsimd.indirect_dma_start(
                        out=cand[:, r, :], out_offset=None, in_=tabv,
                        in_offset=bass.IndirectOffsetOnAxis(ap=adj[:, r:r + 1], axis=0))
                rel = sb_sm.tile([128, 8, 2], f32, tag="rel", name=f"rel_{t}_{m}")
                nc.vector.tensor_tensor(
                    out=rel[:], in0=cand[:, :, 0:2],
                    in1=own4p[m][:, 0:2].unsqueeze(1).to_broadcast([128, 8, 2]),
                    op=Alu.subtract)
                rel2 = sb_sm.tile([128, 8, 2], f32, tag="rel2", name=f"rel2_{t}_{m}")
                nc.vector.tensor_tensor(out=rel2[:], in0=rel[:], in1=rel[:],
                                        op=Alu.mult)
                d2 = sb_sm.tile([128, 8], f32, tag="d2", name=f"d2_{t}_{m}")
                nc.vector.reduce_sum(d2[:], rel2[:], axis=mybir.AxisListType.X)
                d2n = sb_sm.tile([128, 8], f32, tag="d2n", name=f"d2n_{t}_{m}")
                nc.vector.tensor_scalar(d2n[:], d2[:], -1.0, None, op0=Alu.mult)
                srt = sb_sm.tile([128, 8], f32, tag="srt", name=f"srt_{t}_{m}")
                ordv = sb_sm.tile([128, 8], u32, tag="ordv", name=f"ordv_{t}_{m}")
                nc.vector.max(srt[:], d2n[:])
                nc.vector.max_index(ordv[:], srt[:], d2n[:])
                mask4 = sb_sm.tile([128, 4, 8], f32, tag="mask4", name=f"mask4_{t}_{m}")
                nc.vector.tensor_tensor(
                    out=mask4[:],
                    in0=ordv[:, 1:5].unsqueeze(2).to_broadcast([128, 4, 8]),
                    in1=iota8[:].unsqueeze(1).to_broadcast([128, 4, 8]),
                    op=Alu.is_equal)
                prod4 = sb_sm.tile([128, 4, 4, 8], f32, tag="prod4", name=f"prod4_{t}_{m}")
                nc.vector.tensor_tensor(
                    out=prod4[:],
                    in0=mask4[:].unsqueeze(2).to_broadcast([128, 4, 4, 8]),
                    in1=cand[:].rearrange("p s e -> p e s").unsqueeze(1)
                        .to_broadcast([128, 4, 4, 8]),
                    op=Alu.mult)
                selv = sb_sm.tile([128, 4, 4], f32, tag="selv", name=f"selv_{t}_{m}")
                nc.vector.reduce_sum(selv[:], prod4[:], axis=mybir.AxisListType.X)
                nc.vector.tensor_tensor(
                    out=fouts[m][:], in0=selv[:],
                    in1=own4p[m][:].unsqueeze(1).to_broadcast([128, 4, 4]),
                    op=Alu.subtract)

        # ---- phase A: all host-fed neighbor searches, densely packed ----
        nhost = min(NE + 1, nt)
        feat_store = []
        vrhs_store = []
        for t in range(nhost):
            rhs_hi_t = sb_big.tile([4, N], bf16, tag="rhs_hi", name=f"rhsh_{t}")
            rhs_lo_t = sb_big.tile([4, N], bf16, tag="rhs_lo", name=f"rhsl_{t}")
            nc.sync.dma_start(rhs_hi_t[:], rhs_hi_in[t])
            nc.sync.dma_start(rhs_lo_t[:], rhs_lo_in[t])
            lhsh_t = sb_sm.tile([4, RC], bf16, tag="lhsA", name=f"lhsh_{t}")
            lhsl_t = sb_sm.tile([4, RC], bf16, tag="lhsB", name=f"lhsl_{t}")
            nc.sync.dma_start(lhsh_t[:], lhs_hi_in[t])
            nc.sync.dma_start(lhsl_t[:], lhs_lo_in[t])
            vr_t = pers.tile([3, RC], f32r, tag=f"vrA{t}", name=f"vrA{t}")
            nc.sync.dma_start(vr_t[:], vrhs_in[t])
            vrhs_store.append(vr_t)
            o4_t = [sb_sm.tile([128, 4], f32, tag=f"own4A{m}", name=f"own4A_{t}_{m}")
                    for m in range(2)]
            for m in range(2):
                nc.sync.dma_start(o4_t[m][:], own4_in[t, m])
            f_t = [pers.tile([128, 4, 4], f32r, tag=f"featS{t}{m}", name=f"featS{t}{m}")
                   for m in range(2)]
            neighbor_block(t, lhsh_t, lhsl_t, rhs_hi_t, rhs_lo_t, o4_t,
                           table_view, True, f_t)
            feat_store.append(f_t)

        for t in range(nt):
            enc = t < NE
            hostfed = t <= NE
            widx = 0 if enc else 1

            if hostfed:
                pass
            else:
                # rhs from last step's all-gather (ccout): blocks of 2048 f32
                # [table 1024 | xyhi 512b | xylo 512b | sqhi 512b | sqlo 512b]
                rhs_hi = sb_big.tile([4, N], bf16, tag="rhs_hi", name=f"rhshD_{t}")
                rhs_lo = sb_big.tile([4, N], bf16, tag="rhs_lo", name=f"rhslD_{t}")
                cc_bf = ccout[:].bitcast(bf16)  # [8, 4096]
                for dst, off in ((rhs_hi, 2048), (rhs_lo, 2560)):
                    # xy rows -> partitions 0..1 ; sq rows -> partitions 2..3
                    nc.sync.dma_start(
                        dst[0:2, :].rearrange("p (c j) -> p c j", c=8),
                        cc_bf[:, off:off + 512].rearrange("c (p j) -> p c j", p=2))
                    nc.sync.dma_start(
                        dst[2:4, :].rearrange("p (c j) -> p c j", c=8),
                        cc_bf[:, off + 1024:off + 1536].rearrange("c (p j) -> p c j", p=2))

            if hostfed:
                feats = feat_store[t]
            else:
                feats = [sb_sm.tile([128, 4, 4], f32r, tag=f"featD{m}",
                                    name=f"featD_{t}_{m}") for m in range(2)]
                tabv_d = ccout[:].rearrange("c (r e) -> (c r) e", e=4)
                neighbor_block(t, lhs_hi, lhs_lo, rhs_hi, rhs_lo, own4,
                               tabv_d, False, feats)

            # featT -> xg rows 0..15
            ftp = ps_sm.tile([16, RC], f32r, tag="sm")
            for m in range(2):
                nc.tensor.transpose(ftp[:, 128 * m:128 * m + 128],
                                    feats[m][:].rearrange("p a b -> p (a b)"),
                                    ident_r[:])
            nc.scalar.copy(xg[0:16, :], ftp[:])
            gps = ps_sm.tile([POOL_OUT, RC], f32, tag="sm")
            nc.tensor.matmul(gps[:], wblk[:], xg[:], start=True, stop=True)
            nc.scalar.activation(gt_aug[0:32, :], gps[:], Act.Relu)

            # pool LSTM gates: [1024, RC] in 8 tiles; order i,f,g,o x 2
            pg = []
            for mt in range(8):
                sl = slice(128 * mt, 128 * mt + 128)
                pt = ps_med.tile([128, RC], f32, tag="pg")
                nc.tensor.matmul(pt[:], wihp[:, sl], gt_aug[:], start=True, stop=False)
                nc.tensor.matmul(pt[:], whhp[0][:, sl], hpT[0][:], start=False, stop=False)
                nc.tensor.matmul(pt[:], whhp[1][:, sl], hpT[1][:], start=False, stop=True)
                pg.append(pt)
            for ht in range(2):
                i_sb = sb_gate.tile([128, RC], f32, tag="i_sb")
                f_sb = sb_gate.tile([128, RC], f32, tag="f_sb")
                g_sb = sb_gate.tile([128, RC], f32, tag="g_sb")
                o_sb = sb_gate.tile([128, RC], f32, tag="o_sb")
                nc.scalar.activation(i_sb[:], pg[0 + ht][:], Act.Sigmoid)
                nc.scalar.activation(f_sb[:], pg[2 + ht][:], Act.Sigmoid)
                nc.scalar.activation(g_sb[:], pg[4 + ht][:], Act.Tanh)
                nc.scalar.activation(o_sb[:], pg[6 + ht][:], Act.Sigmoid)
                tmp = sb_gate.tile([128, RC], f32, tag="tmp")
                nc.vector.tensor_tensor(out=tmp[:], in0=i_sb[:], in1=g_sb[:], op=Alu.mult)
                nc.vector.tensor_tensor(out=cpT[ht][:], in0=f_sb[:], in1=cpT[ht][:], op=Alu.mult)
                nc.vector.tensor_tensor(out=cpT[ht][:], in0=cpT[ht][:], in1=tmp[:], op=Alu.add)
                th = sb_gate.tile([128, RC], f32, tag="th")
                nc.scalar.activation(th[:], cpT[ht][:], Act.Tanh)
                nc.vector.tensor_tensor(out=hpT[ht][:], in0=o_sb[:], in1=th[:], op=Alu.mult)

            # pooled -> xT rows 64..95 ; emb -> xT rows 0..61
            plp = ps_sm.tile([POOL_OUT, RC], f32, tag="sm")
            nc.tensor.matmul(plp[:], wh2p[0][:], hpT[0][:], start=True, stop=False)
            nc.tensor.matmul(plp[:], wh2p[1][:], hpT[1][:], start=False, stop=False)
            nc.tensor.matmul(plp[:], bh2p[:], ones_r[:], start=False, stop=True)
            nc.scalar.copy(xT[64:96, :], plp[:])
            ebp = ps_sm.tile([EMB - 2, RC], f32, tag="sm")
            nc.tensor.matmul(ebp[:], wie[:], (vrhs_store[t] if hostfed else vrhs)[:], start=True, stop=True)
            nc.scalar.activation(xT[0:62, :], ebp[:], Act.Relu)

            # main LSTM
            mg = []
            for mt in range(4):
                sl = slice(128 * mt, 128 * mt + 128)
                gtl = ps_med.tile([128, RC], f32, tag="pg")
                nc.tensor.matmul(gtl[:], wiha[widx][:, sl], xT[:], start=True, stop=False)
                nc.tensor.matmul(gtl[:], whhm[widx][:, sl], hT[:], start=False, stop=True)
                mg.append(gtl)
            i2 = sb_gate.tile([128, RC], f32, tag="i2")
            f2 = sb_gate.tile([128, RC], f32, tag="f2")
            g2 = sb_gate.tile([128, RC], f32, tag="g2")
            o2 = sb_gate.tile([128, RC], f32, tag="o2")
            nc.scalar.activation(i2[:], mg[0][:], Act.Sigmoid)
            nc.scalar.activation(f2[:], mg[1][:], Act.Sigmoid)
            nc.scalar.activation(g2[:], mg[2][:], Act.Tanh)
            nc.scalar.activation(o2[:], mg[3][:], Act.Sigmoid)
            tmp2 = sb_gate.tile([128, RC], f32, tag="tmp2")
            nc.vector.tensor_tensor(out=tmp2[:], in0=i2[:], in1=g2[:], op=Alu.mult)
            nc.vector.tensor_tensor(out=cT[:], in0=f2[:], in1=cT[:], op=Alu.mult)
            nc.vector.tensor_tensor(out=cT[:], in0=cT[:], in1=tmp2[:], op=Alu.add)
            th2 = sb_gate.tile([128, RC], f32, tag="th2")
            nc.scalar.activation(th2[:], cT[:], Act.Tanh)
            nc.vector.tensor_tensor(out=hT[:], in0=o2[:], in1=th2[:], op=Alu.mult)

            # normal = a*raw + b*sigmoid(raw) + c
            nrp = ps_sm.tile([5, RC], f32, tag="sm")
            nc.tensor.matmul(nrp[:], whn[:], hT[:], start=True, stop=False)
            nc.tensor.matmul(nrp[:], bhn[:], ones_r[:], start=False, stop=True)
            sgm = sb_sm.tile([5, RC], f32, tag="sgm")
            nc.scalar.activation(sgm[:], nrp[:], Act.Sigmoid)
            t1 = sb_sm.tile([5, RC], f32, tag="t1n")
            nc.vector.tensor_scalar(t1[:], nrp[:], scabc[:, 0:1], None, op0=Alu.mult)
            nrm = sb_sm.tile([5, RC], f32, tag="nrm")
            nc.vector.tensor_scalar(nrm[:], sgm[:], scabc[:, 1:2], scabc[:, 2:3],
                                    op0=Alu.mult, op1=Alu.add)
            nc.vector.tensor_tensor(out=nrm[:], in0=nrm[:], in1=t1[:], op=Alu.add)
            nc.sync.dma_start(out_nrm[t], nrm[:])

            if t >= NE:
                nc.vector.tensor_tensor(out=nxt[:], in0=prev1[:], in1=nrm[0:2, :],
                                        op=Alu.add)
                nc.sync.dma_start(out_pos[t - NE], nxt[:])
                if t < nt - 1:
                    velT = sb_sm.tile([2, RC], f32, tag="velT")
                    nc.vector.tensor_tensor(out=velT[:], in0=nxt[:], in1=prev1[:],
                                            op=Alu.subtract)
                    # next-step lhsT rows 0-1 = split(2*nxt)
                    l32 = sb_sm.tile([2, RC], f32, tag="l32")
                    nc.vector.tensor_scalar(l32[:], nxt[:], 2.0, None, op0=Alu.mult)
                    nc.vector.tensor_copy(lhs_hi[0:2, :], l32[:])
                    nc.vector.tensor_tensor(out=lhs_lo[0:2, :], in0=l32[:],
                                            in1=lhs_hi[0:2, :], op=Alu.subtract)
                    nc.vector.tensor_scalar(vrhs[0:2, :], velT[:], 4.0, None,
                                            op0=Alu.mult)
                    # payload pieces
                    sq32 = sb_sm.tile([2, RC], f32, tag="sq32")
                    nc.vector.tensor_tensor(out=sq32[:], in0=nxt[:], in1=nxt[:],
                                            op=Alu.mult)
                    xyhi = sb_sm.tile([2, RC], bf16, tag="xyhi")
                    xylo = sb_sm.tile([2, RC], bf16, tag="xylo")
                    sqhi = sb_sm.tile([2, RC], bf16, tag="sqhi")
                    sqlo = sb_sm.tile([2, RC], bf16, tag="sqlo")
                    nc.vector.tensor_copy(xyhi[:], nxt[:])
                    nc.vector.tensor_tensor(out=xylo[:], in0=nxt[:], in1=xyhi[:],
                                            op=Alu.subtract)
                    nc.vector.tensor_copy(sqhi[:], sq32[:])
                    nc.vector.tensor_tensor(out=sqlo[:], in0=sq32[:], in1=sqhi[:],
                                            op=Alu.subtract)
                    # own4 for next step via PE transposes
                    for m in range(2):
                        tp = ps_sm.tile([128, 2], f32, tag="sm")
                        nc.tensor.transpose(tp[:], nxt[:, 128 * m:128 * m + 128],
                                            ident_f[0:2, 0:2])
                        nc.scalar.copy(own4[m][:, 0:2], tp[:])
                        tv = ps_sm.tile([128, 2], f32, tag="sm")
                        nc.tensor.transpose(tv[:], velT[:, 128 * m:128 * m + 128],
                                            ident_f[0:2, 0:2])
                        nc.scalar.copy(own4[m][:, 2:4], tv[:])
                    # build payload bounce and all-gather
                    bounce = dram.tile([1, 2048], f32, tag="bounce")
                    ccout = dram.tile([8, 2048], f32, tag="ccout")
                    for m in range(2):
                        nc.sync.dma_start(
                            bounce[:, 512 * m:512 * m + 512]
                            .rearrange("o (p e) -> o p e", p=128).squeeze(0),
                            own4[m][:])
                    bb = bounce[:].bitcast(bf16)  # [1, 4096]
                    nc.sync.dma_start(bb[:, 2048:2560].rearrange("o (p j) -> (o p) j", p=2), xyhi[:])
                    nc.sync.dma_start(bb[:, 2560:3072].rearrange("o (p j) -> (o p) j", p=2), xylo[:])
                    nc.sync.dma_start(bb[:, 3072:3584].rearrange("o (p j) -> (o p) j", p=2), sqhi[:])
                    nc.sync.dma_start(bb[:, 3584:4096].rearrange("o (p j) -> (o p) j", p=2), sqlo[:])
                    nc.gpsimd.collective_compute(
                        "AllGather", Alu.bypass,
                        replica_groups=[list(range(NCORES))],
                        ins=[bounce.opt()], outs=[ccout.opt()])
                prev2, prev1, nxt = prev1, nxt, prev2

        ex.close()
    nc.compile()
    return nc


_CACHE = {}


def kernel(observed, goals, batch_split, n_predict,
           W_ie, b_ie, W_pe, b_pe,
           Wih_p, Whh_p, bih_p, bhh_p, W_h2p, b_h2p,
           Wih_e, Whh_e, bih_e, bhh_e,
           Wih_d, Whh_d, bih_d, bhh_d,
           W_hn, b_hn):
    import ml_dtypes

    _install_ntff_hook()
    from concourse.bass_utils import run_bass_kernel_spmd

    observed = np.asarray(observed, np.float32)
    nd = int(n_predict)
    nt = NE + nd

    if nd not in _CACHE:
        _CACHE[nd] = _build_module(nd)
    nc = _CACHE[nd]

    # ---- host-side input prep ----
    obs1 = observed[:-1]                    # [8, N, 2]
    obs2 = observed[1:]                     # [8, N, 2]
    # step t (t=0..7): (obs1[t], obs2[t]); step 8: (observed[-2], observed[-1])
    p_all = np.concatenate([obs2, observed[-1:None]], axis=0)       # [9, N, 2]
    v_all = np.concatenate([obs2 - obs1, (observed[-1] - observed[-2])[None]], axis=0)

    sq_all = p_all * p_all                                          # [9, N, 2]
    rhs = np.concatenate([p_all, sq_all], axis=2).transpose(0, 2, 1)  # [9, 4, N]
    rhs_hi, rhs_lo = _split_bf16(rhs.astype(np.float32))

    table = np.concatenate([p_all, v_all], axis=2).astype(np.float32)  # [9, N, 4]

    iota8 = np.broadcast_to(np.arange(8, dtype=np.uint32), (128, 8)).copy()
    scabc = np.array([[1, 0, 0], [1, 0, 0], [0, 0.2, 0.01], [0, 0.2, 0.01],
                      [0, 0.7, 0]], np.float32)
    ones_row = np.ones((1, RC), np.float32)
    zeros_rows = np.zeros((2, RC), np.float32)

    W_pe = np.asarray(W_pe, np.float32)
    wblk = np.zeros((17, POOL_OUT), np.float32)
    for k in range(4):
        wblk[4 * k:4 * k + 4, 8 * k:8 * k + 8] = W_pe
    wblk[16, :] = np.tile(np.asarray(b_pe, np.float32), 4)

    wihp = np.concatenate([np.asarray(Wih_p, np.float32),
                           (np.asarray(bih_p) + np.asarray(bhh_p)).astype(np.float32)[None]], axis=0)
    whhp = np.asarray(Whh_p, np.float32).reshape(2, 128, 4 * POOL_HID)
    wh2p = np.asarray(W_h2p, np.float32).reshape(2, 128, POOL_OUT)
    bh2p = np.asarray(b_h2p, np.float32)[None]
    wie = np.concatenate([np.asarray(W_ie, np.float32),
                          np.asarray(b_ie, np.float32)[None]], axis=0)  # [3, 62]
    wiha = np.stack([
        np.concatenate([np.asarray(Wih_e, np.float32),
                        (np.asarray(bih_e) + np.asarray(bhh_e)).astype(np.float32)[None]], axis=0),
        np.concatenate([np.asarray(Wih_d, np.float32),
                        (np.asarray(bih_d) + np.asarray(bhh_d)).astype(np.float32)[None]], axis=0)])
    whhm = np.stack([np.asarray(Whh_e, np.float32), np.asarray(Whh_d, np.float32)])
    whn = np.asarray(W_hn, np.float32)
    bhn = np.asarray(b_hn, np.float32)[None]

    in_maps = []
    for c in range(NCORES):
        sl = slice(RC * c, RC * c + RC)
        pm = p_all[:, sl]                       # [9, RC, 2]
        vm = v_all[:, sl]
        lhs = np.concatenate([2 * pm.transpose(0, 2, 1),
                              -np.ones((NE + 1, 2, RC), np.float32)], axis=1)  # [9,4,RC]
        lhs_hi, lhs_lo = _split_bf16(lhs.astype(np.float32))
        vrhs = np.concatenate([4 * vm.transpose(0, 2, 1),
                               np.ones((NE + 1, 1, RC), np.float32)], axis=1)
        own4 = table[:, sl].reshape(NE + 1, 2, 128, 4)
        initp = np.stack([observed[-2, sl].T, observed[-1, sl].T])  # [2, 2, RC]
        in_maps.append({
            "lhs_hi": lhs_hi, "lhs_lo": lhs_lo,
            "rhs_hi": rhs_hi, "rhs_lo": rhs_lo,
            "vrhs": vrhs.astype(np.float32), "own4": own4.astype(np.float32),
            "tab": table.reshape(-1, 4), "initp": initp.astype(np.float32),
            "iota8": iota8, "scabc": scabc, "onesr": ones_row, "zerosr": zeros_rows,
            "wblk": wblk, "wihp": wihp, "whhp": whhp, "wh2p": wh2p, "bh2p": bh2p,
            "wie": wie, "wiha": wiha, "whhm": whhm, "whn": whn, "bhn": bhn,
        })

    kernel.last_in_maps = in_maps
    res = run_bass_kernel_spmd(nc, in_maps=in_maps, core_ids=list(range(NCORES)))
    kernel.last_results = res

    nrm = np.stack([r["out_nrm"] for r in res.results])   # [8c, nt, 5, RC]
    dpos = np.stack([r["out_pos"] for r in res.results])  # [8c, nd, 2, RC]
    normals = nrm.transpose(1, 0, 3, 2).reshape(nt, N, 5)
    dec_pos = dpos.transpose(1, 0, 3, 2).reshape(nd, N, 2)
    enc_pos = observed[1:] + normals[:NE, :, 0:2]
    positions = np.concatenate([enc_pos, dec_pos], axis=0)
    return normals.astype(np.float32), positions.astype(np.float32)


# revision 13
# speedup vs baseline: 1.0032x; 1.0032x over previous
"""Trainium2 Bass kernel for nn_DPoolLSTM (social-pooling LSTM trajectory model).

Sharding: 8 cores x 256 agents (data parallel over agent rows).
Per step: neighbor top-4 search over all 2048 agents (bf16-split score matmul,
DVE max8/max_index, indirect-DMA gather, exact fp32 recheck), pool-LSTM +
main-LSTM in transposed layout (fp32r matmuls). Decode steps exchange
predicted positions across cores with an AllGather collective.
"""

import contextlib
import ctypes
import sys
import types

import numpy as np

N = 2048
RC = 256          # agents per core
NCORES = 8
T_OBS = 9
NE = 8            # encoder steps
POOL_HID = 256
POOL_OUT = 32
EMB = 64
HID = 128
LSTM_IN = 96

_SO_PATH = "/opt/axon/libaxon_pjrt.so"


def _install_ntff_hook():
    """Provide antenv.axon_hooks so run_bass_kernel_spmd(trace=True) works."""
    if "antenv.axon_hooks" in sys.modules:
        return
    state = {"hook": None}

    def set_hook(h):
        state["hook"] = h

    def get_hook():
        return state["hook"]

    mod = types.ModuleType("antenv.axon_hooks")
    mod.set_axon_ntff_profile_hook = set_hook
    mod.get_axon_ntff_profile_hook = get_hook
    sys.modules["antenv.axon_hooks"] = mod

    try:
        lib = ctypes.CDLL(_SO_PATH)
    except OSError:
        return
    if not hasattr(lib, "axon_start_nrt_profile"):
        return
    lib.axon_start_nrt_profile.argtypes = [ctypes.POINTER(ctypes.c_int64), ctypes.c_size_t]
    lib.axon_start_nrt_profile.restype = ctypes.c_int64
    lib.axon_stop_nrt_profile.argtypes = [ctypes.c_char_p]
    lib.axon_stop_nrt_profile.restype = ctypes.c_int64

    @contextlib.contextmanager
    def _hook_cm(output_dir, device_ids):
        import jax

        jax.devices()
        if device_ids:
            ids = (ctypes.c_int64 * len(device_ids))(*device_ids)
            rc = lib.axon_start_nrt_profile(ids, len(device_ids))
        else:
            rc = lib.axon_start_nrt_profile(None, 0)
        if rc != 0:
            raise RuntimeError(f"axon_start_nrt_profile rc={rc}")
        try:
            yield
        finally:
            n = lib.axon_stop_nrt_profile(str(output_dir).encode())
            print(f"ntff profile: {n} file(s) -> {output_dir}", file=sys.stderr)

    set_hook(_hook_cm)


def _split_bf16(x):
    import ml_dtypes

    hi = x.astype(ml_dtypes.bfloat16)
    lo = (x - hi.astype(np.float32)).astype(ml_dtypes.bfloat16)
    return hi, lo


def _build_module(nd):
    import concourse.bass as bass
    import concourse.tile as tile
    from concourse import bacc, mybir
    from concourse.masks import make_identity

    f32 = mybir.dt.float32
    f32r = mybir.dt.float32r
    bf16 = mybir.dt.bfloat16
    u32 = mybir.dt.uint32
    Alu = mybir.AluOpType
    Act = mybir.ActivationFunctionType

    nt = NE + nd  # total steps
    nc = bacc.Bacc("TRN2", target_bir_lowering=False, num_devices=NCORES)

    # ---- external inputs ----
    def din(name, shape, dtype=f32):
        return nc.dram_tensor(name, shape, dtype, kind="ExternalInput")

    lhs_hi_in = din("lhs_hi", [NE + 1, 4, RC], bf16)
    lhs_lo_in = din("lhs_lo", [NE + 1, 4, RC], bf16)
    rhs_hi_in = din("rhs_hi", [NE + 1, 4, N], bf16)
    rhs_lo_in = din("rhs_lo", [NE + 1, 4, N], bf16)
    vrhs_in = din("vrhs", [NE + 1, 3, RC], f32r)
    own4_in = din("own4", [NE + 1, 2, 128, 4])
    table_in = din("tab", [(NE + 1) * N, 4])
    initp_in = din("initp", [2, 2, RC])          # [prev2T, prev1T]
    iota8_in = din("iota8", [128, 8], u32)
    scabc_in = din("scabc", [5, 3])
    ones_in = din("onesr", [1, RC], f32r)
    zeros_in = din("zerosr", [2, RC], f32r)
    wblk_in = din("wblk", [17, POOL_OUT], f32r)
    wihp_in = din("wihp", [33, 4 * POOL_HID], f32r)
    whhp_in = din("whhp", [2, 128, 4 * POOL_HID], f32r)
    wh2p_in = din("wh2p", [2, 128, POOL_OUT], f32r)
    bh2p_in = din("bh2p", [1, POOL_OUT], f32r)
    wie_in = din("wie", [3, EMB - 2], f32r)
    wiha_in = din("wiha", [2, LSTM_IN + 1, 4 * HID], f32r)
    whhm_in = din("whhm", [2, 128, 4 * HID], f32r)
    whn_in = din("whn", [128, 5], f32r)
    bhn_in = din("bhn", [1, 5], f32r)

    out_nrm = nc.dram_tensor("out_nrm", [nt, 5, RC], f32, kind="ExternalOutput")
    out_pos = nc.dram_tensor("out_pos", [max(nd, 1), 2, RC], f32, kind="ExternalOutput")

    with tile.TileContext(nc) as tc:
        ex = contextlib.ExitStack()
        P = ex.enter_context  # pools live until module end

        pers = P(tc.tile_pool(name="pers", bufs=1))
        dram = P(tc.tile_pool(name="dram", bufs=2, space="DRAM"))
        ps_big = P(tc.tile_pool(name="ps_big", bufs=3, space="PSUM"))
        ps_med = P(tc.tile_pool(name="ps_med", bufs=4, space="PSUM"))
        ps_sm = P(tc.tile_pool(name="ps_sm", bufs=1, space="PSUM"))
        sb_big = P(tc.tile_pool(name="sb_big", bufs=3))
        sb_sm = P(tc.tile_pool(name="sb_sm", bufs=4))
        sb_gate = P(tc.tile_pool(name="sb_gate", bufs=4))

        # ---- persistent tiles ----
        ident_r = pers.tile([128, 128], f32r, tag="ident_r")
        ident_f = pers.tile([128, 128], f32, tag="ident_f")
        make_identity(nc, ident_f[:])
        nc.vector.tensor_copy(ident_r[:], ident_f[:])
        iota8 = pers.tile([128, 8], u32, tag="iota8")
        scabc = pers.tile([5, 3], f32, tag="scabc")
        ones_r = pers.tile([1, RC], f32r, tag="ones_r")
        nc.sync.dma_start(iota8[:], iota8_in[:])
        nc.sync.dma_start(scabc[:], scabc_in[:])
        nc.sync.dma_start(ones_r[:], ones_in[:])

        lhs_hi = pers.tile([4, RC], bf16, tag="lhs_hi")
        lhs_lo = pers.tile([4, RC], bf16, tag="lhs_lo")
        vrhs = pers.tile([3, RC], f32r, tag="vrhs")
        nc.sync.dma_start(lhs_hi[:], lhs_hi_in[NE])
        nc.sync.dma_start(lhs_lo[:], lhs_lo_in[NE])
        nc.sync.dma_start(vrhs[:], vrhs_in[NE])
        own4 = [pers.tile([128, 4], f32, tag=f"own4_{m}", name=f"own4_{m}") for m in range(2)]

        xg = pers.tile([17, RC], f32r, tag="xg")
        nc.sync.dma_start(xg[16:17, :], ones_in[:])
        gt_aug = pers.tile([33, RC], f32r, tag="gt_aug")
        nc.sync.dma_start(gt_aug[32:33, :], ones_in[:])
        xT = pers.tile([LSTM_IN + 1, RC], f32r, tag="xT")
        nc.sync.dma_start(xT[62:64, :], zeros_in[:])
        nc.sync.dma_start(xT[96:97, :], ones_in[:])

        wblk = pers.tile([17, POOL_OUT], f32r, tag="wblk")
        wihp = pers.tile([33, 4 * POOL_HID], f32r, tag="wihp")
        whhp = [pers.tile([128, 4 * POOL_HID], f32r, tag=f"whhp{k}", name=f"whhp{k}") for k in range(2)]
        wh2p = [pers.tile([128, POOL_OUT], f32r, tag=f"wh2p{k}", name=f"wh2p{k}") for k in range(2)]
        bh2p = pers.tile([1, POOL_OUT], f32r, tag="bh2p")
        wie = pers.tile([3, EMB - 2], f32r, tag="wie")
        wiha = [pers.tile([LSTM_IN + 1, 4 * HID], f32r, tag=f"wiha{k}", name=f"wiha{k}") for k in range(2)]
        whhm = [pers.tile([128, 4 * HID], f32r, tag=f"whhm{k}", name=f"whhm{k}") for k in range(2)]
        whn = pers.tile([128, 5], f32r, tag="whn")
        bhn = pers.tile([1, 5], f32r, tag="bhn")
        nc.sync.dma_start(wblk[:], wblk_in[:])
        nc.sync.dma_start(wihp[:], wihp_in[:])
        for k in range(2):
            nc.sync.dma_start(whhp[k][:], whhp_in[k])
            nc.sync.dma_start(wh2p[k][:], wh2p_in[k])
            nc.sync.dma_start(wiha[k][:], wiha_in[k])
            nc.sync.dma_start(whhm[k][:], whhm_in[k])
        nc.sync.dma_start(bh2p[:], bh2p_in[:])
        nc.sync.dma_start(wie[:], wie_in[:])
        nc.sync.dma_start(whn[:], whn_in[:])
        nc.sync.dma_start(bhn[:], bhn_in[:])

        hpT = [pers.tile([128, RC], f32r, tag=f"hpT{k}", name=f"hpT{k}") for k in range(2)]
        cpT = [pers.tile([128, RC], f32, tag=f"cpT{k}", name=f"cpT{k}") for k in range(2)]
        hT = pers.tile([128, RC], f32r, tag="hT")
        cT = pers.tile([128, RC], f32, tag="cT")
        for k in range(2):
            nc.vector.memset(cpT[k][:], 0.0)
            nc.vector.tensor_copy(hpT[k][:], cpT[k][:])
        nc.vector.memset(cT[:], 0.0)
        nc.vector.tensor_copy(hT[:], cT[:])

        posT = [pers.tile([2, RC], f32, tag=f"posT{k}", name=f"posT{k}") for k in range(3)]
        nc.sync.dma_start(posT[0][:], initp_in[0])  # prev2T
        nc.sync.dma_start(posT[1][:], initp_in[1])  # prev1T
        prev2, prev1, nxt = posT[0], posT[1], posT[2]

        table_view = table_in[:]  # [(NE+1)*N, 4]

        def neighbor_block(t, lhsh, lhsl, rhsh, rhsl, own4p, tabv, hostfed, fouts):
            for m in range(2):
                s_sb = sb_big.tile([128, N], f32, tag="s_sb", name=f"s_sb_{t}_{m}")
                for b in range(4):
                    sl = slice(512 * b, 512 * b + 512)
                    ps_s = ps_big.tile([128, 512], f32, tag="ps_s", name=f"ps_s_{t}_{m}_{b}")
                    nc.tensor.matmul(ps_s[:], lhsh[:, 128 * m:128 * m + 128],
                                     rhsh[:, sl], start=True, stop=False)
                    nc.tensor.matmul(ps_s[:], lhsh[:, 128 * m:128 * m + 128],
                                     rhsl[:, sl], start=False, stop=False)
                    nc.tensor.matmul(ps_s[:], lhsl[:, 128 * m:128 * m + 128],
                                     rhsh[:, sl], start=False, stop=True)
                    nc.scalar.copy(s_sb[:, sl], ps_s[:])
                mx = sb_sm.tile([128, 8], f32, tag="mx", name=f"mx_{t}_{m}")
                mi = sb_sm.tile([128, 8], u32, tag="mi", name=f"mi_{t}_{m}")
                nc.vector.max(mx[:], s_sb[:])
                nc.vector.max_index(mi[:], mx[:], s_sb[:])
                adj = sb_sm.tile([128, 8], u32, tag="adj", name=f"adj_{t}_{m}")
                if hostfed:
                    nc.vector.tensor_scalar(adj[:], mi[:], N * t, None, op0=Alu.add)
                else:
                    nc.vector.tensor_scalar(adj[:], mi[:], 0xFFFFFF00, None,
                                            op0=Alu.bitwise_and)
                    nc.vector.tensor_tensor(out=adj[:], in0=adj[:], in1=mi[:],
                                            op=Alu.add)
                cand = sb_sm.tile([128, 8, 4], f32, tag="cand", name=f"cand_{t}_{m}")
                for r in range(8):
                    nc.gpsimd.indirect_dma_start(
                        out=cand[:, r, :], out_offset=None, in_=tabv,
                        in_offset=bass.IndirectOffsetOnAxis(ap=adj[:, r:r + 1], axis=0))
                rel = sb_sm.tile([128, 8, 2], f32, tag="rel", name=f"rel_{t}_{m}")
                nc.vector.tensor_tensor(
                    out=rel[:], in0=cand[:, :, 0:2],
                    in1=own4p[m][:, 0:2].unsqueeze(1).to_broadcast([128, 8, 2]),
                    op=Alu.subtract)
                rel2 = sb_sm.tile([128, 8, 2], f32, tag="rel2", name=f"rel2_{t}_{m}")
                nc.vector.tensor_tensor(out=rel2[:], in0=rel[:], in1=rel[:],
                                        op=Alu.mult)
                d2 = sb_sm.tile([128, 8], f32, tag="d2", name=f"d2_{t}_{m}")
                nc.vector.reduce_sum(d2[:], rel2[:], axis=mybir.AxisListType.X)
                d2n = sb_sm.tile([128, 8], f32, tag="d2n", name=f"d2n_{t}_{m}")
                nc.vector.tensor_scalar(d2n[:], d2[:], -1.0, None, op0=Alu.mult)
                srt = sb_sm.tile([128, 8], f32, tag="srt", name=f"srt_{t}_{m}")
                ordv = sb_sm.tile([128, 8], u32, tag="ordv", name=f"ordv_{t}_{m}")
                nc.vector.max(srt[:], d2n[:])
                nc.vector.max_index(ordv[:], srt[:], d2n[:])
                mask4 = sb_sm.tile([128, 4, 8], f32, tag="mask4", name=f"mask4_{t}_{m}")
                nc.vector.tensor_tensor(
                    out=mask4[:],
                    in0=ordv[:, 1:5].unsqueeze(2).to_broadcast([128, 4, 8]),
                    in1=iota8[:].unsqueeze(1).to_broadcast([128, 4, 8]),
                    op=Alu.is_equal)
                prod4 = sb_sm.tile([128, 4, 4, 8], f32, tag="prod4", name=f"prod4_{t}_{m}")
                nc.vector.tensor_tensor(
                    out=prod4[:],
                    in0=mask4[:].unsqueeze(2).to_broadcast([128, 4, 4, 8]),
                    in1=cand[:].rearrange("p s e -> p e s").unsqueeze(1)
                        .to_broadcast([128, 4, 4, 8]),
                    op=Alu.mult)
                selv = sb_sm.tile([128, 4, 4], f32, tag="selv", name=f"selv_{t}_{m}")
                nc.vector.reduce_sum(selv[:], prod4[:], axis=mybir.AxisListType.X)
                nc.vector.tensor_tensor(
                    out=fouts[m][:], in0=selv[:],
                    in1=own4p[m][:].unsqueeze(1).to_broadcast([128, 4, 4]),
                    op=Alu.subtract)

        # ---- phase A: all host-fed neighbor searches, densely packed ----
        nhost = min(NE + 1, nt)
        feat_store = []
        vrhs_store = []
        for t in range(nhost):
            rhs_hi_t = sb_big.tile([4, N], bf16, tag="rhs_hi", name=f"rhsh_{t}")
            rhs_lo_t = sb_big.tile([4, N], bf16, tag="rhs_lo", name=f"rhsl_{t}")
            nc.sync.dma_start(rhs_hi_t[:], rhs_hi_in[t])
            nc.sync.dma_start(rhs_lo_t[:], rhs_lo_in[t])
            lhsh_t = sb_sm.tile([4, RC], bf16, tag="lhsA", name=f"lhsh_{t}")
            lhsl_t = sb_sm.tile([4, RC], bf16, tag="lhsB", name=f"lhsl_{t}")
            nc.sync.dma_start(lhsh_t[:], lhs_hi_in[t])
            nc.sync.dma_start(lhsl_t[:], lhs_lo_in[t])
            vr_t = pers.tile([3, RC], f32r, tag=f"vrA{t}", name=f"vrA{t}")
            nc.sync.dma_start(vr_t[:], vrhs_in[t])
            vrhs_store.append(vr_t)
            o4_t = [sb_sm.tile([128, 4], f32, tag=f"own4A{m}", name=f"own4A_{t}_{m}")
                    for m in range(2)]
            for m in range(2):
                nc.sync.dma_start(o4_t[m][:], own4_in[t, m])
            f_t = [pers.tile([128, 4, 4], f32r, tag=f"featS{t}{m}", name=f"featS{t}{m}")
                   for m in range(2)]
            neighbor_block(t, lhsh_t, lhsl_t, rhs_hi_t, rhs_lo_t, o4_t,
                           table_view, True, f_t)
            feat_store.append(f_t)

        for t in range(nt):
            enc = t < NE
            hostfed = t <= NE
            widx = 0 if enc else 1

            if hostfed:
                pass
            else:
                # rhs from last step's all-gather (ccout): blocks of 2048 f32
                # [table 1024 | xyhi 512b | xylo 512b | sqhi 512b | sqlo 512b]
                rhs_hi = sb_big.tile([4, N], bf16, tag="rhs_hi", name=f"rhshD_{t}")
                rhs_lo = sb_big.tile([4, N], bf16, tag="rhs_lo", name=f"rhslD_{t}")
                cc_bf = ccout[:].bitcast(bf16)  # [8, 4096]
                for dst, off in ((rhs_hi, 2048), (rhs_lo, 2560)):
                    # xy rows -> partitions 0..1 ; sq rows -> partitions 2..3
                    nc.sync.dma_start(
                        dst[0:2, :].rearrange("p (c j) -> p c j", c=8),
                        cc_bf[:, off:off + 512].rearrange("c (p j) -> p c j", p=2))
                    nc.sync.dma_start(
                        dst[2:4, :].rearrange("p (c j) -> p c j", c=8),
                        cc_bf[:, off + 1024:off + 1536].rearrange("c (p j) -> p c j", p=2))

            if hostfed:
                feats = feat_store[t]
            else:
                feats = [sb_sm.tile([128, 4, 4], f32r, tag=f"featD{m}",
                                    name=f"featD_{t}_{m}") for m in range(2)]
                tabv_d = ccout[:].rearrange("c (r e) -> (c r) e", e=4)
                neighbor_block(t, lhs_hi, lhs_lo, rhs_hi, rhs_lo, own4,
                               tabv_d, False, feats)

            # featT -> xg rows 0..15
            ftp = ps_sm.tile([16, RC], f32r, tag="sm")
            for m in range(2):
                nc.tensor.transpose(ftp[:, 128 * m:128 * m + 128],
                                    feats[m][:].rearrange("p a b -> p (a b)"),
                                    ident_r[:])
            nc.scalar.copy(xg[0:16, :], ftp[:])
            gps = ps_sm.tile([POOL_OUT, RC], f32, tag="sm")
            nc.tensor.matmul(gps[:], wblk[:], xg[:], start=True, stop=True)
            nc.scalar.activation(gt_aug[0:32, :], gps[:], Act.Relu)

            # pool LSTM gates: [1024, RC] in 8 tiles; order i,f,g,o x 2
            pg = []
            for mt in range(8):
                sl = slice(128 * mt, 128 * mt + 128)
                pt = ps_med.tile([128, RC], f32, tag="pg")
                nc.tensor.matmul(pt[:], wihp[:, sl], gt_aug[:], start=True, stop=False)
                nc.tensor.matmul(pt[:], whhp[0][:, sl], hpT[0][:], start=False, stop=False)
                nc.tensor.matmul(pt[:], whhp[1][:, sl], hpT[1][:], start=False, stop=True)
                pg.append(pt)
            for ht in range(2):
                i_sb = sb_gate.tile([128, RC], f32, tag="i_sb")
                f_sb = sb_gate.tile([128, RC], f32, tag="f_sb")
                g_sb = sb_gate.tile([128, RC], f32, tag="g_sb")
                o_sb = sb_gate.tile([128, RC], f32, tag="o_sb")
                nc.scalar.activation(i_sb[:], pg[0 + ht][:], Act.Sigmoid)
                nc.scalar.activation(f_sb[:], pg[2 + ht][:], Act.Sigmoid)
                nc.scalar.activation(g_sb[:], pg[4 + ht][:], Act.Tanh)
                nc.scalar.activation(o_sb[:], pg[6 + ht][:], Act.Sigmoid)
                tmp = sb_gate.tile([128, RC], f32, tag="tmp")
                nc.vector.tensor_tensor(out=tmp[:], in0=i_sb[:], in1=g_sb[:], op=Alu.mult)
                nc.vector.tensor_tensor(out=cpT[ht][:], in0=f_sb[:], in1=cpT[ht][:], op=Alu.mult)
                nc.vector.tensor_tensor(out=cpT[ht][:], in0=cpT[ht][:], in1=tmp[:], op=Alu.add)
                th = sb_gate.tile([128, RC], f32, tag="th")
                nc.scalar.activation(th[:], cpT[ht][:], Act.Tanh)
                nc.vector.tensor_tensor(out=hpT[ht][:], in0=o_sb[:], in1=th[:], op=Alu.mult)

            # pooled -> xT rows 64..95 ; emb -> xT rows 0..61
            plp = ps_sm.tile([POOL_OUT, RC], f32, tag="sm")
            nc.tensor.matmul(plp[:], wh2p[0][:], hpT[0][:], start=True, stop=False)
            nc.tensor.matmul(plp[:], wh2p[1][:], hpT[1][:], start=False, stop=False)
            nc.tensor.matmul(plp[:], bh2p[:], ones_r[:], start=False, stop=True)
            nc.scalar.copy(xT[64:96, :], plp[:])
            ebp = ps_sm.tile([EMB - 2, RC], f32, tag="sm")
            nc.tensor.matmul(ebp[:], wie[:], (vrhs_store[t] if hostfed else vrhs)[:], start=True, stop=True)
            nc.scalar.activation(xT[0:62, :], ebp[:], Act.Relu)

            # main LSTM
            mg = []
            for mt in range(4):
                sl = slice(128 * mt, 128 * mt + 128)
                gtl = ps_med.tile([128, RC], f32, tag="pg")
                nc.tensor.matmul(gtl[:], wiha[widx][:, sl], xT[:], start=True, stop=False)
                nc.tensor.matmul(gtl[:], whhm[widx][:, sl], hT[:], start=False, stop=True)
                mg.append(gtl)
            i2 = sb_gate.tile([128, RC], f32, tag="i2")
            f2 = sb_gate.tile([128, RC], f32, tag="f2")
            g2 = sb_gate.tile([128, RC], f32, tag="g2")
            o2 = sb_gate.tile([128, RC], f32, tag="o2")
            nc.scalar.activation(i2[:], mg[0][:], Act.Sigmoid)
            nc.scalar.activation(f2[:], mg[1][:], Act.Sigmoid)
            nc.scalar.activation(g2[:], mg[2][:], Act.Tanh)
            nc.scalar.activation(o2[:], mg[3][:], Act.Sigmoid)
            tmp2 = sb_gate.tile([128, RC], f32, tag="tmp2")
            nc.vector.tensor_tensor(out=tmp2[:], in0=i2[:], in1=g2[:], op=Alu.mult)
            nc.vector.tensor_tensor(out=cT[:], in0=f2[:], in1=cT[:], op=Alu.mult)
            nc.vector.tensor_tensor(out=cT[:], in0=cT[:], in1=tmp2[:], op=Alu.add)
            th2 = sb_gate.tile([128, RC], f32, tag="th2")
            nc.scalar.activation(th2[:], cT[:], Act.Tanh)
            nc.vector.tensor_tensor(out=hT[:], in0=o2[:], in1=th2[:], op=Alu.mult)

            # normal = a*raw + b*sigmoid(raw) + c
            nrp = ps_sm.tile([5, RC], f32, tag="sm")
            nc.tensor.matmul(nrp[:], whn[:], hT[:], start=True, stop=False)
            nc.tensor.matmul(nrp[:], bhn[:], ones_r[:], start=False, stop=True)
            sgm = sb_sm.tile([5, RC], f32, tag="sgm")
            nc.scalar.activation(sgm[:], nrp[:], Act.Sigmoid)
            t1 = sb_sm.tile([5, RC], f32, tag="t1n")
            nc.vector.tensor_scalar(t1[:], nrp[:], scabc[:, 0:1], None, op0=Alu.mult)
            nrm = sb_sm.tile([5, RC], f32, tag="nrm")
            nc.vector.tensor_scalar(nrm[:], sgm[:], scabc[:, 1:2], scabc[:, 2:3],
                                    op0=Alu.mult, op1=Alu.add)
            nc.vector.tensor_tensor(out=nrm[:], in0=nrm[:], in1=t1[:], op=Alu.add)
            nc.sync.dma_start(out_nrm[t], nrm[:])

            if t >= NE:
                nc.vector.tensor_tensor(out=nxt[:], in0=prev1[:], in1=nrm[0:2, :],
                                        op=Alu.add)
                nc.sync.dma_start(out_pos[t - NE], nxt[:])
                if t < nt - 1:
                    velT = sb_sm.tile([2, RC], f32, tag="velT")
                    nc.vector.tensor_tensor(out=velT[:], in0=nxt[:], in1=prev1[:],
                                            op=Alu.subtract)
                    # next-step lhsT rows 0-1 = split(2*nxt)
                    l32 = sb_sm.tile([2, RC], f32, tag="l32")
                    nc.vector.tensor_scalar(l32[:], nxt[:], 2.0, None, op0=Alu.mult)
                    nc.vector.tensor_copy(lhs_hi[0:2, :], l32[:])
                    nc.vector.tensor_tensor(out=lhs_lo[0:2, :], in0=l32[:],
                                            in1=lhs_hi[0:2, :], op=Alu.subtract)
                    nc.vector.tensor_scalar(vrhs[0:2, :], velT[:], 4.0, None,
                                            op0=Alu.mult)
                    # payload pieces
                    sq32 = sb_sm.tile([2, RC], f32, tag="sq32")
                    nc.vector.tensor_tensor(out=sq32[:], in0=nxt[:], in1=nxt[:],
                                            op=Alu.mult)
                    xyhi = sb_sm.tile([2, RC], bf16, tag="xyhi")
                    xylo = sb_sm.tile([2, RC], bf16, tag="xylo")
                    sqhi = sb_sm.tile([2, RC], bf16, tag="sqhi")
                    sqlo = sb_sm.tile([2, RC], bf16, tag="sqlo")
                    nc.vector.tensor_copy(xyhi[:], nxt[:])
                    nc.vector.tensor_tensor(out=xylo[:], in0=nxt[:], in1=xyhi[:],
                                            op=Alu.subtract)
                    nc.vector.tensor_copy(sqhi[:], sq32[:])
                    nc.vector.tensor_tensor(out=sqlo[:], in0=sq32[:], in1=sqhi[:],
                                            op=Alu.subtract)
                    # own4 for next step via PE transposes
                    for m in range(2):
                        tp = ps_sm.tile([128, 2], f32, tag="sm")
                        nc.tensor.transpose(tp[:], nxt[:, 128 * m:128 * m + 128],
                                            ident_f[0:2, 0:2])
                        nc.scalar.copy(own4[m][:, 0:2], tp[:])
                        tv = ps_sm.tile([128, 2], f32, tag="sm")
                        nc.tensor.transpose(tv[:], velT[:, 128 * m:128 * m + 128],
                                            ident_f[0:2, 0:2])
                        nc.scalar.copy(own4[m][:, 2:4], tv[:])
                    # build payload bounce and all-gather
                    bounce = dram.tile([1, 2048], f32, tag="bounce")
                    ccout = dram.tile([8, 2048], f32, tag="ccout")
                    for m in range(2):
                        nc.sync.dma_start(
                            bounce[:, 512 * m:512 * m + 512]
                            .rearrange("o (p e) -> o p e", p=128).squeeze(0),
                            own4[m][:])
                    bb = bounce[:].bitcast(bf16)  # [1, 4096]
                    nc.sync.dma_start(bb[:, 2048:2560].rearrange("o (p j) -> (o p) j", p=2), xyhi[:])
                    nc.sync.dma_start(bb[:, 2560:3072].rearrange("o (p j) -> (o p) j", p=2), xylo[:])
                    nc.sync.dma_start(bb[:, 3072:3584].rearrange("o (p j) -> (o p) j", p=2), sqhi[:])
                    nc.sync.dma_start(bb[:, 3584:4096].rearrange("o (p j) -> (o p) j", p=2), sqlo[:])
                    nc.gpsimd.collective_compute(
                        "AllGather", Alu.bypass,
                        replica_groups=[list(range(NCORES))],
                        ins=[bounce.opt()], outs=[ccout.opt()])
                prev2, prev1, nxt = prev1, nxt, prev2

        ex.close()
    nc.compile()
    return nc


_CACHE = {}


def kernel(observed, goals, batch_split, n_predict,
           W_ie, b_ie, W_pe, b_pe,
           Wih_p, Whh_p, bih_p, bhh_p, W_h2p, b_h2p,
           Wih_e, Whh_e, bih_e, bhh_e,
           Wih_d, Whh_d, bih_d, bhh_d,
           W_hn, b_hn):
    import ml_dtypes

    _install_ntff_hook()
    from concourse.bass_utils import run_bass_kernel_spmd

    observed = np.asarray(observed, np.float32)
    nd = int(n_predict)
    nt = NE + nd

    if nd not in _CACHE:
        _CACHE[nd] = _build_module(nd)
    nc = _CACHE[nd]

    # ---- host-side input prep ----
    obs1 = observed[:-1]                    # [8, N, 2]
    obs2 = observed[1:]                     # [8, N, 2]
    # step t (t=0..7): (obs1[t], obs2[t]); step 8: (observed[-2], observed[-1])
    p_all = np.concatenate([obs2, observed[-1:None]], axis=0)       # [9, N, 2]
    v_all = np.concatenate([obs2 - obs1, (observed[-1] - observed[-2])[None]], axis=0)

    sq_all = p_all * p_all                                          # [9, N, 2]
    rhs = np.concatenate([p_all, sq_all], axis=2).transpose(0, 2, 1)  # [9, 4, N]
    rhs_hi, rhs_lo = _split_bf16(rhs.astype(np.float32))

    table = np.concatenate([p_all, v_all], axis=2).astype(np.float32)  # [9, N, 4]

    iota8 = np.broadcast_to(np.arange(8, dtype=np.uint32), (128, 8)).copy()
    scabc = np.array([[1, 0, 0], [1, 0, 0], [0, 0.2, 0.01], [0, 0.2, 0.01],
                      [0, 0.7, 0]], np.float32)
    ones_row = np.ones((1, RC), np.float32)
    zeros_rows = np.zeros((2, RC), np.float32)

    W_pe = np.asarray(W_pe, np.float32)
    wblk = np.zeros((17, POOL_OUT), np.float32)
    for k in range(4):
        wblk[4 * k:4 * k + 4, 8 * k:8 * k + 8] = W_pe
    wblk[16, :] = np.tile(np.asarray(b_pe, np.float32), 4)

    wihp = np.concatenate([np.asarray(Wih_p, np.float32),
                           (np.asarray(bih_p) + np.asarray(bhh_p)).astype(np.float32)[None]], axis=0)
    whhp = np.asarray(Whh_p, np.float32).reshape(2, 128, 4 * POOL_HID)
    wh2p = np.asarray(W_h2p, np.float32).reshape(2, 128, POOL_OUT)
    bh2p = np.asarray(b_h2p, np.float32)[None]
    wie = np.concatenate([np.asarray(W_ie, np.float32),
                          np.asarray(b_ie, np.float32)[None]], axis=0)  # [3, 62]
    wiha = np.stack([
        np.concatenate([np.asarray(Wih_e, np.float32),
                        (np.asarray(bih_e) + np.asarray(bhh_e)).astype(np.float32)[None]], axis=0),
        np.concatenate([np.asarray(Wih_d, np.float32),
                        (np.asarray(bih_d) + np.asarray(bhh_d)).astype(np.float32)[None]], axis=0)])
    whhm = np.stack([np.asarray(Whh_e, np.float32), np.asarray(Whh_d, np.float32)])
    whn = np.asarray(W_hn, np.float32)
    bhn = np.asarray(b_hn, np.float32)[None]

    in_maps = []
    for c in range(NCORES):
        sl = slice(RC * c, RC * c + RC)
        pm = p_all[:, sl]                       # [9, RC, 2]
        vm = v_all[:, sl]
        lhs = np.concatenate([2 * pm.transpose(0, 2, 1),
                              -np.ones((NE + 1, 2, RC), np.float32)], axis=1)  # [9,4,RC]
        lhs_hi, lhs_lo = _split_bf16(lhs.astype(np.float32))
        vrhs = np.concatenate([4 * vm.transpose(0, 2, 1),
                               np.ones((NE + 1, 1, RC), np.float32)], axis=1)
        own4 = table[:, sl].reshape(NE + 1, 2, 128, 4)
        initp = np.stack([observed[-2, sl].T, observed[-1, sl].T])  # [2, 2, RC]
        in_maps.append({
            "lhs_hi": lhs_hi, "lhs_lo": lhs_lo,
            "rhs_hi": rhs_hi, "rhs_lo": rhs_lo,
            "vrhs": vrhs.astype(np.float32), "own4": own4.astype(np.float32),
            "tab": table.reshape(-1, 4), "initp": initp.astype(np.float32),
            "iota8": iota8, "scabc": scabc, "onesr": ones_row, "zerosr": zeros_rows,
            "wblk": wblk, "wihp": wihp, "whhp": whhp, "wh2p": wh2p, "bh2p": bh2p,
            "wie": wie, "wiha": wiha, "whhm": whhm, "whn": whn, "bhn": bhn,
        })

    kernel.last_in_maps = in_maps
    res = run_bass_kernel_spmd(nc, in_maps=in_maps, core_ids=list(range(NCORES)))
    kernel.last_results = res

    nrm = np.stack([r["out_nrm"] for r in res.results])   # [8c, nt, 5, RC]
    dpos = np.stack([r["out_pos"] for r in res.results])  # [8c, nd, 2, RC]
    normals = nrm.transpose(1, 0, 3, 2).reshape(nt, N, 5)
    dec_pos = dpos.transpose(1, 0, 3, 2).reshape(nd, N, 2)
    enc_pos = observed[1:] + normals[:NE, :, 0:2]
    positions = np.concatenate([enc_pos, dec_pos], axis=0)
    return normals.astype(np.float32), positions.astype(np.float32)


# revision 14
# speedup vs baseline: 1.0583x; 1.0550x over previous
"""Trainium2 Bass kernel for nn_DPoolLSTM (social-pooling LSTM trajectory model).

Sharding: 8 cores x 256 agents (data parallel over agent rows).
Per step: neighbor top-4 search over all 2048 agents (bf16-split score matmul,
DVE max8/max_index, indirect-DMA gather, exact fp32 recheck), pool-LSTM +
main-LSTM in transposed layout (fp32r matmuls). Decode steps exchange
predicted positions across cores with an AllGather collective.
"""

import contextlib
import ctypes
import sys
import types

import numpy as np

N = 2048
RC = 256          # agents per core
NCORES = 8
T_OBS = 9
NE = 8            # encoder steps
POOL_HID = 256
POOL_OUT = 32
EMB = 64
HID = 128
LSTM_IN = 96

_SO_PATH = "/opt/axon/libaxon_pjrt.so"


def _install_ntff_hook():
    """Provide antenv.axon_hooks so run_bass_kernel_spmd(trace=True) works."""
    if "antenv.axon_hooks" in sys.modules:
        return
    state = {"hook": None}

    def set_hook(h):
        state["hook"] = h

    def get_hook():
        return state["hook"]

    mod = types.ModuleType("antenv.axon_hooks")
    mod.set_axon_ntff_profile_hook = set_hook
    mod.get_axon_ntff_profile_hook = get_hook
    sys.modules["antenv.axon_hooks"] = mod

    try:
        lib = ctypes.CDLL(_SO_PATH)
    except OSError:
        return
    if not hasattr(lib, "axon_start_nrt_profile"):
        return
    lib.axon_start_nrt_profile.argtypes = [ctypes.POINTER(ctypes.c_int64), ctypes.c_size_t]
    lib.axon_start_nrt_profile.restype = ctypes.c_int64
    lib.axon_stop_nrt_profile.argtypes = [ctypes.c_char_p]
    lib.axon_stop_nrt_profile.restype = ctypes.c_int64

    @contextlib.contextmanager
    def _hook_cm(output_dir, device_ids):
        import jax

        jax.devices()
        if device_ids:
            ids = (ctypes.c_int64 * len(device_ids))(*device_ids)
            rc = lib.axon_start_nrt_profile(ids, len(device_ids))
        else:
            rc = lib.axon_start_nrt_profile(None, 0)
        if rc != 0:
            raise RuntimeError(f"axon_start_nrt_profile rc={rc}")
        try:
            yield
        finally:
            n = lib.axon_stop_nrt_profile(str(output_dir).encode())
            print(f"ntff profile: {n} file(s) -> {output_dir}", file=sys.stderr)

    set_hook(_hook_cm)


def _split_bf16(x):
    import ml_dtypes

    hi = x.astype(ml_dtypes.bfloat16)
    lo = (x - hi.astype(np.float32)).astype(ml_dtypes.bfloat16)
    return hi, lo


def _build_module(nd):
    import concourse.bass as bass
    import concourse.tile as tile
    from concourse import bacc, mybir
    from concourse.masks import make_identity

    f32 = mybir.dt.float32
    f32r = mybir.dt.float32r
    bf16 = mybir.dt.bfloat16
    u32 = mybir.dt.uint32
    Alu = mybir.AluOpType
    Act = mybir.ActivationFunctionType

    nt = NE + nd  # total steps
    nc = bacc.Bacc("TRN2", target_bir_lowering=False, num_devices=NCORES)

    # ---- external inputs ----
    def din(name, shape, dtype=f32):
        return nc.dram_tensor(name, shape, dtype, kind="ExternalInput")

    lhs_hi_in = din("lhs_hi", [NE + 1, 4, RC], bf16)
    lhs_lo_in = din("lhs_lo", [NE + 1, 4, RC], bf16)
    rhs_hi_in = din("rhs_hi", [NE + 1, 4, N], bf16)
    rhs_lo_in = din("rhs_lo", [NE + 1, 4, N], bf16)
    vrhs_in = din("vrhs", [NE + 1, 3, RC], f32r)
    own4_in = din("own4", [NE + 1, 2, 128, 4])
    table_in = din("tab", [(NE + 1) * N, 4])
    initp_in = din("initp", [2, 2, RC])          # [prev2T, prev1T]
    iota8_in = din("iota8", [128, 8], u32)
    scabc_in = din("scabc", [5, 3])
    ones_in = din("onesr", [1, RC], f32r)
    zeros_in = din("zerosr", [2, RC], f32r)
    wblk_in = din("wblk", [17, POOL_OUT], f32r)
    wihp_in = din("wihp", [33, 4 * POOL_HID], f32r)
    whhp_in = din("whhp", [2, 128, 4 * POOL_HID], f32r)
    wh2p_in = din("wh2p", [2, 128, POOL_OUT], f32r)
    bh2p_in = din("bh2p", [1, POOL_OUT], f32r)
    wie_in = din("wie", [3, EMB - 2], f32r)
    wiha_in = din("wiha", [2, LSTM_IN + 1, 4 * HID], f32r)
    whhm_in = din("whhm", [2, 128, 4 * HID], f32r)
    whn_in = din("whn", [128, 5], f32r)
    bhn_in = din("bhn", [1, 5], f32r)

    out_nrm = nc.dram_tensor("out_nrm", [nt, 5, RC], f32, kind="ExternalOutput")
    out_pos = nc.dram_tensor("out_pos", [max(nd, 1), 2, RC], f32, kind="ExternalOutput")

    with tile.TileContext(nc) as tc:
        ex = contextlib.ExitStack()
        P = ex.enter_context  # pools live until module end

        pers = P(tc.tile_pool(name="pers", bufs=1))
        dram = P(tc.tile_pool(name="dram", bufs=2, space="DRAM"))
        ps_big = P(tc.tile_pool(name="ps_big", bufs=3, space="PSUM"))
        ps_med = P(tc.tile_pool(name="ps_med", bufs=4, space="PSUM"))
        ps_sm = P(tc.tile_pool(name="ps_sm", bufs=1, space="PSUM"))
        sb_big = P(tc.tile_pool(name="sb_big", bufs=3))
        sb_sm = P(tc.tile_pool(name="sb_sm", bufs=4))
        sb_gate = P(tc.tile_pool(name="sb_gate", bufs=4))

        # ---- persistent tiles ----
        ident_r = pers.tile([128, 128], f32r, tag="ident_r")
        ident_f = pers.tile([128, 128], f32, tag="ident_f")
        make_identity(nc, ident_f[:])
        nc.vector.tensor_copy(ident_r[:], ident_f[:])
        iota8 = pers.tile([128, 8], u32, tag="iota8")
        scabc = pers.tile([5, 3], f32, tag="scabc")
        ones_r = pers.tile([1, RC], f32r, tag="ones_r")
        nc.sync.dma_start(iota8[:], iota8_in[:])
        nc.sync.dma_start(scabc[:], scabc_in[:])
        nc.sync.dma_start(ones_r[:], ones_in[:])

        lhs_hi = pers.tile([4, RC], bf16, tag="lhs_hi")
        lhs_lo = pers.tile([4, RC], bf16, tag="lhs_lo")
        vrhs = pers.tile([3, RC], f32r, tag="vrhs")
        nc.sync.dma_start(lhs_hi[:], lhs_hi_in[NE])
        nc.sync.dma_start(lhs_lo[:], lhs_lo_in[NE])
        nc.sync.dma_start(vrhs[:], vrhs_in[NE])
        own4 = [pers.tile([128, 4], f32, tag=f"own4_{m}", name=f"own4_{m}") for m in range(2)]

        xg = pers.tile([17, RC], f32r, tag="xg")
        nc.sync.dma_start(xg[16:17, :], ones_in[:])
        gt_aug = pers.tile([33, RC], f32r, tag="gt_aug")
        nc.sync.dma_start(gt_aug[32:33, :], ones_in[:])
        xT = pers.tile([LSTM_IN + 1, RC], f32r, tag="xT")
        nc.sync.dma_start(xT[62:64, :], zeros_in[:])
        nc.sync.dma_start(xT[96:97, :], ones_in[:])

        wblk = pers.tile([17, POOL_OUT], f32r, tag="wblk")
        wihp = pers.tile([33, 4 * POOL_HID], f32r, tag="wihp")
        whhp = [pers.tile([128, 4 * POOL_HID], f32r, tag=f"whhp{k}", name=f"whhp{k}") for k in range(2)]
        wh2p = [pers.tile([128, POOL_OUT], f32r, tag=f"wh2p{k}", name=f"wh2p{k}") for k in range(2)]
        bh2p = pers.tile([1, POOL_OUT], f32r, tag="bh2p")
        wie = pers.tile([3, EMB - 2], f32r, tag="wie")
        wiha = [pers.tile([LSTM_IN + 1, 4 * HID], f32r, tag=f"wiha{k}", name=f"wiha{k}") for k in range(2)]
        whhm = [pers.tile([128, 4 * HID], f32r, tag=f"whhm{k}", name=f"whhm{k}") for k in range(2)]
        whn = pers.tile([128, 5], f32r, tag="whn")
        bhn = pers.tile([1, 5], f32r, tag="bhn")
        nc.sync.dma_start(wblk[:], wblk_in[:])
        nc.sync.dma_start(wihp[:], wihp_in[:])
        for k in range(2):
            nc.sync.dma_start(whhp[k][:], whhp_in[k])
            nc.sync.dma_start(wh2p[k][:], wh2p_in[k])
            nc.sync.dma_start(wiha[k][:], wiha_in[k])
            nc.sync.dma_start(whhm[k][:], whhm_in[k])
        nc.sync.dma_start(bh2p[:], bh2p_in[:])
        nc.sync.dma_start(wie[:], wie_in[:])
        nc.sync.dma_start(whn[:], whn_in[:])
        nc.sync.dma_start(bhn[:], bhn_in[:])

        hpT = [pers.tile([128, RC], f32r, tag=f"hpT{k}", name=f"hpT{k}") for k in range(2)]
        cpT = [pers.tile([128, RC], f32, tag=f"cpT{k}", name=f"cpT{k}") for k in range(2)]
        hT = pers.tile([128, RC], f32r, tag="hT")
        cT = pers.tile([128, RC], f32, tag="cT")
        for k in range(2):
            nc.vector.memset(cpT[k][:], 0.0)
            nc.vector.tensor_copy(hpT[k][:], cpT[k][:])
        nc.vector.memset(cT[:], 0.0)
        nc.vector.tensor_copy(hT[:], cT[:])

        posT = [pers.tile([2, RC], f32, tag=f"posT{k}", name=f"posT{k}") for k in range(3)]
        nc.sync.dma_start(posT[0][:], initp_in[0])  # prev2T
        nc.sync.dma_start(posT[1][:], initp_in[1])  # prev1T
        prev2, prev1, nxt = posT[0], posT[1], posT[2]

        table_view = table_in[:]  # [(NE+1)*N, 4]

        def neighbor_block(t, lhsh, lhsl, rhsh, rhsl, own4p, tabv, hostfed, fouts):
            for m in range(2):
                s_sb = sb_big.tile([128, N], f32, tag="s_sb", name=f"s_sb_{t}_{m}")
                for b in range(4):
                    sl = slice(512 * b, 512 * b + 512)
                    ps_s = ps_big.tile([128, 512], f32, tag="ps_s", name=f"ps_s_{t}_{m}_{b}")
                    nc.tensor.matmul(ps_s[:], lhsh[:, 128 * m:128 * m + 128],
                                     rhsh[:, sl], start=True, stop=False)
                    nc.tensor.matmul(ps_s[:], lhsh[:, 128 * m:128 * m + 128],
                                     rhsl[:, sl], start=False, stop=False)
                    nc.tensor.matmul(ps_s[:], lhsl[:, 128 * m:128 * m + 128],
                                     rhsh[:, sl], start=False, stop=True)
                    nc.scalar.copy(s_sb[:, sl], ps_s[:])
                mx = sb_sm.tile([128, 8], f32, tag="mx", name=f"mx_{t}_{m}")
                mi = sb_sm.tile([128, 8], u32, tag="mi", name=f"mi_{t}_{m}")
                nc.vector.max(mx[:], s_sb[:])
                nc.vector.max_index(mi[:], mx[:], s_sb[:])
                adj = sb_sm.tile([128, 8], u32, tag="adj", name=f"adj_{t}_{m}")
                if hostfed:
                    nc.vector.tensor_scalar(adj[:], mi[:], N * t, None, op0=Alu.add)
                else:
                    nc.vector.tensor_scalar(adj[:], mi[:], 0xFFFFFF00, None,
                                            op0=Alu.bitwise_and)
                    nc.vector.tensor_tensor(out=adj[:], in0=adj[:], in1=mi[:],
                                            op=Alu.add)
                cand = sb_sm.tile([128, 8, 4], f32, tag="cand", name=f"cand_{t}_{m}")
                for r in range(8):
                    nc.gpsimd.indirect_dma_start(
                        out=cand[:, r, :], out_offset=None, in_=tabv,
                        in_offset=bass.IndirectOffsetOnAxis(ap=adj[:, r:r + 1], axis=0))
                rel = sb_sm.tile([128, 8, 2], f32, tag="rel", name=f"rel_{t}_{m}")
                nc.vector.tensor_tensor(
                    out=rel[:], in0=cand[:, :, 0:2],
                    in1=own4p[m][:, 0:2].unsqueeze(1).to_broadcast([128, 8, 2]),
                    op=Alu.subtract)
                rel2 = sb_sm.tile([128, 8, 2], f32, tag="rel2", name=f"rel2_{t}_{m}")
                nc.vector.tensor_tensor(out=rel2[:], in0=rel[:], in1=rel[:],
                                        op=Alu.mult)
                d2 = sb_sm.tile([128, 8], f32, tag="d2", name=f"d2_{t}_{m}")
                nc.vector.reduce_sum(d2[:], rel2[:], axis=mybir.AxisListType.X)
                d2n = sb_sm.tile([128, 8], f32, tag="d2n", name=f"d2n_{t}_{m}")
                nc.vector.tensor_scalar(d2n[:], d2[:], -1.0, None, op0=Alu.mult)
                srt = sb_sm.tile([128, 8], f32, tag="srt", name=f"srt_{t}_{m}")
                ordv = sb_sm.tile([128, 8], u32, tag="ordv", name=f"ordv_{t}_{m}")
                nc.vector.max(srt[:], d2n[:])
                nc.vector.max_index(ordv[:], srt[:], d2n[:])
                mask4 = sb_sm.tile([128, 4, 8], f32, tag="mask4", name=f"mask4_{t}_{m}")
                nc.vector.tensor_tensor(
                    out=mask4[:],
                    in0=ordv[:, 1:5].unsqueeze(2).to_broadcast([128, 4, 8]),
                    in1=iota8[:].unsqueeze(1).to_broadcast([128, 4, 8]),
                    op=Alu.is_equal)
                prod4 = sb_sm.tile([128, 4, 4, 8], f32, tag="prod4", name=f"prod4_{t}_{m}")
                nc.vector.tensor_tensor(
                    out=prod4[:],
                    in0=mask4[:].unsqueeze(2).to_broadcast([128, 4, 4, 8]),
                    in1=cand[:].rearrange("p s e -> p e s").unsqueeze(1)
                        .to_broadcast([128, 4, 4, 8]),
                    op=Alu.mult)
                selv = sb_sm.tile([128, 4, 4], f32, tag="selv", name=f"selv_{t}_{m}")
                nc.vector.reduce_sum(selv[:], prod4[:], axis=mybir.AxisListType.X)
                nc.vector.tensor_tensor(
                    out=fouts[m][:], in0=selv[:],
                    in1=own4p[m][:].unsqueeze(1).to_broadcast([128, 4, 4]),
                    op=Alu.subtract)

        # ---- phase A: all host-fed neighbor searches, densely packed ----
        nhost = min(NE + 1, nt)
        feat_store = []
        vrhs_store = []
        for t in range(nhost):
            rhs_hi_t = sb_big.tile([4, N], bf16, tag="rhs_hi", name=f"rhsh_{t}")
            rhs_lo_t = sb_big.tile([4, N], bf16, tag="rhs_lo", name=f"rhsl_{t}")
            nc.sync.dma_start(rhs_hi_t[:], rhs_hi_in[t])
            nc.sync.dma_start(rhs_lo_t[:], rhs_lo_in[t])
            lhsh_t = sb_sm.tile([4, RC], bf16, tag="lhsA", name=f"lhsh_{t}")
            lhsl_t = sb_sm.tile([4, RC], bf16, tag="lhsB", name=f"lhsl_{t}")
            nc.sync.dma_start(lhsh_t[:], lhs_hi_in[t])
            nc.sync.dma_start(lhsl_t[:], lhs_lo_in[t])
            vr_t = pers.tile([3, RC], f32r, tag=f"vrA{t}", name=f"vrA{t}")
            nc.sync.dma_start(vr_t[:], vrhs_in[t])
            vrhs_store.append(vr_t)
            o4_t = [sb_sm.tile([128, 4], f32, tag=f"own4A{m}", name=f"own4A_{t}_{m}")
                    for m in range(2)]
            for m in range(2):
                nc.sync.dma_start(o4_t[m][:], own4_in[t, m])
            f_t = [pers.tile([128, 4, 4], f32r, tag=f"featS{t}{m}", name=f"featS{t}{m}")
                   for m in range(2)]
            neighbor_block(t, lhsh_t, lhsl_t, rhs_hi_t, rhs_lo_t, o4_t,
                           table_view, True, f_t)
            feat_store.append(f_t)

        for t in range(nt):
            enc = t < NE
            hostfed = t <= NE
            widx = 0 if enc else 1

            if hostfed:
                pass
            else:
                # rhs from last step's all-gather (ccout): blocks of 2048 f32
                # [table 1024 | xyhi 512b | xylo 512b | sqhi 512b | sqlo 512b]
                rhs_hi = sb_big.tile([4, N], bf16, tag="rhs_hi", name=f"rhshD_{t}")
                rhs_lo = sb_big.tile([4, N], bf16, tag="rhs_lo", name=f"rhslD_{t}")
                cc_bf = ccout[:].bitcast(bf16)  # [8, 4096]
                for dst, off in ((rhs_hi, 2048), (rhs_lo, 2560)):
                    # xy rows -> partitions 0..1 ; sq rows -> partitions 2..3
                    nc.sync.dma_start(
                        dst[0:2, :].rearrange("p (c j) -> p c j", c=8),
                        cc_bf[:, off:off + 512].rearrange("c (p j) -> p c j", p=2))
                    nc.sync.dma_start(
                        dst[2:4, :].rearrange("p (c j) -> p c j", c=8),
                        cc_bf[:, off + 1024:off + 1536].rearrange("c (p j) -> p c j", p=2))

            if hostfed:
                feats = feat_store[t]
            else:
                feats = [sb_sm.tile([128, 4, 4], f32r, tag=f"featD{m}",
                                    name=f"featD_{t}_{m}") for m in range(2)]
                tabv_d = ccout[:].rearrange("c (r e) -> (c r) e", e=4)
                neighbor_block(t, lhs_hi, lhs_lo, rhs_hi, rhs_lo, own4,
                               tabv_d, False, feats)

            # featT -> xg rows 0..15
            ftp = ps_sm.tile([16, RC], f32r, tag="sm")
            for m in range(2):
                nc.tensor.transpose(ftp[:, 128 * m:128 * m + 128],
                                    feats[m][:].rearrange("p a b -> p (a b)"),
                                    ident_r[:])
            nc.scalar.copy(xg[0:16, :], ftp[:])
            gps = ps_sm.tile([POOL_OUT, RC], f32, tag="sm")
            nc.tensor.matmul(gps[:], wblk[:], xg[:], start=True, stop=True)
            nc.scalar.activation(gt_aug[0:32, :], gps[:], Act.Relu)

            # pool LSTM gates: [1024, RC] in 8 tiles; order i,f,g,o x 2
            pg = []
            for mt in range(8):
                sl = slice(128 * mt, 128 * mt + 128)
                pt = ps_med.tile([128, RC], f32, tag="pg")
                nc.tensor.matmul(pt[:], whhp[0][:, sl], hpT[0][:], start=True, stop=False)
                nc.tensor.matmul(pt[:], whhp[1][:, sl], hpT[1][:], start=False, stop=False)
                nc.tensor.matmul(pt[:], wihp[:, sl], gt_aug[:], start=False, stop=True)
                pg.append(pt)
            for ht in range(2):
                i_sb = sb_gate.tile([128, RC], f32, tag="i_sb")
                f_sb = sb_gate.tile([128, RC], f32, tag="f_sb")
                g_sb = sb_gate.tile([128, RC], f32, tag="g_sb")
                o_sb = sb_gate.tile([128, RC], f32, tag="o_sb")
                nc.scalar.activation(i_sb[:], pg[0 + ht][:], Act.Sigmoid)
                nc.scalar.activation(f_sb[:], pg[2 + ht][:], Act.Sigmoid)
                nc.scalar.activation(g_sb[:], pg[4 + ht][:], Act.Tanh)
                nc.scalar.activation(o_sb[:], pg[6 + ht][:], Act.Sigmoid)
                tmp = sb_gate.tile([128, RC], f32, tag="tmp")
                nc.vector.tensor_tensor(out=tmp[:], in0=i_sb[:], in1=g_sb[:], op=Alu.mult)
                nc.vector.tensor_tensor(out=cpT[ht][:], in0=f_sb[:], in1=cpT[ht][:], op=Alu.mult)
                nc.vector.tensor_tensor(out=cpT[ht][:], in0=cpT[ht][:], in1=tmp[:], op=Alu.add)
                th = sb_gate.tile([128, RC], f32, tag="th")
                nc.scalar.activation(th[:], cpT[ht][:], Act.Tanh)
                nc.vector.tensor_tensor(out=hpT[ht][:], in0=o_sb[:], in1=th[:], op=Alu.mult)

            # pooled -> xT rows 64..95 ; emb -> xT rows 0..61
            plp = ps_sm.tile([POOL_OUT, RC], f32, tag="sm")
            nc.tensor.matmul(plp[:], wh2p[0][:], hpT[0][:], start=True, stop=False)
            nc.tensor.matmul(plp[:], wh2p[1][:], hpT[1][:], start=False, stop=False)
            nc.tensor.matmul(plp[:], bh2p[:], ones_r[:], start=False, stop=True)
            nc.scalar.copy(xT[64:96, :], plp[:])
            ebp = ps_sm.tile([EMB - 2, RC], f32, tag="sm")
            nc.tensor.matmul(ebp[:], wie[:], (vrhs_store[t] if hostfed else vrhs)[:], start=True, stop=True)
            nc.scalar.activation(xT[0:62, :], ebp[:], Act.Relu)

            # main LSTM
            mg = []
            for mt in range(4):
                sl = slice(128 * mt, 128 * mt + 128)
                gtl = ps_med.tile([128, RC], f32, tag="pg")
                nc.tensor.matmul(gtl[:], whhm[widx][:, sl], hT[:], start=True, stop=False)
                nc.tensor.matmul(gtl[:], wiha[widx][:, sl], xT[:], start=False, stop=True)
                mg.append(gtl)
            i2 = sb_gate.tile([128, RC], f32, tag="i2")
            f2 = sb_gate.tile([128, RC], f32, tag="f2")
            g2 = sb_gate.tile([128, RC], f32, tag="g2")
            o2 = sb_gate.tile([128, RC], f32, tag="o2")
            nc.scalar.activation(i2[:], mg[0][:], Act.Sigmoid)
            nc.scalar.activation(f2[:], mg[1][:], Act.Sigmoid)
            nc.scalar.activation(g2[:], mg[2][:], Act.Tanh)
            nc.scalar.activation(o2[:], mg[3][:], Act.Sigmoid)
            tmp2 = sb_gate.tile([128, RC], f32, tag="tmp2")
            nc.vector.tensor_tensor(out=tmp2[:], in0=i2[:], in1=g2[:], op=Alu.mult)
            nc.vector.tensor_tensor(out=cT[:], in0=f2[:], in1=cT[:], op=Alu.mult)
            nc.vector.tensor_tensor(out=cT[:], in0=cT[:], in1=tmp2[:], op=Alu.add)
            th2 = sb_gate.tile([128, RC], f32, tag="th2")
            nc.scalar.activation(th2[:], cT[:], Act.Tanh)
            nc.vector.tensor_tensor(out=hT[:], in0=o2[:], in1=th2[:], op=Alu.mult)

            # normal = a*raw + b*sigmoid(raw) + c
            nrp = ps_sm.tile([5, RC], f32, tag="sm")
            nc.tensor.matmul(nrp[:], whn[:], hT[:], start=True, stop=False)
            nc.tensor.matmul(nrp[:], bhn[:], ones_r[:], start=False, stop=True)
            sgm = sb_sm.tile([5, RC], f32, tag="sgm")
            nc.scalar.activation(sgm[:], nrp[:], Act.Sigmoid)
            t1 = sb_sm.tile([5, RC], f32, tag="t1n")
            nc.vector.tensor_scalar(t1[:], nrp[:], scabc[:, 0:1], None, op0=Alu.mult)
            nrm = sb_sm.tile([5, RC], f32, tag="nrm")
            nc.vector.tensor_scalar(nrm[:], sgm[:], scabc[:, 1:2], scabc[:, 2:3],
                                    op0=Alu.mult, op1=Alu.add)
            nc.vector.tensor_tensor(out=nrm[:], in0=nrm[:], in1=t1[:], op=Alu.add)
            nc.sync.dma_start(out_nrm[t], nrm[:])

            if t >= NE:
                nc.vector.tensor_tensor(out=nxt[:], in0=prev1[:], in1=nrm[0:2, :],
                                        op=Alu.add)
                nc.sync.dma_start(out_pos[t - NE], nxt[:])
                if t < nt - 1:
                    velT = sb_sm.tile([2, RC], f32, tag="velT")
                    nc.vector.tensor_tensor(out=velT[:], in0=nxt[:], in1=prev1[:],
                                            op=Alu.subtract)
                    # next-step lhsT rows 0-1 = split(2*nxt)
                    l32 = sb_sm.tile([2, RC], f32, tag="l32")
                    nc.vector.tensor_scalar(l32[:], nxt[:], 2.0, None, op0=Alu.mult)
                    nc.vector.tensor_copy(lhs_hi[0:2, :], l32[:])
                    nc.vector.tensor_tensor(out=lhs_lo[0:2, :], in0=l32[:],
                                            in1=lhs_hi[0:2, :], op=Alu.subtract)
                    nc.vector.tensor_scalar(vrhs[0:2, :], velT[:], 4.0, None,
                                            op0=Alu.mult)
                    # payload pieces
                    sq32 = sb_sm.tile([2, RC], f32, tag="sq32")
                    nc.vector.tensor_tensor(out=sq32[:], in0=nxt[:], in1=nxt[:],
                                            op=Alu.mult)
                    xyhi = sb_sm.tile([2, RC], bf16, tag="xyhi")
                    xylo = sb_sm.tile([2, RC], bf16, tag="xylo")
                    sqhi = sb_sm.tile([2, RC], bf16, tag="sqhi")
                    sqlo = sb_sm.tile([2, RC], bf16, tag="sqlo")
                    nc.vector.tensor_copy(xyhi[:], nxt[:])
                    nc.vector.tensor_tensor(out=xylo[:], in0=nxt[:], in1=xyhi[:],
                                            op=Alu.subtract)
                    nc.vector.tensor_copy(sqhi[:], sq32[:])
                    nc.vector.tensor_tensor(out=sqlo[:], in0=sq32[:], in1=sqhi[:],
                                            op=Alu.subtract)
                    # own4 for next step via PE transposes
                    for m in range(2):
                        tp = ps_sm.tile([128, 2], f32, tag="sm")
                        nc.tensor.transpose(tp[:], nxt[:, 128 * m:128 * m + 128],
                                            ident_f[0:2, 0:2])
                        nc.scalar.copy(own4[m][:, 0:2], tp[:])
                        tv = ps_sm.tile([128, 2], f32, tag="sm")
                        nc.tensor.transpose(tv[:], velT[:, 128 * m:128 * m + 128],
                                            ident_f[0:2, 0:2])
                        nc.scalar.copy(own4[m][:, 2:4], tv[:])
                    # build payload bounce and all-gather
                    bounce = dram.tile([1, 2048], f32, tag="bounce")
                    ccout = dram.tile([8, 2048], f32, tag="ccout")
                    for m in range(2):
                        nc.sync.dma_start(
                            bounce[:, 512 * m:512 * m + 512]
                            .rearrange("o (p e) -> o p e", p=128).squeeze(0),
                            own4[m][:])
                    bb = bounce[:].bitcast(bf16)  # [1, 4096]
                    nc.sync.dma_start(bb[:, 2048:2560].rearrange("o (p j) -> (o p) j", p=2), xyhi[:])
                    nc.sync.dma_start(bb[:, 2560:3072].rearrange("o (p j) -> (o p) j", p=2), xylo[:])
                    nc.sync.dma_start(bb[:, 3072:3584].rearrange("o (p j) -> (o p) j", p=2), sqhi[:])
                    nc.sync.dma_start(bb[:, 3584:4096].rearrange("o (p j) -> (o p) j", p=2), sqlo[:])
                    nc.gpsimd.collective_compute(
                        "AllGather", Alu.bypass,
                        replica_groups=[list(range(NCORES))],
                        ins=[bounce.opt()], outs=[ccout.opt()])
                prev2, prev1, nxt = prev1, nxt, prev2

        ex.close()
    nc.compile()
    return nc


_CACHE = {}


def kernel(observed, goals, batch_split, n_predict,
           W_ie, b_ie, W_pe, b_pe,
           Wih_p, Whh_p, bih_p, bhh_p, W_h2p, b_h2p,
           Wih_e, Whh_e, bih_e, bhh_e,
           Wih_d, Whh_d, bih_d, bhh_d,
           W_hn, b_hn):
    import ml_dtypes

    _install_ntff_hook()
    from concourse.bass_utils import run_bass_kernel_spmd

    observed = np.asarray(observed, np.float32)
    nd = int(n_predict)
    nt = NE + nd

    if nd not in _CACHE:
        _CACHE[nd] = _build_module(nd)
    nc = _CACHE[nd]

    # ---- host-side input prep ----
    obs1 = observed[:-1]                    # [8, N, 2]
    obs2 = observed[1:]                     # [8, N, 2]
    # step t (t=0..7): (obs1[t], obs2[t]); step 8: (observed[-2], observed[-1])
    p_all = np.concatenate([obs2, observed[-1:None]], axis=0)       # [9, N, 2]
    v_all = np.concatenate([obs2 - obs1, (observed[-1] - observed[-2])[None]], axis=0)

    sq_all = p_all * p_all                                          # [9, N, 2]
    rhs = np.concatenate([p_all, sq_all], axis=2).transpose(0, 2, 1)  # [9, 4, N]
    rhs_hi, rhs_lo = _split_bf16(rhs.astype(np.float32))

    table = np.concatenate([p_all, v_all], axis=2).astype(np.float32)  # [9, N, 4]

    iota8 = np.broadcast_to(np.arange(8, dtype=np.uint32), (128, 8)).copy()
    scabc = np.array([[1, 0, 0], [1, 0, 0], [0, 0.2, 0.01], [0, 0.2, 0.01],
                      [0, 0.7, 0]], np.float32)
    ones_row = np.ones((1, RC), np.float32)
    zeros_rows = np.zeros((2, RC), np.float32)

    W_pe = np.asarray(W_pe, np.float32)
    wblk = np.zeros((17, POOL_OUT), np.float32)
    for k in range(4):
        wblk[4 * k:4 * k + 4, 8 * k:8 * k + 8] = W_pe
    wblk[16, :] = np.tile(np.asarray(b_pe, np.float32), 4)

    wihp = np.concatenate([np.asarray(Wih_p, np.float32),
                           (np.asarray(bih_p) + np.asarray(bhh_p)).astype(np.float32)[None]], axis=0)
    whhp = np.asarray(Whh_p, np.float32).reshape(2, 128, 4 * POOL_HID)
    wh2p = np.asarray(W_h2p, np.float32).reshape(2, 128, POOL_OUT)
    bh2p = np.asarray(b_h2p, np.float32)[None]
    wie = np.concatenate([np.asarray(W_ie, np.float32),
                          np.asarray(b_ie, np.float32)[None]], axis=0)  # [3, 62]
    wiha = np.stack([
        np.concatenate([np.asarray(Wih_e, np.float32),
                        (np.asarray(bih_e) + np.asarray(bhh_e)).astype(np.float32)[None]], axis=0),
        np.concatenate([np.asarray(Wih_d, np.float32),
                        (np.asarray(bih_d) + np.asarray(bhh_d)).astype(np.float32)[None]], axis=0)])
    whhm = np.stack([np.asarray(Whh_e, np.float32), np.asarray(Whh_d, np.float32)])
    whn = np.asarray(W_hn, np.float32)
    bhn = np.asarray(b_hn, np.float32)[None]

    in_maps = []
    for c in range(NCORES):
        sl = slice(RC * c, RC * c + RC)
        pm = p_all[:, sl]                       # [9, RC, 2]
        vm = v_all[:, sl]
        lhs = np.concatenate([2 * pm.transpose(0, 2, 1),
                              -np.ones((NE + 1, 2, RC), np.float32)], axis=1)  # [9,4,RC]
        lhs_hi, lhs_lo = _split_bf16(lhs.astype(np.float32))
        vrhs = np.concatenate([4 * vm.transpose(0, 2, 1),
                               np.ones((NE + 1, 1, RC), np.float32)], axis=1)
        own4 = table[:, sl].reshape(NE + 1, 2, 128, 4)
        initp = np.stack([observed[-2, sl].T, observed[-1, sl].T])  # [2, 2, RC]
        in_maps.append({
            "lhs_hi": lhs_hi, "lhs_lo": lhs_lo,
            "rhs_hi": rhs_hi, "rhs_lo": rhs_lo,
            "vrhs": vrhs.astype(np.float32), "own4": own4.astype(np.float32),
            "tab": table.reshape(-1, 4), "initp": initp.astype(np.float32),
            "iota8": iota8, "scabc": scabc, "onesr": ones_row, "zerosr": zeros_rows,
            "wblk": wblk, "wihp": wihp, "whhp": whhp, "wh2p": wh2p, "bh2p": bh2p,
            "wie": wie, "wiha": wiha, "whhm": whhm, "whn": whn, "bhn": bhn,
        })

    kernel.last_in_maps = in_maps
    res = run_bass_kernel_spmd(nc, in_maps=in_maps, core_ids=list(range(NCORES)))
    kernel.last_results = res

    nrm = np.stack([r["out_nrm"] for r in res.results])   # [8c, nt, 5, RC]
    dpos = np.stack([r["out_pos"] for r in res.results])  # [8c, nd, 2, RC]
    normals = nrm.transpose(1, 0, 3, 2).reshape(nt, N, 5)
    dec_pos = dpos.transpose(1, 0, 3, 2).reshape(nd, N, 2)
    enc_pos = observed[1:] + normals[:NE, :, 0:2]
    positions = np.concatenate([enc_pos, dec_pos], axis=0)
    return normals.astype(np.float32), positions.astype(np.float32)


# revision 15
# speedup vs baseline: 1.0742x; 1.0150x over previous
"""Trainium2 Bass kernel for nn_DPoolLSTM (social-pooling LSTM trajectory model).

Sharding: 8 cores x 256 agents (data parallel over agent rows).
Per step: neighbor top-4 search over all 2048 agents (bf16-split score matmul,
DVE max8/max_index, indirect-DMA gather, exact fp32 recheck), pool-LSTM +
main-LSTM in transposed layout (fp32r matmuls). Decode steps exchange
predicted positions across cores with an AllGather collective.
"""

import contextlib
import ctypes
import sys
import types

import numpy as np

N = 2048
RC = 256          # agents per core
NCORES = 8
T_OBS = 9
NE = 8            # encoder steps
POOL_HID = 256
POOL_OUT = 32
EMB = 64
HID = 128
LSTM_IN = 96

_SO_PATH = "/opt/axon/libaxon_pjrt.so"


def _install_ntff_hook():
    """Provide antenv.axon_hooks so run_bass_kernel_spmd(trace=True) works."""
    if "antenv.axon_hooks" in sys.modules:
        return
    state = {"hook": None}

    def set_hook(h):
        state["hook"] = h

    def get_hook():
        return state["hook"]

    mod = types.ModuleType("antenv.axon_hooks")
    mod.set_axon_ntff_profile_hook = set_hook
    mod.get_axon_ntff_profile_hook = get_hook
    sys.modules["antenv.axon_hooks"] = mod

    try:
        lib = ctypes.CDLL(_SO_PATH)
    except OSError:
        return
    if not hasattr(lib, "axon_start_nrt_profile"):
        return
    lib.axon_start_nrt_profile.argtypes = [ctypes.POINTER(ctypes.c_int64), ctypes.c_size_t]
    lib.axon_start_nrt_profile.restype = ctypes.c_int64
    lib.axon_stop_nrt_profile.argtypes = [ctypes.c_char_p]
    lib.axon_stop_nrt_profile.restype = ctypes.c_int64

    @contextlib.contextmanager
    def _hook_cm(output_dir, device_ids):
        import jax

        jax.devices()
        if device_ids:
            ids = (ctypes.c_int64 * len(device_ids))(*device_ids)
            rc = lib.axon_start_nrt_profile(ids, len(device_ids))
        else:
            rc = lib.axon_start_nrt_profile(None, 0)
        if rc != 0:
            raise RuntimeError(f"axon_start_nrt_profile rc={rc}")
        try:
            yield
        finally:
            n = lib.axon_stop_nrt_profile(str(output_dir).encode())
            print(f"ntff profile: {n} file(s) -> {output_dir}", file=sys.stderr)

    set_hook(_hook_cm)


def _split_bf16(x):
    import ml_dtypes

    hi = x.astype(ml_dtypes.bfloat16)
    lo = (x - hi.astype(np.float32)).astype(ml_dtypes.bfloat16)
    return hi, lo


def _build_module(nd):
    import concourse.bass as bass
    import concourse.tile as tile
    from concourse import bacc, mybir
    from concourse.masks import make_identity

    f32 = mybir.dt.float32
    f32r = mybir.dt.float32r
    bf16 = mybir.dt.bfloat16
    u32 = mybir.dt.uint32
    Alu = mybir.AluOpType
    Act = mybir.ActivationFunctionType

    nt = NE + nd  # total steps
    nc = bacc.Bacc("TRN2", target_bir_lowering=False, num_devices=NCORES)

    # ---- external inputs ----
    def din(name, shape, dtype=f32):
        return nc.dram_tensor(name, shape, dtype, kind="ExternalInput")

    lhs_hi_in = din("lhs_hi", [NE + 1, 4, RC], bf16)
    lhs_lo_in = din("lhs_lo", [NE + 1, 4, RC], bf16)
    rhs_hi_in = din("rhs_hi", [NE + 1, 4, N], bf16)
    rhs_lo_in = din("rhs_lo", [NE + 1, 4, N], bf16)
    vrhs_in = din("vrhs", [NE + 1, 3, RC], f32r)
    own4_in = din("own4", [NE + 1, 2, 128, 4])
    table_in = din("tab", [(NE + 1) * N, 4])
    initp_in = din("initp", [2, 2, RC])          # [prev2T, prev1T]
    iota8_in = din("iota8", [128, 8], u32)
    scabc_in = din("scabc", [5, 3])
    ones_in = din("onesr", [1, RC], f32r)
    zeros_in = din("zerosr", [2, RC], f32r)
    wblk_in = din("wblk", [17, POOL_OUT], f32r)
    wihp_in = din("wihp", [33, 4 * POOL_HID], f32r)
    whhp_in = din("whhp", [2, 128, 4 * POOL_HID], f32r)
    wh2p_in = din("wh2p", [2, 128, POOL_OUT], f32r)
    bh2p_in = din("bh2p", [1, POOL_OUT], f32r)
    wie_in = din("wie", [3, EMB - 2], f32r)
    wiha_in = din("wiha", [2, LSTM_IN + 1, 4 * HID], f32r)
    whhm_in = din("whhm", [2, 128, 4 * HID], f32r)
    whn_in = din("whn", [128, 5], f32r)
    bhn_in = din("bhn", [1, 5], f32r)

    out_nrm = nc.dram_tensor("out_nrm", [nt, 5, RC], f32, kind="ExternalOutput")
    out_pos = nc.dram_tensor("out_pos", [max(nd, 1), 2, RC], f32, kind="ExternalOutput")

    with tile.TileContext(nc) as tc:
        ex = contextlib.ExitStack()
        P = ex.enter_context  # pools live until module end

        pers = P(tc.tile_pool(name="pers", bufs=1))
        dram = P(tc.tile_pool(name="dram", bufs=2, space="DRAM"))
        ps_big = P(tc.tile_pool(name="ps_big", bufs=3, space="PSUM"))
        ps_med = P(tc.tile_pool(name="ps_med", bufs=4, space="PSUM"))
        ps_sm = P(tc.tile_pool(name="ps_sm", bufs=1, space="PSUM"))
        sb_big = P(tc.tile_pool(name="sb_big", bufs=3))
        sb_sm = P(tc.tile_pool(name="sb_sm", bufs=4))
        sb_gate = P(tc.tile_pool(name="sb_gate", bufs=4))

        # ---- persistent tiles ----
        ident_r = pers.tile([128, 128], f32r, tag="ident_r")
        ident_f = pers.tile([128, 128], f32, tag="ident_f")
        make_identity(nc, ident_f[:])
        nc.vector.tensor_copy(ident_r[:], ident_f[:])
        iota8 = pers.tile([128, 8], u32, tag="iota8")
        scabc = pers.tile([5, 3], f32, tag="scabc")
        ones_r = pers.tile([1, RC], f32r, tag="ones_r")
        nc.sync.dma_start(iota8[:], iota8_in[:])
        nc.sync.dma_start(scabc[:], scabc_in[:])
        nc.sync.dma_start(ones_r[:], ones_in[:])

        lhs_hi = pers.tile([4, RC], bf16, tag="lhs_hi")
        lhs_lo = pers.tile([4, RC], bf16, tag="lhs_lo")
        vrhs = pers.tile([3, RC], f32r, tag="vrhs")
        nc.sync.dma_start(lhs_hi[:], lhs_hi_in[NE])
        nc.sync.dma_start(lhs_lo[:], lhs_lo_in[NE])
        nc.sync.dma_start(vrhs[:], vrhs_in[NE])
        own4 = [pers.tile([128, 4], f32, tag=f"own4_{m}", name=f"own4_{m}") for m in range(2)]

        xg = pers.tile([17, RC], f32r, tag="xg")
        nc.sync.dma_start(xg[16:17, :], ones_in[:])
        gt_aug = pers.tile([33, RC], f32r, tag="gt_aug")
        nc.sync.dma_start(gt_aug[32:33, :], ones_in[:])
        xT = pers.tile([LSTM_IN + 1, RC], f32r, tag="xT")
        nc.sync.dma_start(xT[62:64, :], zeros_in[:])
        nc.sync.dma_start(xT[96:97, :], ones_in[:])

        wblk = pers.tile([17, POOL_OUT], f32r, tag="wblk")
        wihp = pers.tile([33, 4 * POOL_HID], f32r, tag="wihp")
        whhp = [pers.tile([128, 4 * POOL_HID], f32r, tag=f"whhp{k}", name=f"whhp{k}") for k in range(2)]
        wh2p = [pers.tile([128, POOL_OUT], f32r, tag=f"wh2p{k}", name=f"wh2p{k}") for k in range(2)]
        bh2p = pers.tile([1, POOL_OUT], f32r, tag="bh2p")
        wie = pers.tile([3, EMB - 2], f32r, tag="wie")
        wiha = [pers.tile([LSTM_IN + 1, 4 * HID], f32r, tag=f"wiha{k}", name=f"wiha{k}") for k in range(2)]
        whhm = [pers.tile([128, 4 * HID], f32r, tag=f"whhm{k}", name=f"whhm{k}") for k in range(2)]
        whn = pers.tile([128, 5], f32r, tag="whn")
        bhn = pers.tile([1, 5], f32r, tag="bhn")
        nc.sync.dma_start(wblk[:], wblk_in[:])
        nc.sync.dma_start(wihp[:], wihp_in[:])
        for k in range(2):
            nc.sync.dma_start(whhp[k][:], whhp_in[k])
            nc.sync.dma_start(wh2p[k][:], wh2p_in[k])
            nc.sync.dma_start(wiha[k][:], wiha_in[k])
            nc.sync.dma_start(whhm[k][:], whhm_in[k])
        nc.sync.dma_start(bh2p[:], bh2p_in[:])
        nc.sync.dma_start(wie[:], wie_in[:])
        nc.sync.dma_start(whn[:], whn_in[:])
        nc.sync.dma_start(bhn[:], bhn_in[:])

        hpT = [pers.tile([128, RC], f32r, tag=f"hpT{k}", name=f"hpT{k}") for k in range(2)]
        cpT = [pers.tile([128, RC], f32, tag=f"cpT{k}", name=f"cpT{k}") for k in range(2)]
        hT = pers.tile([128, RC], f32r, tag="hT")
        cT = pers.tile([128, RC], f32, tag="cT")
        for k in range(2):
            nc.vector.memset(cpT[k][:], 0.0)
            nc.vector.tensor_copy(hpT[k][:], cpT[k][:])
        nc.vector.memset(cT[:], 0.0)
        nc.vector.tensor_copy(hT[:], cT[:])

        posT = [pers.tile([2, RC], f32, tag=f"posT{k}", name=f"posT{k}") for k in range(3)]
        nc.sync.dma_start(posT[0][:], initp_in[0])  # prev2T
        nc.sync.dma_start(posT[1][:], initp_in[1])  # prev1T
        prev2, prev1, nxt = posT[0], posT[1], posT[2]

        table_view = table_in[:]  # [(NE+1)*N, 4]

        def neighbor_block(t, lhsh, lhsl, rhsh, rhsl, own4p, tabv, hostfed, fouts):
            for m in range(2):
                s_sb = sb_big.tile([128, N], f32, tag="s_sb", name=f"s_sb_{t}_{m}")
                for b in range(4):
                    sl = slice(512 * b, 512 * b + 512)
                    ps_s = ps_big.tile([128, 512], f32, tag="ps_s", name=f"ps_s_{t}_{m}_{b}")
                    nc.tensor.matmul(ps_s[:], lhsh[:, 128 * m:128 * m + 128],
                                     rhsh[:, sl], start=True, stop=False)
                    nc.tensor.matmul(ps_s[:], lhsh[:, 128 * m:128 * m + 128],
                                     rhsl[:, sl], start=False, stop=False)
                    nc.tensor.matmul(ps_s[:], lhsl[:, 128 * m:128 * m + 128],
                                     rhsh[:, sl], start=False, stop=True)
                    nc.scalar.copy(s_sb[:, sl], ps_s[:])
                mx = sb_sm.tile([128, 8], f32, tag="mx", name=f"mx_{t}_{m}")
                mi = sb_sm.tile([128, 8], u32, tag="mi", name=f"mi_{t}_{m}")
                nc.vector.max(mx[:], s_sb[:])
                nc.vector.max_index(mi[:], mx[:], s_sb[:])
                adj = sb_sm.tile([128, 8], u32, tag="adj", name=f"adj_{t}_{m}")
                if hostfed:
                    nc.vector.tensor_scalar(adj[:], mi[:], N * t, None, op0=Alu.add)
                else:
                    nc.vector.tensor_scalar(adj[:], mi[:], 0xFFFFFF00, None,
                                            op0=Alu.bitwise_and)
                    nc.vector.tensor_tensor(out=adj[:], in0=adj[:], in1=mi[:],
                                            op=Alu.add)
                cand = sb_sm.tile([128, 8, 4], f32, tag="cand", name=f"cand_{t}_{m}")
                for r in range(8):
                    nc.gpsimd.indirect_dma_start(
                        out=cand[:, r, :], out_offset=None, in_=tabv,
                        in_offset=bass.IndirectOffsetOnAxis(ap=adj[:, r:r + 1], axis=0))
                rel = sb_sm.tile([128, 8, 2], f32, tag="rel", name=f"rel_{t}_{m}")
                nc.vector.tensor_tensor(
                    out=rel[:], in0=cand[:, :, 0:2],
                    in1=own4p[m][:, 0:2].unsqueeze(1).to_broadcast([128, 8, 2]),
                    op=Alu.subtract)
                rel2 = sb_sm.tile([128, 8, 2], f32, tag="rel2", name=f"rel2_{t}_{m}")
                nc.vector.tensor_tensor(out=rel2[:], in0=rel[:], in1=rel[:],
                                        op=Alu.mult)
                d2 = sb_sm.tile([128, 8], f32, tag="d2", name=f"d2_{t}_{m}")
                nc.vector.reduce_sum(d2[:], rel2[:], axis=mybir.AxisListType.X)
                d2n = sb_sm.tile([128, 8], f32, tag="d2n", name=f"d2n_{t}_{m}")
                nc.vector.tensor_scalar(d2n[:], d2[:], -1.0, None, op0=Alu.mult)
                srt = sb_sm.tile([128, 8], f32, tag="srt", name=f"srt_{t}_{m}")
                ordv = sb_sm.tile([128, 8], u32, tag="ordv", name=f"ordv_{t}_{m}")
                nc.vector.max(srt[:], d2n[:])
                nc.vector.max_index(ordv[:], srt[:], d2n[:])
                mask4 = sb_sm.tile([128, 4, 8], f32, tag="mask4", name=f"mask4_{t}_{m}")
                nc.vector.tensor_tensor(
                    out=mask4[:],
                    in0=ordv[:, 1:5].unsqueeze(2).to_broadcast([128, 4, 8]),
                    in1=iota8[:].unsqueeze(1).to_broadcast([128, 4, 8]),
                    op=Alu.is_equal)
                prod4 = sb_sm.tile([128, 4, 4, 8], f32, tag="prod4", name=f"prod4_{t}_{m}")
                nc.vector.tensor_tensor(
                    out=prod4[:],
                    in0=mask4[:].unsqueeze(2).to_broadcast([128, 4, 4, 8]),
                    in1=cand[:].rearrange("p s e -> p e s").unsqueeze(1)
                        .to_broadcast([128, 4, 4, 8]),
                    op=Alu.mult)
                selv = sb_sm.tile([128, 4, 4], f32, tag="selv", name=f"selv_{t}_{m}")
                nc.vector.reduce_sum(selv[:], prod4[:], axis=mybir.AxisListType.X)
                nc.vector.tensor_tensor(
                    out=fouts[m][:], in0=selv[:],
                    in1=own4p[m][:].unsqueeze(1).to_broadcast([128, 4, 4]),
                    op=Alu.subtract)

        # ---- phase A: all host-fed neighbor searches, densely packed ----
        nhost = min(NE + 1, nt)
        feat_store = []
        vrhs_store = []
        for t in range(nhost):
            rhs_hi_t = sb_big.tile([4, N], bf16, tag="rhs_hi", name=f"rhsh_{t}")
            rhs_lo_t = sb_big.tile([4, N], bf16, tag="rhs_lo", name=f"rhsl_{t}")
            nc.sync.dma_start(rhs_hi_t[:], rhs_hi_in[t])
            nc.sync.dma_start(rhs_lo_t[:], rhs_lo_in[t])
            lhsh_t = sb_sm.tile([4, RC], bf16, tag="lhsA", name=f"lhsh_{t}")
            lhsl_t = sb_sm.tile([4, RC], bf16, tag="lhsB", name=f"lhsl_{t}")
            nc.sync.dma_start(lhsh_t[:], lhs_hi_in[t])
            nc.sync.dma_start(lhsl_t[:], lhs_lo_in[t])
            vr_t = pers.tile([3, RC], f32r, tag=f"vrA{t}", name=f"vrA{t}")
            nc.sync.dma_start(vr_t[:], vrhs_in[t])
            vrhs_store.append(vr_t)
            o4_t = [sb_sm.tile([128, 4], f32, tag=f"own4A{m}", name=f"own4A_{t}_{m}")
                    for m in range(2)]
            for m in range(2):
                nc.sync.dma_start(o4_t[m][:], own4_in[t, m])
            f_t = [pers.tile([128, 4, 4], f32r, tag=f"featS{t}{m}", name=f"featS{t}{m}")
                   for m in range(2)]
            neighbor_block(t, lhsh_t, lhsl_t, rhs_hi_t, rhs_lo_t, o4_t,
                           table_view, True, f_t)
            feat_store.append(f_t)

        for t in range(nt):
            enc = t < NE
            hostfed = t <= NE
            widx = 0 if enc else 1

            if hostfed:
                pass
            else:
                # rhs from last step's all-gather (ccout): blocks of 2048 f32
                # [table 1024 | xyhi 512b | xylo 512b | sqhi 512b | sqlo 512b]
                rhs_hi = sb_big.tile([4, N], bf16, tag="rhs_hi", name=f"rhshD_{t}")
                rhs_lo = sb_big.tile([4, N], bf16, tag="rhs_lo", name=f"rhslD_{t}")
                cc_bf = ccout[:].bitcast(bf16)  # [8, 4096]
                for dst, off in ((rhs_hi, 2048), (rhs_lo, 2560)):
                    # xy rows -> partitions 0..1 ; sq rows -> partitions 2..3
                    nc.sync.dma_start(
                        dst[0:2, :].rearrange("p (c j) -> p c j", c=8),
                        cc_bf[:, off:off + 512].rearrange("c (p j) -> p c j", p=2))
                    nc.sync.dma_start(
                        dst[2:4, :].rearrange("p (c j) -> p c j", c=8),
                        cc_bf[:, off + 1024:off + 1536].rearrange("c (p j) -> p c j", p=2))

            if hostfed:
                feats = feat_store[t]
            else:
                feats = [sb_sm.tile([128, 4, 4], f32r, tag=f"featD{m}",
                                    name=f"featD_{t}_{m}") for m in range(2)]
                tabv_d = ccout[:].rearrange("c (r e) -> (c r) e", e=4)
                neighbor_block(t, lhs_hi, lhs_lo, rhs_hi, rhs_lo, own4,
                               tabv_d, False, feats)

            # featT -> xg rows 0..15
            ftp = ps_sm.tile([16, RC], f32r, tag="sm")
            for m in range(2):
                nc.tensor.transpose(ftp[:, 128 * m:128 * m + 128],
                                    feats[m][:].rearrange("p a b -> p (a b)"),
                                    ident_r[:])
            nc.scalar.copy(xg[0:16, :], ftp[:])
            gps = ps_sm.tile([POOL_OUT, RC], f32, tag="sm")
            nc.tensor.matmul(gps[:], wblk[:], xg[:], start=True, stop=True)
            nc.scalar.activation(gt_aug[0:32, :], gps[:], Act.Relu)

            # pool LSTM gates: [1024, RC] in 8 tiles; order i,f,g,o x 2
            pg = []
            for mt in range(8):
                sl = slice(128 * mt, 128 * mt + 128)
                pt = ps_med.tile([128, RC], f32, tag="pg")
                nc.tensor.matmul(pt[:], whhp[0][:, sl], hpT[0][:], start=True, stop=False)
                nc.tensor.matmul(pt[:], whhp[1][:, sl], hpT[1][:], start=False, stop=False)
                nc.tensor.matmul(pt[:], wihp[:, sl], gt_aug[:], start=False, stop=True)
                pg.append(pt)
            for ht in range(2):
                i_sb = sb_gate.tile([128, RC], f32, tag="i_sb")
                f_sb = sb_gate.tile([128, RC], f32, tag="f_sb")
                g_sb = sb_gate.tile([128, RC], f32, tag="g_sb")
                o_sb = sb_gate.tile([128, RC], f32, tag="o_sb")
                nc.scalar.activation(i_sb[:], pg[0 + ht][:], Act.Sigmoid)
                nc.scalar.activation(f_sb[:], pg[2 + ht][:], Act.Sigmoid)
                nc.scalar.activation(g_sb[:], pg[4 + ht][:], Act.Tanh)
                nc.scalar.activation(o_sb[:], pg[6 + ht][:], Act.Sigmoid)
                tmp = sb_gate.tile([128, RC], f32, tag="tmp")
                nc.vector.tensor_tensor(out=tmp[:], in0=i_sb[:], in1=g_sb[:], op=Alu.mult)
                nc.vector.tensor_tensor(out=cpT[ht][:], in0=f_sb[:], in1=cpT[ht][:], op=Alu.mult)
                nc.vector.tensor_tensor(out=cpT[ht][:], in0=cpT[ht][:], in1=tmp[:], op=Alu.add)
                th = sb_gate.tile([128, RC], f32, tag="th")
                nc.scalar.activation(th[:], cpT[ht][:], Act.Tanh)
                nc.vector.tensor_tensor(out=hpT[ht][:], in0=o_sb[:], in1=th[:], op=Alu.mult)

            # pooled -> xT rows 64..95 ; emb -> xT rows 0..61
            plp = ps_sm.tile([POOL_OUT, RC], f32, tag="sm")
            nc.tensor.matmul(plp[:], bh2p[:], ones_r[:], start=True, stop=False)
            nc.tensor.matmul(plp[:], wh2p[0][:], hpT[0][:], start=False, stop=False)
            nc.tensor.matmul(plp[:], wh2p[1][:], hpT[1][:], start=False, stop=True)
            nc.scalar.copy(xT[64:96, :], plp[:])
            ebp = ps_sm.tile([EMB - 2, RC], f32, tag="sm")
            nc.tensor.matmul(ebp[:], wie[:], (vrhs_store[t] if hostfed else vrhs)[:], start=True, stop=True)
            nc.scalar.activation(xT[0:62, :], ebp[:], Act.Relu)

            # main LSTM
            mg = []
            for mt in range(4):
                sl = slice(128 * mt, 128 * mt + 128)
                gtl = ps_med.tile([128, RC], f32, tag="pg")
                nc.tensor.matmul(gtl[:], whhm[widx][:, sl], hT[:], start=True, stop=False)
                nc.tensor.matmul(gtl[:], wiha[widx][:, sl], xT[:], start=False, stop=True)
                mg.append(gtl)
            i2 = sb_gate.tile([128, RC], f32, tag="i2")
            f2 = sb_gate.tile([128, RC], f32, tag="f2")
            g2 = sb_gate.tile([128, RC], f32, tag="g2")
            o2 = sb_gate.tile([128, RC], f32, tag="o2")
            nc.scalar.activation(i2[:], mg[0][:], Act.Sigmoid)
            nc.scalar.activation(f2[:], mg[1][:], Act.Sigmoid)
            nc.scalar.activation(g2[:], mg[2][:], Act.Tanh)
            nc.scalar.activation(o2[:], mg[3][:], Act.Sigmoid)
            tmp2 = sb_gate.tile([128, RC], f32, tag="tmp2")
            nc.vector.tensor_tensor(out=tmp2[:], in0=i2[:], in1=g2[:], op=Alu.mult)
            nc.vector.tensor_tensor(out=cT[:], in0=f2[:], in1=cT[:], op=Alu.mult)
            nc.vector.tensor_tensor(out=cT[:], in0=cT[:], in1=tmp2[:], op=Alu.add)
            th2 = sb_gate.tile([128, RC], f32, tag="th2")
            nc.scalar.activation(th2[:], cT[:], Act.Tanh)
            nc.vector.tensor_tensor(out=hT[:], in0=o2[:], in1=th2[:], op=Alu.mult)

            # normal = a*raw + b*sigmoid(raw) + c
            nrp = ps_sm.tile([5, RC], f32, tag="sm")
            nc.tensor.matmul(nrp[:], bhn[:], ones_r[:], start=True, stop=False)
            nc.tensor.matmul(nrp[:], whn[:], hT[:], start=False, stop=True)
            sgm = sb_sm.tile([5, RC], f32, tag="sgm")
            nc.scalar.activation(sgm[:], nrp[:], Act.Sigmoid)
            t1 = sb_sm.tile([5, RC], f32, tag="t1n")
            nc.vector.tensor_scalar(t1[:], nrp[:], scabc[:, 0:1], None, op0=Alu.mult)
            nrm = sb_sm.tile([5, RC], f32, tag="nrm")
            nc.vector.tensor_scalar(nrm[:], sgm[:], scabc[:, 1:2], scabc[:, 2:3],
                                    op0=Alu.mult, op1=Alu.add)
            nc.vector.tensor_tensor(out=nrm[:], in0=nrm[:], in1=t1[:], op=Alu.add)
            nc.sync.dma_start(out_nrm[t], nrm[:])

            if t >= NE:
                nc.vector.tensor_tensor(out=nxt[:], in0=prev1[:], in1=nrp[0:2, :],
                                        op=Alu.add)
                nc.sync.dma_start(out_pos[t - NE], nxt[:])
                if t < nt - 1:
                    velT = sb_sm.tile([2, RC], f32, tag="velT")
                    nc.vector.tensor_tensor(out=velT[:], in0=nxt[:], in1=prev1[:],
                                            op=Alu.subtract)
                    # next-step lhsT rows 0-1 = split(2*nxt)
                    l32 = sb_sm.tile([2, RC], f32, tag="l32")
                    nc.vector.tensor_scalar(l32[:], nxt[:], 2.0, None, op0=Alu.mult)
                    nc.vector.tensor_copy(lhs_hi[0:2, :], l32[:])
                    nc.vector.tensor_tensor(out=lhs_lo[0:2, :], in0=l32[:],
                                            in1=lhs_hi[0:2, :], op=Alu.subtract)
                    nc.vector.tensor_scalar(vrhs[0:2, :], velT[:], 4.0, None,
                                            op0=Alu.mult)
                    # payload pieces
                    sq32 = sb_sm.tile([2, RC], f32, tag="sq32")
                    nc.vector.tensor_tensor(out=sq32[:], in0=nxt[:], in1=nxt[:],
                                            op=Alu.mult)
                    xyhi = sb_sm.tile([2, RC], bf16, tag="xyhi")
                    xylo = sb_sm.tile([2, RC], bf16, tag="xylo")
                    sqhi = sb_sm.tile([2, RC], bf16, tag="sqhi")
                    sqlo = sb_sm.tile([2, RC], bf16, tag="sqlo")
                    nc.vector.tensor_copy(xyhi[:], nxt[:])
                    nc.vector.tensor_tensor(out=xylo[:], in0=nxt[:], in1=xyhi[:],
                                            op=Alu.subtract)
                    nc.vector.tensor_copy(sqhi[:], sq32[:])
                    nc.vector.tensor_tensor(out=sqlo[:], in0=sq32[:], in1=sqhi[:],
                                            op=Alu.subtract)
                    # own4 for next step via PE transposes
                    for m in range(2):
                        tp = ps_sm.tile([128, 2], f32, tag="sm")
                        nc.tensor.transpose(tp[:], nxt[:, 128 * m:128 * m + 128],
                                            ident_f[0:2, 0:2])
                        nc.scalar.copy(own4[m][:, 0:2], tp[:])
                        tv = ps_sm.tile([128, 2], f32, tag="sm")
                        nc.tensor.transpose(tv[:], velT[:, 128 * m:128 * m + 128],
                                            ident_f[0:2, 0:2])
                        nc.scalar.copy(own4[m][:, 2:4], tv[:])
                    # build payload bounce and all-gather
                    bounce = dram.tile([1, 2048], f32, tag="bounce")
                    ccout = dram.tile([8, 2048], f32, tag="ccout")
                    for m in range(2):
                        nc.sync.dma_start(
                            bounce[:, 512 * m:512 * m + 512]
                            .rearrange("o (p e) -> o p e", p=128).squeeze(0),
                            own4[m][:])
                    bb = bounce[:].bitcast(bf16)  # [1, 4096]
                    nc.sync.dma_start(bb[:, 2048:2560].rearrange("o (p j) -> (o p) j", p=2), xyhi[:])
                    nc.sync.dma_start(bb[:, 2560:3072].rearrange("o (p j) -> (o p) j", p=2), xylo[:])
                    nc.sync.dma_start(bb[:, 3072:3584].rearrange("o (p j) -> (o p) j", p=2), sqhi[:])
                    nc.sync.dma_start(bb[:, 3584:4096].rearrange("o (p j) -> (o p) j", p=2), sqlo[:])
                    nc.gpsimd.collective_compute(
                        "AllGather", Alu.bypass,
                        replica_groups=[list(range(NCORES))],
                        ins=[bounce.opt()], outs=[ccout.opt()])
                prev2, prev1, nxt = prev1, nxt, prev2

        ex.close()
    nc.compile()
    return nc


_CACHE = {}


def kernel(observed, goals, batch_split, n_predict,
           W_ie, b_ie, W_pe, b_pe,
           Wih_p, Whh_p, bih_p, bhh_p, W_h2p, b_h2p,
           Wih_e, Whh_e, bih_e, bhh_e,
           Wih_d, Whh_d, bih_d, bhh_d,
           W_hn, b_hn):
    import ml_dtypes

    _install_ntff_hook()
    from concourse.bass_utils import run_bass_kernel_spmd

    observed = np.asarray(observed, np.float32)
    nd = int(n_predict)
    nt = NE + nd

    if nd not in _CACHE:
        _CACHE[nd] = _build_module(nd)
    nc = _CACHE[nd]

    # ---- host-side input prep ----
    obs1 = observed[:-1]                    # [8, N, 2]
    obs2 = observed[1:]                     # [8, N, 2]
    # step t (t=0..7): (obs1[t], obs2[t]); step 8: (observed[-2], observed[-1])
    p_all = np.concatenate([obs2, observed[-1:None]], axis=0)       # [9, N, 2]
    v_all = np.concatenate([obs2 - obs1, (observed[-1] - observed[-2])[None]], axis=0)

    sq_all = p_all * p_all                                          # [9, N, 2]
    rhs = np.concatenate([p_all, sq_all], axis=2).transpose(0, 2, 1)  # [9, 4, N]
    rhs_hi, rhs_lo = _split_bf16(rhs.astype(np.float32))

    table = np.concatenate([p_all, v_all], axis=2).astype(np.float32)  # [9, N, 4]

    iota8 = np.broadcast_to(np.arange(8, dtype=np.uint32), (128, 8)).copy()
    scabc = np.array([[1, 0, 0], [1, 0, 0], [0, 0.2, 0.01], [0, 0.2, 0.01],
                      [0, 0.7, 0]], np.float32)
    ones_row = np.ones((1, RC), np.float32)
    zeros_rows = np.zeros((2, RC), np.float32)

    W_pe = np.asarray(W_pe, np.float32)
    wblk = np.zeros((17, POOL_OUT), np.float32)
    for k in range(4):
        wblk[4 * k:4 * k + 4, 8 * k:8 * k + 8] = W_pe
    wblk[16, :] = np.tile(np.asarray(b_pe, np.float32), 4)

    wihp = np.concatenate([np.asarray(Wih_p, np.float32),
                           (np.asarray(bih_p) + np.asarray(bhh_p)).astype(np.float32)[None]], axis=0)
    whhp = np.asarray(Whh_p, np.float32).reshape(2, 128, 4 * POOL_HID)
    wh2p = np.asarray(W_h2p, np.float32).reshape(2, 128, POOL_OUT)
    bh2p = np.asarray(b_h2p, np.float32)[None]
    wie = np.concatenate([np.asarray(W_ie, np.float32),
                          np.asarray(b_ie, np.float32)[None]], axis=0)  # [3, 62]
    wiha = np.stack([
        np.concatenate([np.asarray(Wih_e, np.float32),
                        (np.asarray(bih_e) + np.asarray(bhh_e)).astype(np.float32)[None]], axis=0),
        np.concatenate([np.asarray(Wih_d, np.float32),
                        (np.asarray(bih_d) + np.asarray(bhh_d)).astype(np.float32)[None]], axis=0)])
    whhm = np.stack([np.asarray(Whh_e, np.float32), np.asarray(Whh_d, np.float32)])
    whn = np.asarray(W_hn, np.float32)
    bhn = np.asarray(b_hn, np.float32)[None]

    in_maps = []
    for c in range(NCORES):
        sl = slice(RC * c, RC * c + RC)
        pm = p_all[:, sl]                       # [9, RC, 2]
        vm = v_all[:, sl]
        lhs = np.concatenate([2 * pm.transpose(0, 2, 1),
                              -np.ones((NE + 1, 2, RC), np.float32)], axis=1)  # [9,4,RC]
        lhs_hi, lhs_lo = _split_bf16(lhs.astype(np.float32))
        vrhs = np.concatenate([4 * vm.transpose(0, 2, 1),
                               np.ones((NE + 1, 1, RC), np.float32)], axis=1)
        own4 = table[:, sl].reshape(NE + 1, 2, 128, 4)
        initp = np.stack([observed[-2, sl].T, observed[-1, sl].T])  # [2, 2, RC]
        in_maps.append({
            "lhs_hi": lhs_hi, "lhs_lo": lhs_lo,
            "rhs_hi": rhs_hi, "rhs_lo": rhs_lo,
            "vrhs": vrhs.astype(np.float32), "own4": own4.astype(np.float32),
            "tab": table.reshape(-1, 4), "initp": initp.astype(np.float32),
            "iota8": iota8, "scabc": scabc, "onesr": ones_row, "zerosr": zeros_rows,
            "wblk": wblk, "wihp": wihp, "whhp": whhp, "wh2p": wh2p, "bh2p": bh2p,
            "wie": wie, "wiha": wiha, "whhm": whhm, "whn": whn, "bhn": bhn,
        })

    kernel.last_in_maps = in_maps
    res = run_bass_kernel_spmd(nc, in_maps=in_maps, core_ids=list(range(NCORES)))
    kernel.last_results = res

    nrm = np.stack([r["out_nrm"] for r in res.results])   # [8c, nt, 5, RC]
    dpos = np.stack([r["out_pos"] for r in res.results])  # [8c, nd, 2, RC]
    normals = nrm.transpose(1, 0, 3, 2).reshape(nt, N, 5)
    dec_pos = dpos.transpose(1, 0, 3, 2).reshape(nd, N, 2)
    enc_pos = observed[1:] + normals[:NE, :, 0:2]
    positions = np.concatenate([enc_pos, dec_pos], axis=0)
    return normals.astype(np.float32), positions.astype(np.float32)


# revision 16
# speedup vs baseline: 1.0961x; 1.0204x over previous
"""Trainium2 Bass kernel for nn_DPoolLSTM (social-pooling LSTM trajectory model).

Sharding: 8 cores x 256 agents (data parallel over agent rows).
Per step: neighbor top-4 search over all 2048 agents (bf16-split score matmul,
DVE max8/max_index, indirect-DMA gather, exact fp32 recheck), pool-LSTM +
main-LSTM in transposed layout (fp32r matmuls). Decode steps exchange
predicted positions across cores with an AllGather collective.
"""

import contextlib
import ctypes
import sys
import types

import numpy as np

N = 2048
RC = 256          # agents per core
NCORES = 8
T_OBS = 9
NE = 8            # encoder steps
POOL_HID = 256
POOL_OUT = 32
EMB = 64
HID = 128
LSTM_IN = 96

_SO_PATH = "/opt/axon/libaxon_pjrt.so"


def _install_ntff_hook():
    """Provide antenv.axon_hooks so run_bass_kernel_spmd(trace=True) works."""
    if "antenv.axon_hooks" in sys.modules:
        return
    state = {"hook": None}

    def set_hook(h):
        state["hook"] = h

    def get_hook():
        return state["hook"]

    mod = types.ModuleType("antenv.axon_hooks")
    mod.set_axon_ntff_profile_hook = set_hook
    mod.get_axon_ntff_profile_hook = get_hook
    sys.modules["antenv.axon_hooks"] = mod

    try:
        lib = ctypes.CDLL(_SO_PATH)
    except OSError:
        return
    if not hasattr(lib, "axon_start_nrt_profile"):
        return
    lib.axon_start_nrt_profile.argtypes = [ctypes.POINTER(ctypes.c_int64), ctypes.c_size_t]
    lib.axon_start_nrt_profile.restype = ctypes.c_int64
    lib.axon_stop_nrt_profile.argtypes = [ctypes.c_char_p]
    lib.axon_stop_nrt_profile.restype = ctypes.c_int64

    @contextlib.contextmanager
    def _hook_cm(output_dir, device_ids):
        import jax

        jax.devices()
        if device_ids:
            ids = (ctypes.c_int64 * len(device_ids))(*device_ids)
            rc = lib.axon_start_nrt_profile(ids, len(device_ids))
        else:
            rc = lib.axon_start_nrt_profile(None, 0)
        if rc != 0:
            raise RuntimeError(f"axon_start_nrt_profile rc={rc}")
        try:
            yield
        finally:
            n = lib.axon_stop_nrt_profile(str(output_dir).encode())
            print(f"ntff profile: {n} file(s) -> {output_dir}", file=sys.stderr)

    set_hook(_hook_cm)


def _split_bf16(x):
    import ml_dtypes

    hi = x.astype(ml_dtypes.bfloat16)
    lo = (x - hi.astype(np.float32)).astype(ml_dtypes.bfloat16)
    return hi, lo


def _build_module(nd):
    import concourse.bass as bass
    import concourse.tile as tile
    from concourse import bacc, mybir
    from concourse.masks import make_identity

    f32 = mybir.dt.float32
    f32r = mybir.dt.float32r
    bf16 = mybir.dt.bfloat16
    u32 = mybir.dt.uint32
    Alu = mybir.AluOpType
    Act = mybir.ActivationFunctionType

    nt = NE + nd  # total steps
    nc = bacc.Bacc("TRN2", target_bir_lowering=False, num_devices=NCORES)

    # ---- external inputs ----
    def din(name, shape, dtype=f32):
        return nc.dram_tensor(name, shape, dtype, kind="ExternalInput")

    lhs_hi_in = din("lhs_hi", [NE + 1, 4, RC], bf16)
    lhs_lo_in = din("lhs_lo", [NE + 1, 4, RC], bf16)
    rhs_hi_in = din("rhs_hi", [NE + 1, 4, N], bf16)
    rhs_lo_in = din("rhs_lo", [NE + 1, 4, N], bf16)
    vrhs_in = din("vrhs", [NE + 1, 3, RC], f32r)
    own4_in = din("own4", [NE + 1, 2, 128, 4])
    table_in = din("tab", [(NE + 1) * N, 4])
    initp_in = din("initp", [2, 2, RC])          # [prev2T, prev1T]
    iota8_in = din("iota8", [128, 8], u32)
    scabc_in = din("scabc", [5, 3])
    ones_in = din("onesr", [1, RC], f32r)
    zeros_in = din("zerosr", [2, RC], f32r)
    wblk_in = din("wblk", [17, POOL_OUT], f32r)
    wihp_in = din("wihp", [33, 4 * POOL_HID], f32r)
    whhp_in = din("whhp", [2, 128, 4 * POOL_HID], f32r)
    wh2p_in = din("wh2p", [2, 128, POOL_OUT], f32r)
    bh2p_in = din("bh2p", [1, POOL_OUT], f32r)
    wie_in = din("wie", [3, EMB - 2], f32r)
    wiha_in = din("wiha", [2, LSTM_IN + 1, 4 * HID], f32r)
    whhm_in = din("whhm", [2, 128, 4 * HID], f32r)
    whn_in = din("whn", [128, 5], f32r)
    bhn_in = din("bhn", [1, 5], f32r)

    out_nrm = nc.dram_tensor("out_nrm", [nt, 5, RC], f32, kind="ExternalOutput")
    out_pos = nc.dram_tensor("out_pos", [max(nd, 1), 2, RC], f32, kind="ExternalOutput")

    with tile.TileContext(nc) as tc:
        ex = contextlib.ExitStack()
        P = ex.enter_context  # pools live until module end

        pers = P(tc.tile_pool(name="pers", bufs=1))
        dram = P(tc.tile_pool(name="dram", bufs=2, space="DRAM"))
        ps_big = P(tc.tile_pool(name="ps_big", bufs=3, space="PSUM"))
        ps_med = P(tc.tile_pool(name="ps_med", bufs=3, space="PSUM"))
        ps_sm = P(tc.tile_pool(name="ps_sm", bufs=2, space="PSUM"))
        sb_big = P(tc.tile_pool(name="sb_big", bufs=3))
        sb_sm = P(tc.tile_pool(name="sb_sm", bufs=4))
        sb_gate = P(tc.tile_pool(name="sb_gate", bufs=4))

        # ---- persistent tiles ----
        ident_r = pers.tile([128, 128], f32r, tag="ident_r")
        ident_f = pers.tile([128, 128], f32, tag="ident_f")
        make_identity(nc, ident_f[:])
        nc.vector.tensor_copy(ident_r[:], ident_f[:])
        iota8 = pers.tile([128, 8], u32, tag="iota8")
        scabc = pers.tile([5, 3], f32, tag="scabc")
        ones_r = pers.tile([1, RC], f32r, tag="ones_r")
        nc.sync.dma_start(iota8[:], iota8_in[:])
        nc.sync.dma_start(scabc[:], scabc_in[:])
        nc.sync.dma_start(ones_r[:], ones_in[:])

        lhs_hi = pers.tile([4, RC], bf16, tag="lhs_hi")
        lhs_lo = pers.tile([4, RC], bf16, tag="lhs_lo")
        vrhs = pers.tile([3, RC], f32r, tag="vrhs")
        nc.sync.dma_start(lhs_hi[:], lhs_hi_in[NE])
        nc.sync.dma_start(lhs_lo[:], lhs_lo_in[NE])
        nc.sync.dma_start(vrhs[:], vrhs_in[NE])
        own4 = [pers.tile([128, 4], f32, tag=f"own4_{m}", name=f"own4_{m}") for m in range(2)]

        xg = pers.tile([17, RC], f32r, tag="xg")
        nc.sync.dma_start(xg[16:17, :], ones_in[:])
        gt_aug = pers.tile([33, RC], f32r, tag="gt_aug")
        nc.sync.dma_start(gt_aug[32:33, :], ones_in[:])
        xT = pers.tile([LSTM_IN + 1, RC], f32r, tag="xT")
        nc.sync.dma_start(xT[62:64, :], zeros_in[:])
        nc.sync.dma_start(xT[96:97, :], ones_in[:])

        wblk = pers.tile([17, POOL_OUT], f32r, tag="wblk")
        wihp = pers.tile([33, 4 * POOL_HID], f32r, tag="wihp")
        whhp = [pers.tile([128, 4 * POOL_HID], f32r, tag=f"whhp{k}", name=f"whhp{k}") for k in range(2)]
        wh2p = [pers.tile([128, POOL_OUT], f32r, tag=f"wh2p{k}", name=f"wh2p{k}") for k in range(2)]
        bh2p = pers.tile([1, POOL_OUT], f32r, tag="bh2p")
        wie = pers.tile([3, EMB - 2], f32r, tag="wie")
        wiha = [pers.tile([LSTM_IN + 1, 4 * HID], f32r, tag=f"wiha{k}", name=f"wiha{k}") for k in range(2)]
        whhm = [pers.tile([128, 4 * HID], f32r, tag=f"whhm{k}", name=f"whhm{k}") for k in range(2)]
        whn = pers.tile([128, 5], f32r, tag="whn")
        bhn = pers.tile([1, 5], f32r, tag="bhn")
        nc.sync.dma_start(wblk[:], wblk_in[:])
        nc.sync.dma_start(wihp[:], wihp_in[:])
        for k in range(2):
            nc.sync.dma_start(whhp[k][:], whhp_in[k])
            nc.sync.dma_start(wh2p[k][:], wh2p_in[k])
            nc.sync.dma_start(wiha[k][:], wiha_in[k])
            nc.sync.dma_start(whhm[k][:], whhm_in[k])
        nc.sync.dma_start(bh2p[:], bh2p_in[:])
        nc.sync.dma_start(wie[:], wie_in[:])
        nc.sync.dma_start(whn[:], whn_in[:])
        nc.sync.dma_start(bhn[:], bhn_in[:])

        hpT = [pers.tile([128, RC], f32r, tag=f"hpT{k}", name=f"hpT{k}") for k in range(2)]
        cpT = [pers.tile([128, RC], f32, tag=f"cpT{k}", name=f"cpT{k}") for k in range(2)]
        hT = pers.tile([128, RC], f32r, tag="hT")
        cT = pers.tile([128, RC], f32, tag="cT")
        for k in range(2):
            nc.vector.memset(cpT[k][:], 0.0)
            nc.vector.tensor_copy(hpT[k][:], cpT[k][:])
        nc.vector.memset(cT[:], 0.0)
        nc.vector.tensor_copy(hT[:], cT[:])

        posT = [pers.tile([2, RC], f32, tag=f"posT{k}", name=f"posT{k}") for k in range(3)]
        nc.sync.dma_start(posT[0][:], initp_in[0])  # prev2T
        nc.sync.dma_start(posT[1][:], initp_in[1])  # prev1T
        prev2, prev1, nxt = posT[0], posT[1], posT[2]

        table_view = table_in[:]  # [(NE+1)*N, 4]

        def neighbor_block(t, lhsh, lhsl, rhsh, rhsl, own4p, tabv, hostfed, fouts):
            for m in range(2):
                s_sb = sb_big.tile([128, N], f32, tag="s_sb", name=f"s_sb_{t}_{m}")
                for b in range(4):
                    sl = slice(512 * b, 512 * b + 512)
                    ps_s = ps_big.tile([128, 512], f32, tag="ps_s", name=f"ps_s_{t}_{m}_{b}")
                    nc.tensor.matmul(ps_s[:], lhsh[:, 128 * m:128 * m + 128],
                                     rhsh[:, sl], start=True, stop=False)
                    nc.tensor.matmul(ps_s[:], lhsh[:, 128 * m:128 * m + 128],
                                     rhsl[:, sl], start=False, stop=False)
                    nc.tensor.matmul(ps_s[:], lhsl[:, 128 * m:128 * m + 128],
                                     rhsh[:, sl], start=False, stop=True)
                    nc.scalar.copy(s_sb[:, sl], ps_s[:])
                mx = sb_sm.tile([128, 8], f32, tag="mx", name=f"mx_{t}_{m}")
                mi = sb_sm.tile([128, 8], u32, tag="mi", name=f"mi_{t}_{m}")
                nc.vector.max(mx[:], s_sb[:])
                nc.vector.max_index(mi[:], mx[:], s_sb[:])
                adj = sb_sm.tile([128, 8], u32, tag="adj", name=f"adj_{t}_{m}")
                if hostfed:
                    nc.vector.tensor_scalar(adj[:], mi[:], N * t, None, op0=Alu.add)
                else:
                    nc.vector.tensor_scalar(adj[:], mi[:], 0xFFFFFF00, None,
                                            op0=Alu.bitwise_and)
                    nc.vector.tensor_tensor(out=adj[:], in0=adj[:], in1=mi[:],
                                            op=Alu.add)
                cand = sb_sm.tile([128, 8, 4], f32, tag="cand", name=f"cand_{t}_{m}")
                for r in range(8):
                    nc.gpsimd.indirect_dma_start(
                        out=cand[:, r, :], out_offset=None, in_=tabv,
                        in_offset=bass.IndirectOffsetOnAxis(ap=adj[:, r:r + 1], axis=0))
                rel = sb_sm.tile([128, 8, 2], f32, tag="rel", name=f"rel_{t}_{m}")
                nc.vector.tensor_tensor(
                    out=rel[:], in0=cand[:, :, 0:2],
                    in1=own4p[m][:, 0:2].unsqueeze(1).to_broadcast([128, 8, 2]),
                    op=Alu.subtract)
                rel2 = sb_sm.tile([128, 8, 2], f32, tag="rel2", name=f"rel2_{t}_{m}")
                nc.vector.tensor_tensor(out=rel2[:], in0=rel[:], in1=rel[:],
                                        op=Alu.mult)
                d2 = sb_sm.tile([128, 8], f32, tag="d2", name=f"d2_{t}_{m}")
                nc.vector.reduce_sum(d2[:], rel2[:], axis=mybir.AxisListType.X)
                d2n = sb_sm.tile([128, 8], f32, tag="d2n", name=f"d2n_{t}_{m}")
                nc.vector.tensor_scalar(d2n[:], d2[:], -1.0, None, op0=Alu.mult)
                srt = sb_sm.tile([128, 8], f32, tag="srt", name=f"srt_{t}_{m}")
                ordv = sb_sm.tile([128, 8], u32, tag="ordv", name=f"ordv_{t}_{m}")
                nc.vector.max(srt[:], d2n[:])
                nc.vector.max_index(ordv[:], srt[:], d2n[:])
                mask4 = sb_sm.tile([128, 4, 8], f32, tag="mask4", name=f"mask4_{t}_{m}")
                nc.vector.tensor_tensor(
                    out=mask4[:],
                    in0=ordv[:, 1:5].unsqueeze(2).to_broadcast([128, 4, 8]),
                    in1=iota8[:].unsqueeze(1).to_broadcast([128, 4, 8]),
                    op=Alu.is_equal)
                prod4 = sb_sm.tile([128, 4, 4, 8], f32, tag="prod4", name=f"prod4_{t}_{m}")
                nc.vector.tensor_tensor(
                    out=prod4[:],
                    in0=mask4[:].unsqueeze(2).to_broadcast([128, 4, 4, 8]),
                    in1=cand[:].rearrange("p s e -> p e s").unsqueeze(1)
                        .to_broadcast([128, 4, 4, 8]),
                    op=Alu.mult)
                selv = sb_sm.tile([128, 4, 4], f32, tag="selv", name=f"selv_{t}_{m}")
                nc.vector.reduce_sum(selv[:], prod4[:], axis=mybir.AxisListType.X)
                nc.vector.tensor_tensor(
                    out=fouts[m][:], in0=selv[:],
                    in1=own4p[m][:].unsqueeze(1).to_broadcast([128, 4, 4]),
                    op=Alu.subtract)

        # ---- phase A: all host-fed neighbor searches, densely packed ----
        nhost = min(NE + 1, nt)
        feat_store = []
        vrhs_store = []
        for t in range(nhost):
            rhs_hi_t = sb_big.tile([4, N], bf16, tag="rhs_hi", name=f"rhsh_{t}")
            rhs_lo_t = sb_big.tile([4, N], bf16, tag="rhs_lo", name=f"rhsl_{t}")
            nc.sync.dma_start(rhs_hi_t[:], rhs_hi_in[t])
            nc.sync.dma_start(rhs_lo_t[:], rhs_lo_in[t])
            lhsh_t = sb_sm.tile([4, RC], bf16, tag="lhsA", name=f"lhsh_{t}")
            lhsl_t = sb_sm.tile([4, RC], bf16, tag="lhsB", name=f"lhsl_{t}")
            nc.sync.dma_start(lhsh_t[:], lhs_hi_in[t])
            nc.sync.dma_start(lhsl_t[:], lhs_lo_in[t])
            vr_t = pers.tile([3, RC], f32r, tag=f"vrA{t}", name=f"vrA{t}")
            nc.sync.dma_start(vr_t[:], vrhs_in[t])
            vrhs_store.append(vr_t)
            o4_t = [sb_sm.tile([128, 4], f32, tag=f"own4A{m}", name=f"own4A_{t}_{m}")
                    for m in range(2)]
            for m in range(2):
                nc.sync.dma_start(o4_t[m][:], own4_in[t, m])
            f_t = [pers.tile([128, 4, 4], f32r, tag=f"featS{t}{m}", name=f"featS{t}{m}")
                   for m in range(2)]
            neighbor_block(t, lhsh_t, lhsl_t, rhs_hi_t, rhs_lo_t, o4_t,
                           table_view, True, f_t)
            feat_store.append(f_t)

        for t in range(nt):
            enc = t < NE
            hostfed = t <= NE
            widx = 0 if enc else 1

            if hostfed:
                pass
            else:
                # rhs from last step's all-gather (ccout): blocks of 2048 f32
                # [table 1024 | xyhi 512b | xylo 512b | sqhi 512b | sqlo 512b]
                rhs_hi = sb_big.tile([4, N], bf16, tag="rhs_hi", name=f"rhshD_{t}")
                rhs_lo = sb_big.tile([4, N], bf16, tag="rhs_lo", name=f"rhslD_{t}")
                cc_bf = ccout[:].bitcast(bf16)  # [8, 4096]
                for dst, off in ((rhs_hi, 2048), (rhs_lo, 2560)):
                    # xy rows -> partitions 0..1 ; sq rows -> partitions 2..3
                    nc.sync.dma_start(
                        dst[0:2, :].rearrange("p (c j) -> p c j", c=8),
                        cc_bf[:, off:off + 512].rearrange("c (p j) -> p c j", p=2))
                    nc.sync.dma_start(
                        dst[2:4, :].rearrange("p (c j) -> p c j", c=8),
                        cc_bf[:, off + 1024:off + 1536].rearrange("c (p j) -> p c j", p=2))

            if hostfed:
                feats = feat_store[t]
            else:
                feats = [sb_sm.tile([128, 4, 4], f32r, tag=f"featD{m}",
                                    name=f"featD_{t}_{m}") for m in range(2)]
                tabv_d = ccout[:].rearrange("c (r e) -> (c r) e", e=4)
                neighbor_block(t, lhs_hi, lhs_lo, rhs_hi, rhs_lo, own4,
                               tabv_d, False, feats)

            # featT -> xg rows 0..15
            ftp = ps_sm.tile([16, RC], f32r, tag="sm")
            for m in range(2):
                nc.tensor.transpose(ftp[:, 128 * m:128 * m + 128],
                                    feats[m][:].rearrange("p a b -> p (a b)"),
                                    ident_r[:])
            nc.scalar.copy(xg[0:16, :], ftp[:])
            gps = ps_sm.tile([POOL_OUT, RC], f32, tag="sm")
            nc.tensor.matmul(gps[:], wblk[:], xg[:], start=True, stop=True)
            nc.scalar.activation(gt_aug[0:32, :], gps[:], Act.Relu)

            # pool LSTM gates: [1024, RC] in 8 tiles; order i,f,g,o x 2
            pg = []
            for mt in range(8):
                sl = slice(128 * mt, 128 * mt + 128)
                pt = ps_med.tile([128, RC], f32, tag="pg")
                nc.tensor.matmul(pt[:], whhp[0][:, sl], hpT[0][:], start=True, stop=False)
                nc.tensor.matmul(pt[:], whhp[1][:, sl], hpT[1][:], start=False, stop=False)
                nc.tensor.matmul(pt[:], wihp[:, sl], gt_aug[:], start=False, stop=True)
                pg.append(pt)
            for ht in range(2):
                i_sb = sb_gate.tile([128, RC], f32, tag="i_sb")
                f_sb = sb_gate.tile([128, RC], f32, tag="f_sb")
                g_sb = sb_gate.tile([128, RC], f32, tag="g_sb")
                o_sb = sb_gate.tile([128, RC], f32, tag="o_sb")
                nc.scalar.activation(i_sb[:], pg[0 + ht][:], Act.Sigmoid)
                nc.scalar.activation(f_sb[:], pg[2 + ht][:], Act.Sigmoid)
                nc.scalar.activation(g_sb[:], pg[4 + ht][:], Act.Tanh)
                nc.scalar.activation(o_sb[:], pg[6 + ht][:], Act.Sigmoid)
                tmp = sb_gate.tile([128, RC], f32, tag="tmp")
                nc.vector.tensor_tensor(out=tmp[:], in0=i_sb[:], in1=g_sb[:], op=Alu.mult)
                nc.vector.tensor_tensor(out=cpT[ht][:], in0=f_sb[:], in1=cpT[ht][:], op=Alu.mult)
                nc.vector.tensor_tensor(out=cpT[ht][:], in0=cpT[ht][:], in1=tmp[:], op=Alu.add)
                th = sb_gate.tile([128, RC], f32, tag="th")
                nc.scalar.activation(th[:], cpT[ht][:], Act.Tanh)
                nc.vector.tensor_tensor(out=hpT[ht][:], in0=o_sb[:], in1=th[:], op=Alu.mult)

            # pooled -> xT rows 64..95 ; emb -> xT rows 0..61
            plp = ps_sm.tile([POOL_OUT, RC], f32, tag="sm")
            nc.tensor.matmul(plp[:], bh2p[:], ones_r[:], start=True, stop=False)
            nc.tensor.matmul(plp[:], wh2p[0][:], hpT[0][:], start=False, stop=False)
            nc.tensor.matmul(plp[:], wh2p[1][:], hpT[1][:], start=False, stop=True)
            nc.scalar.copy(xT[64:96, :], plp[:])
            ebp = ps_sm.tile([EMB - 2, RC], f32, tag="sm")
            nc.tensor.matmul(ebp[:], wie[:], (vrhs_store[t] if hostfed else vrhs)[:], start=True, stop=True)
            nc.scalar.activation(xT[0:62, :], ebp[:], Act.Relu)

            # main LSTM
            mg = []
            for mt in range(4):
                sl = slice(128 * mt, 128 * mt + 128)
                gtl = ps_med.tile([128, RC], f32, tag="pg")
                nc.tensor.matmul(gtl[:], whhm[widx][:, sl], hT[:], start=True, stop=False)
                nc.tensor.matmul(gtl[:], wiha[widx][:, sl], xT[:], start=False, stop=True)
                mg.append(gtl)
            i2 = sb_gate.tile([128, RC], f32, tag="i2")
            f2 = sb_gate.tile([128, RC], f32, tag="f2")
            g2 = sb_gate.tile([128, RC], f32, tag="g2")
            o2 = sb_gate.tile([128, RC], f32, tag="o2")
            nc.scalar.activation(i2[:], mg[0][:], Act.Sigmoid)
            nc.scalar.activation(f2[:], mg[1][:], Act.Sigmoid)
            nc.scalar.activation(g2[:], mg[2][:], Act.Tanh)
            nc.scalar.activation(o2[:], mg[3][:], Act.Sigmoid)
            tmp2 = sb_gate.tile([128, RC], f32, tag="tmp2")
            nc.vector.tensor_tensor(out=tmp2[:], in0=i2[:], in1=g2[:], op=Alu.mult)
            nc.vector.tensor_tensor(out=cT[:], in0=f2[:], in1=cT[:], op=Alu.mult)
            nc.vector.tensor_tensor(out=cT[:], in0=cT[:], in1=tmp2[:], op=Alu.add)
            th2 = sb_gate.tile([128, RC], f32, tag="th2")
            nc.scalar.activation(th2[:], cT[:], Act.Tanh)
            nc.vector.tensor_tensor(out=hT[:], in0=o2[:], in1=th2[:], op=Alu.mult)

            # normal = a*raw + b*sigmoid(raw) + c
            nrp = ps_sm.tile([5, RC], f32, tag="sm")
            nc.tensor.matmul(nrp[:], bhn[:], ones_r[:], start=True, stop=False)
            nc.tensor.matmul(nrp[:], whn[:], hT[:], start=False, stop=True)
            sgm = sb_sm.tile([5, RC], f32, tag="sgm")
            nc.scalar.activation(sgm[:], nrp[:], Act.Sigmoid)
            t1 = sb_sm.tile([5, RC], f32, tag="t1n")
            nc.vector.tensor_scalar(t1[:], nrp[:], scabc[:, 0:1], None, op0=Alu.mult)
            nrm = sb_sm.tile([5, RC], f32, tag="nrm")
            nc.vector.tensor_scalar(nrm[:], sgm[:], scabc[:, 1:2], scabc[:, 2:3],
                                    op0=Alu.mult, op1=Alu.add)
            nc.vector.tensor_tensor(out=nrm[:], in0=nrm[:], in1=t1[:], op=Alu.add)
            nc.sync.dma_start(out_nrm[t], nrm[:])

            if t >= NE:
                nc.vector.tensor_tensor(out=nxt[:], in0=prev1[:], in1=nrp[0:2, :],
                                        op=Alu.add)
                nc.sync.dma_start(out_pos[t - NE], nxt[:])
                if t < nt - 1:
                    velT = sb_sm.tile([2, RC], f32, tag="velT")
                    nc.vector.tensor_tensor(out=velT[:], in0=nxt[:], in1=prev1[:],
                                            op=Alu.subtract)
                    # next-step lhsT rows 0-1 = split(2*nxt)
                    l32 = sb_sm.tile([2, RC], f32, tag="l32")
                    nc.vector.tensor_scalar(l32[:], nxt[:], 2.0, None, op0=Alu.mult)
                    nc.vector.tensor_copy(lhs_hi[0:2, :], l32[:])
                    nc.vector.tensor_tensor(out=lhs_lo[0:2, :], in0=l32[:],
                                            in1=lhs_hi[0:2, :], op=Alu.subtract)
                    nc.vector.tensor_scalar(vrhs[0:2, :], velT[:], 4.0, None,
                                            op0=Alu.mult)
                    # payload pieces
                    sq32 = sb_sm.tile([2, RC], f32, tag="sq32")
                    nc.vector.tensor_tensor(out=sq32[:], in0=nxt[:], in1=nxt[:],
                                            op=Alu.mult)
                    xyhi = sb_sm.tile([2, RC], bf16, tag="xyhi")
                    xylo = sb_sm.tile([2, RC], bf16, tag="xylo")
                    sqhi = sb_sm.tile([2, RC], bf16, tag="sqhi")
                    sqlo = sb_sm.tile([2, RC], bf16, tag="sqlo")
                    nc.vector.tensor_copy(xyhi[:], nxt[:])
                    nc.vector.tensor_tensor(out=xylo[:], in0=nxt[:], in1=xyhi[:],
                                            op=Alu.subtract)
                    nc.vector.tensor_copy(sqhi[:], sq32[:])
                    nc.vector.tensor_tensor(out=sqlo[:], in0=sq32[:], in1=sqhi[:],
                                            op=Alu.subtract)
                    # payload bf16 blocks first (ready early; Sync queue is in-order)
                    bounce = dram.tile([1, 2048], f32, tag="bounce")
                    ccout = dram.tile([8, 2048], f32, tag="ccout")
                    bb = bounce[:].bitcast(bf16)  # [1, 4096]
                    nc.sync.dma_start(bb[:, 2048:2560].rearrange("o (p j) -> (o p) j", p=2), xyhi[:])
                    nc.sync.dma_start(bb[:, 2560:3072].rearrange("o (p j) -> (o p) j", p=2), xylo[:])
                    nc.sync.dma_start(bb[:, 3072:3584].rearrange("o (p j) -> (o p) j", p=2), sqhi[:])
                    nc.sync.dma_start(bb[:, 3584:4096].rearrange("o (p j) -> (o p) j", p=2), sqlo[:])
                    for m in range(2):
                        tp = ps_sm.tile([128, 2], f32, tag="sm")
                        nc.tensor.transpose(tp[:], nxt[:, 128 * m:128 * m + 128],
                                            ident_f[0:2, 0:2])
                        nc.scalar.copy(own4[m][:, 0:2], tp[:])
                        tv = ps_sm.tile([128, 2], f32, tag="sm")
                        nc.tensor.transpose(tv[:], velT[:, 128 * m:128 * m + 128],
                                            ident_f[0:2, 0:2])
                        nc.scalar.copy(own4[m][:, 2:4], tv[:])
                    for m in range(2):
                        nc.sync.dma_start(
                            bounce[:, 512 * m:512 * m + 512]
                            .rearrange("o (p e) -> o p e", p=128).squeeze(0),
                            own4[m][:])
                    nc.gpsimd.collective_compute(
                        "AllGather", Alu.bypass,
                        replica_groups=[list(range(NCORES))],
                        ins=[bounce.opt()], outs=[ccout.opt()])
                prev2, prev1, nxt = prev1, nxt, prev2

        ex.close()
    nc.compile()
    return nc


_CACHE = {}


def kernel(observed, goals, batch_split, n_predict,
           W_ie, b_ie, W_pe, b_pe,
           Wih_p, Whh_p, bih_p, bhh_p, W_h2p, b_h2p,
           Wih_e, Whh_e, bih_e, bhh_e,
           Wih_d, Whh_d, bih_d, bhh_d,
           W_hn, b_hn):
    import ml_dtypes

    _install_ntff_hook()
    from concourse.bass_utils import run_bass_kernel_spmd

    observed = np.asarray(observed, np.float32)
    nd = int(n_predict)
    nt = NE + nd

    if nd not in _CACHE:
        _CACHE[nd] = _build_module(nd)
    nc = _CACHE[nd]

    # ---- host-side input prep ----
    obs1 = observed[:-1]                    # [8, N, 2]
    obs2 = observed[1:]                     # [8, N, 2]
    # step t (t=0..7): (obs1[t], obs2[t]); step 8: (observed[-2], observed[-1])
    p_all = np.concatenate([obs2, observed[-1:None]], axis=0)       # [9, N, 2]
    v_all = np.concatenate([obs2 - obs1, (observed[-1] - observed[-2])[None]], axis=0)

    sq_all = p_all * p_all                                          # [9, N, 2]
    rhs = np.concatenate([p_all, sq_all], axis=2).transpose(0, 2, 1)  # [9, 4, N]
    rhs_hi, rhs_lo = _split_bf16(rhs.astype(np.float32))

    table = np.concatenate([p_all, v_all], axis=2).astype(np.float32)  # [9, N, 4]

    iota8 = np.broadcast_to(np.arange(8, dtype=np.uint32), (128, 8)).copy()
    scabc = np.array([[1, 0, 0], [1, 0, 0], [0, 0.2, 0.01], [0, 0.2, 0.01],
                      [0, 0.7, 0]], np.float32)
    ones_row = np.ones((1, RC), np.float32)
    zeros_rows = np.zeros((2, RC), np.float32)

    W_pe = np.asarray(W_pe, np.float32)
    wblk = np.zeros((17, POOL_OUT), np.float32)
    for k in range(4):
        wblk[4 * k:4 * k + 4, 8 * k:8 * k + 8] = W_pe
    wblk[16, :] = np.tile(np.asarray(b_pe, np.float32), 4)

    wihp = np.concatenate([np.asarray(Wih_p, np.float32),
                           (np.asarray(bih_p) + np.asarray(bhh_p)).astype(np.float32)[None]], axis=0)
    whhp = np.asarray(Whh_p, np.float32).reshape(2, 128, 4 * POOL_HID)
    wh2p = np.asarray(W_h2p, np.float32).reshape(2, 128, POOL_OUT)
    bh2p = np.asarray(b_h2p, np.float32)[None]
    wie = np.concatenate([np.asarray(W_ie, np.float32),
                          np.asarray(b_ie, np.float32)[None]], axis=0)  # [3, 62]
    wiha = np.stack([
        np.concatenate([np.asarray(Wih_e, np.float32),
                        (np.asarray(bih_e) + np.asarray(bhh_e)).astype(np.float32)[None]], axis=0),
        np.concatenate([np.asarray(Wih_d, np.float32),
                        (np.asarray(bih_d) + np.asarray(bhh_d)).astype(np.float32)[None]], axis=0)])
    whhm = np.stack([np.asarray(Whh_e, np.float32), np.asarray(Whh_d, np.float32)])
    whn = np.asarray(W_hn, np.float32)
    bhn = np.asarray(b_hn, np.float32)[None]

    in_maps = []
    for c in range(NCORES):
        sl = slice(RC * c, RC * c + RC)
        pm = p_all[:, sl]                       # [9, RC, 2]
        vm = v_all[:, sl]
        lhs = np.concatenate([2 * pm.transpose(0, 2, 1),
                              -np.ones((NE + 1, 2, RC), np.float32)], axis=1)  # [9,4,RC]
        lhs_hi, lhs_lo = _split_bf16(lhs.astype(np.float32))
        vrhs = np.concatenate([4 * vm.transpose(0, 2, 1),
                               np.ones((NE + 1, 1, RC), np.float32)], axis=1)
        own4 = table[:, sl].reshape(NE + 1, 2, 128, 4)
        initp = np.stack([observed[-2, sl].T, observed[-1, sl].T])  # [2, 2, RC]
        in_maps.append({
            "lhs_hi": lhs_hi, "lhs_lo": lhs_lo,
            "rhs_hi": rhs_hi, "rhs_lo": rhs_lo,
            "vrhs": vrhs.astype(np.float32), "own4": own4.astype(np.float32),
            "tab": table.reshape(-1, 4), "initp": initp.astype(np.float32),
            "iota8": iota8, "scabc": scabc, "onesr": ones_row, "zerosr": zeros_rows,
            "wblk": wblk, "wihp": wihp, "whhp": whhp, "wh2p": wh2p, "bh2p": bh2p,
            "wie": wie, "wiha": wiha, "whhm": whhm, "whn": whn, "bhn": bhn,
        })

    kernel.last_in_maps = in_maps
    res = run_bass_kernel_spmd(nc, in_maps=in_maps, core_ids=list(range(NCORES)))
    kernel.last_results = res

    nrm = np.stack([r["out_nrm"] for r in res.results])   # [8c, nt, 5, RC]
    dpos = np.stack([r["out_pos"] for r in res.results])  # [8c, nd, 2, RC]
    normals = nrm.transpose(1, 0, 3, 2).reshape(nt, N, 5)
    dec_pos = dpos.transpose(1, 0, 3, 2).reshape(nd, N, 2)
    enc_pos = observed[1:] + normals[:NE, :, 0:2]
    positions = np.concatenate([enc_pos, dec_pos], axis=0)
    return normals.astype(np.float32), positions.astype(np.float32)


# revision 17
# speedup vs baseline: 1.1268x; 1.0280x over previous
"""Trainium2 Bass kernel for nn_DPoolLSTM (social-pooling LSTM trajectory model).

Sharding: 8 cores x 256 agents (data parallel over agent rows).
Per step: neighbor top-4 search over all 2048 agents (bf16-split score matmul,
DVE max8/max_index, indirect-DMA gather, exact fp32 recheck), pool-LSTM +
main-LSTM in transposed layout (fp32r matmuls). Decode steps exchange
predicted positions across cores with an AllGather collective.
"""

import contextlib
import ctypes
import sys
import types

import numpy as np

N = 2048
RC = 256          # agents per core
NCORES = 8
T_OBS = 9
NE = 8            # encoder steps
POOL_HID = 256
POOL_OUT = 32
EMB = 64
HID = 128
LSTM_IN = 96

_SO_PATH = "/opt/axon/libaxon_pjrt.so"


def _install_ntff_hook():
    """Provide antenv.axon_hooks so run_bass_kernel_spmd(trace=True) works."""
    if "antenv.axon_hooks" in sys.modules:
        return
    state = {"hook": None}

    def set_hook(h):
        state["hook"] = h

    def get_hook():
        return state["hook"]

    mod = types.ModuleType("antenv.axon_hooks")
    mod.set_axon_ntff_profile_hook = set_hook
    mod.get_axon_ntff_profile_hook = get_hook
    sys.modules["antenv.axon_hooks"] = mod

    try:
        lib = ctypes.CDLL(_SO_PATH)
    except OSError:
        return
    if not hasattr(lib, "axon_start_nrt_profile"):
        return
    lib.axon_start_nrt_profile.argtypes = [ctypes.POINTER(ctypes.c_int64), ctypes.c_size_t]
    lib.axon_start_nrt_profile.restype = ctypes.c_int64
    lib.axon_stop_nrt_profile.argtypes = [ctypes.c_char_p]
    lib.axon_stop_nrt_profile.restype = ctypes.c_int64

    @contextlib.contextmanager
    def _hook_cm(output_dir, device_ids):
        import jax

        jax.devices()
        if device_ids:
            ids = (ctypes.c_int64 * len(device_ids))(*device_ids)
            rc = lib.axon_start_nrt_profile(ids, len(device_ids))
        else:
            rc = lib.axon_start_nrt_profile(None, 0)
        if rc != 0:
            raise RuntimeError(f"axon_start_nrt_profile rc={rc}")
        try:
            yield
        finally:
            n = lib.axon_stop_nrt_profile(str(output_dir).encode())
            print(f"ntff profile: {n} file(s) -> {output_dir}", file=sys.stderr)

    set_hook(_hook_cm)


def _split_bf16(x):
    import ml_dtypes

    hi = x.astype(ml_dtypes.bfloat16)
    lo = (x - hi.astype(np.float32)).astype(ml_dtypes.bfloat16)
    return hi, lo


def _build_module(nd):
    import concourse.bass as bass
    import concourse.tile as tile
    from concourse import bacc, mybir
    from concourse.masks import make_identity

    f32 = mybir.dt.float32
    f32r = mybir.dt.float32r
    bf16 = mybir.dt.bfloat16
    u32 = mybir.dt.uint32
    Alu = mybir.AluOpType
    Act = mybir.ActivationFunctionType

    nt = NE + nd  # total steps
    nc = bacc.Bacc("TRN2", target_bir_lowering=False, num_devices=NCORES)

    # ---- external inputs ----
    def din(name, shape, dtype=f32):
        return nc.dram_tensor(name, shape, dtype, kind="ExternalInput")

    lhs_hi_in = din("lhs_hi", [NE + 1, 4, RC], bf16)
    lhs_lo_in = din("lhs_lo", [NE + 1, 4, RC], bf16)
    rhs_hi_in = din("rhs_hi", [NE + 1, 4, N], bf16)
    rhs_lo_in = din("rhs_lo", [NE + 1, 4, N], bf16)
    vrhs_in = din("vrhs", [NE + 1, 3, RC], f32r)
    own4_in = din("own4", [NE + 1, 2, 128, 4])
    table_in = din("tab", [(NE + 1) * N, 4])
    initp_in = din("initp", [2, 2, RC])          # [prev2T, prev1T]
    iota8_in = din("iota8", [128, 8], u32)
    scabc_in = din("scabc", [5, 3])
    ones_in = din("onesr", [1, RC], f32r)
    zeros_in = din("zerosr", [2, RC], f32r)
    wblk_in = din("wblk", [17, POOL_OUT], f32r)
    wihp_in = din("wihp", [33, 4 * POOL_HID], f32r)
    whhp_in = din("whhp", [2, 128, 4 * POOL_HID], f32r)
    wh2p_in = din("wh2p", [2, 128, POOL_OUT], f32r)
    bh2p_in = din("bh2p", [1, POOL_OUT], f32r)
    wie_in = din("wie", [3, EMB - 2], f32r)
    wiha_in = din("wiha", [2, LSTM_IN + 1, 4 * HID], f32r)
    whhm_in = din("whhm", [2, 128, 4 * HID], f32r)
    whn_in = din("whn", [128, 5], f32r)
    bhn_in = din("bhn", [1, 5], f32r)

    out_nrm = nc.dram_tensor("out_nrm", [nt, 5, RC], f32, kind="ExternalOutput")
    out_pos = nc.dram_tensor("out_pos", [max(nd, 1), 2, RC], f32, kind="ExternalOutput")

    with tile.TileContext(nc) as tc:
        ex = contextlib.ExitStack()
        P = ex.enter_context  # pools live until module end

        pers = P(tc.tile_pool(name="pers", bufs=1))
        dram = P(tc.tile_pool(name="dram", bufs=2, space="DRAM"))
        ps_big = P(tc.tile_pool(name="ps_big", bufs=3, space="PSUM"))
        ps_med = P(tc.tile_pool(name="ps_med", bufs=3, space="PSUM"))
        ps_sm = P(tc.tile_pool(name="ps_sm", bufs=2, space="PSUM"))
        sb_big = P(tc.tile_pool(name="sb_big", bufs=3))
        sb_sm = P(tc.tile_pool(name="sb_sm", bufs=4))
        sb_gate = P(tc.tile_pool(name="sb_gate", bufs=4))

        # ---- persistent tiles ----
        ident_r = pers.tile([128, 128], f32r, tag="ident_r")
        ident_f = pers.tile([128, 128], f32, tag="ident_f")
        make_identity(nc, ident_f[:])
        nc.vector.tensor_copy(ident_r[:], ident_f[:])
        iota8 = pers.tile([128, 8], u32, tag="iota8")
        scabc = pers.tile([5, 3], f32, tag="scabc")
        ones_r = pers.tile([1, RC], f32r, tag="ones_r")
        nc.sync.dma_start(iota8[:], iota8_in[:])
        nc.sync.dma_start(scabc[:], scabc_in[:])
        nc.sync.dma_start(ones_r[:], ones_in[:])

        lhs_hi = pers.tile([4, RC], bf16, tag="lhs_hi")
        lhs_lo = pers.tile([4, RC], bf16, tag="lhs_lo")
        vrhs = pers.tile([3, RC], f32r, tag="vrhs")
        nc.sync.dma_start(lhs_hi[:], lhs_hi_in[NE])
        nc.sync.dma_start(lhs_lo[:], lhs_lo_in[NE])
        nc.sync.dma_start(vrhs[:], vrhs_in[NE])
        own4 = [pers.tile([128, 4], f32, tag=f"own4_{m}", name=f"own4_{m}") for m in range(2)]

        xg = pers.tile([17, RC], f32r, tag="xg")
        nc.sync.dma_start(xg[16:17, :], ones_in[:])
        gt_aug = pers.tile([33, RC], f32r, tag="gt_aug")
        nc.sync.dma_start(gt_aug[32:33, :], ones_in[:])
        xT = pers.tile([LSTM_IN + 1, RC], f32r, tag="xT")
        nc.sync.dma_start(xT[62:64, :], zeros_in[:])
        nc.sync.dma_start(xT[96:97, :], ones_in[:])

        wblk = pers.tile([17, POOL_OUT], f32r, tag="wblk")
        wihp = pers.tile([33, 4 * POOL_HID], f32r, tag="wihp")
        whhp = [pers.tile([128, 4 * POOL_HID], f32r, tag=f"whhp{k}", name=f"whhp{k}") for k in range(2)]
        wh2p = [pers.tile([128, POOL_OUT], f32r, tag=f"wh2p{k}", name=f"wh2p{k}") for k in range(2)]
        bh2p = pers.tile([1, POOL_OUT], f32r, tag="bh2p")
        wie = pers.tile([3, EMB - 2], f32r, tag="wie")
        wiha = [pers.tile([LSTM_IN + 1, 4 * HID], f32r, tag=f"wiha{k}", name=f"wiha{k}") for k in range(2)]
        whhm = [pers.tile([128, 4 * HID], f32r, tag=f"whhm{k}", name=f"whhm{k}") for k in range(2)]
        whn = pers.tile([128, 5], f32r, tag="whn")
        bhn = pers.tile([1, 5], f32r, tag="bhn")
        nc.sync.dma_start(wblk[:], wblk_in[:])
        nc.sync.dma_start(wihp[:], wihp_in[:])
        for k in range(2):
            nc.sync.dma_start(whhp[k][:], whhp_in[k])
            nc.sync.dma_start(wh2p[k][:], wh2p_in[k])
            nc.sync.dma_start(wiha[k][:], wiha_in[k])
            nc.sync.dma_start(whhm[k][:], whhm_in[k])
        nc.sync.dma_start(bh2p[:], bh2p_in[:])
        nc.sync.dma_start(wie[:], wie_in[:])
        nc.sync.dma_start(whn[:], whn_in[:])
        nc.sync.dma_start(bhn[:], bhn_in[:])

        hpT = [pers.tile([128, RC], f32r, tag=f"hpT{k}", name=f"hpT{k}") for k in range(2)]
        cpT = [pers.tile([128, RC], f32, tag=f"cpT{k}", name=f"cpT{k}") for k in range(2)]
        hT = pers.tile([128, RC], f32r, tag="hT")
        cT = pers.tile([128, RC], f32, tag="cT")
        for k in range(2):
            nc.vector.memset(cpT[k][:], 0.0)
            nc.vector.tensor_copy(hpT[k][:], cpT[k][:])
        nc.vector.memset(cT[:], 0.0)
        nc.vector.tensor_copy(hT[:], cT[:])

        posT = [pers.tile([2, RC], f32, tag=f"posT{k}", name=f"posT{k}") for k in range(3)]
        nc.sync.dma_start(posT[0][:], initp_in[0])  # prev2T
        nc.sync.dma_start(posT[1][:], initp_in[1])  # prev1T
        prev2, prev1, nxt = posT[0], posT[1], posT[2]

        table_view = table_in[:]  # [(NE+1)*N, 4]

        def neighbor_block(t, lhsh, lhsl, rhsh, rhsl, own4p, tabv, hostfed, fouts):
            for m in range(2):
                s_sb = sb_big.tile([128, N], f32, tag="s_sb", name=f"s_sb_{t}_{m}")
                for b in range(4):
                    sl = slice(512 * b, 512 * b + 512)
                    ps_s = ps_big.tile([128, 512], f32, tag="ps_s", name=f"ps_s_{t}_{m}_{b}")
                    nc.tensor.matmul(ps_s[:], lhsh[:, 128 * m:128 * m + 128],
                                     rhsh[:, sl], start=True, stop=False)
                    nc.tensor.matmul(ps_s[:], lhsh[:, 128 * m:128 * m + 128],
                                     rhsl[:, sl], start=False, stop=False)
                    nc.tensor.matmul(ps_s[:], lhsl[:, 128 * m:128 * m + 128],
                                     rhsh[:, sl], start=False, stop=True)
                    nc.scalar.copy(s_sb[:, sl], ps_s[:])
                mx = sb_sm.tile([128, 8], f32, tag="mx", name=f"mx_{t}_{m}")
                mi = sb_sm.tile([128, 8], u32, tag="mi", name=f"mi_{t}_{m}")
                nc.vector.max(mx[:], s_sb[:])
                nc.vector.max_index(mi[:], mx[:], s_sb[:])
                adj = sb_sm.tile([128, 8], u32, tag="adj", name=f"adj_{t}_{m}")
                if hostfed:
                    nc.vector.tensor_scalar(adj[:], mi[:], N * t, None, op0=Alu.add)
                else:
                    nc.vector.tensor_scalar(adj[:], mi[:], 0xFFFFFF00, None,
                                            op0=Alu.bitwise_and)
                    nc.vector.tensor_tensor(out=adj[:], in0=adj[:], in1=mi[:],
                                            op=Alu.add)
                cand = sb_sm.tile([128, 8, 4], f32, tag="cand", name=f"cand_{t}_{m}")
                # slot 7 = far-away sentinel: top-7 candidates suffice (a miss
                # needs a 3-way score tie within the ~1e-3 bf16-split noise)
                nc.vector.memset(cand[:, 7, :], 1.0e9)
                for r in range(7):
                    nc.gpsimd.indirect_dma_start(
                        out=cand[:, r, :], out_offset=None, in_=tabv,
                        in_offset=bass.IndirectOffsetOnAxis(ap=adj[:, r:r + 1], axis=0))
                rel = sb_sm.tile([128, 8, 2], f32, tag="rel", name=f"rel_{t}_{m}")
                nc.vector.tensor_tensor(
                    out=rel[:], in0=cand[:, :, 0:2],
                    in1=own4p[m][:, 0:2].unsqueeze(1).to_broadcast([128, 8, 2]),
                    op=Alu.subtract)
                rel2 = sb_sm.tile([128, 8, 2], f32, tag="rel2", name=f"rel2_{t}_{m}")
                nc.vector.tensor_tensor(out=rel2[:], in0=rel[:], in1=rel[:],
                                        op=Alu.mult)
                d2 = sb_sm.tile([128, 8], f32, tag="d2", name=f"d2_{t}_{m}")
                nc.vector.reduce_sum(d2[:], rel2[:], axis=mybir.AxisListType.X)
                d2n = sb_sm.tile([128, 8], f32, tag="d2n", name=f"d2n_{t}_{m}")
                nc.vector.tensor_scalar(d2n[:], d2[:], -1.0, None, op0=Alu.mult)
                srt = sb_sm.tile([128, 8], f32, tag="srt", name=f"srt_{t}_{m}")
                ordv = sb_sm.tile([128, 8], u32, tag="ordv", name=f"ordv_{t}_{m}")
                nc.vector.max(srt[:], d2n[:])
                nc.vector.max_index(ordv[:], srt[:], d2n[:])
                mask4 = sb_sm.tile([128, 4, 8], f32, tag="mask4", name=f"mask4_{t}_{m}")
                nc.vector.tensor_tensor(
                    out=mask4[:],
                    in0=ordv[:, 1:5].unsqueeze(2).to_broadcast([128, 4, 8]),
                    in1=iota8[:].unsqueeze(1).to_broadcast([128, 4, 8]),
                    op=Alu.is_equal)
                prod4 = sb_sm.tile([128, 4, 4, 8], f32, tag="prod4", name=f"prod4_{t}_{m}")
                nc.vector.tensor_tensor(
                    out=prod4[:],
                    in0=mask4[:].unsqueeze(2).to_broadcast([128, 4, 4, 8]),
                    in1=cand[:].rearrange("p s e -> p e s").unsqueeze(1)
                        .to_broadcast([128, 4, 4, 8]),
                    op=Alu.mult)
                selv = sb_sm.tile([128, 4, 4], f32, tag="selv", name=f"selv_{t}_{m}")
                nc.vector.reduce_sum(selv[:], prod4[:], axis=mybir.AxisListType.X)
                nc.vector.tensor_tensor(
                    out=fouts[m][:], in0=selv[:],
                    in1=own4p[m][:].unsqueeze(1).to_broadcast([128, 4, 4]),
                    op=Alu.subtract)

        # ---- phase A: all host-fed neighbor searches, densely packed ----
        nhost = min(NE + 1, nt)
        feat_store = []
        vrhs_store = []
        for t in range(nhost):
            rhs_hi_t = sb_big.tile([4, N], bf16, tag="rhs_hi", name=f"rhsh_{t}")
            rhs_lo_t = sb_big.tile([4, N], bf16, tag="rhs_lo", name=f"rhsl_{t}")
            nc.sync.dma_start(rhs_hi_t[:], rhs_hi_in[t])
            nc.sync.dma_start(rhs_lo_t[:], rhs_lo_in[t])
            lhsh_t = sb_sm.tile([4, RC], bf16, tag="lhsA", name=f"lhsh_{t}")
            lhsl_t = sb_sm.tile([4, RC], bf16, tag="lhsB", name=f"lhsl_{t}")
            nc.sync.dma_start(lhsh_t[:], lhs_hi_in[t])
            nc.sync.dma_start(lhsl_t[:], lhs_lo_in[t])
            vr_t = pers.tile([3, RC], f32r, tag=f"vrA{t}", name=f"vrA{t}")
            nc.sync.dma_start(vr_t[:], vrhs_in[t])
            vrhs_store.append(vr_t)
            o4_t = [sb_sm.tile([128, 4], f32, tag=f"own4A{m}", name=f"own4A_{t}_{m}")
                    for m in range(2)]
            for m in range(2):
                nc.sync.dma_start(o4_t[m][:], own4_in[t, m])
            f_t = [pers.tile([128, 4, 4], f32r, tag=f"featS{t}{m}", name=f"featS{t}{m}")
                   for m in range(2)]
            neighbor_block(t, lhsh_t, lhsl_t, rhs_hi_t, rhs_lo_t, o4_t,
                           table_view, True, f_t)
            feat_store.append(f_t)

        for t in range(nt):
            enc = t < NE
            hostfed = t <= NE
            widx = 0 if enc else 1

            if hostfed:
                pass
            else:
                # rhs from last step's all-gather (ccout): blocks of 2048 f32
                # [table 1024 | xyhi 512b | xylo 512b | sqhi 512b | sqlo 512b]
                rhs_hi = sb_big.tile([4, N], bf16, tag="rhs_hi", name=f"rhshD_{t}")
                rhs_lo = sb_big.tile([4, N], bf16, tag="rhs_lo", name=f"rhslD_{t}")
                cc_bf = ccout[:].bitcast(bf16)  # [8, 4096]
                for dst, off in ((rhs_hi, 2048), (rhs_lo, 2560)):
                    # xy rows -> partitions 0..1 ; sq rows -> partitions 2..3
                    nc.sync.dma_start(
                        dst[0:2, :].rearrange("p (c j) -> p c j", c=8),
                        cc_bf[:, off:off + 512].rearrange("c (p j) -> p c j", p=2))
                    nc.sync.dma_start(
                        dst[2:4, :].rearrange("p (c j) -> p c j", c=8),
                        cc_bf[:, off + 1024:off + 1536].rearrange("c (p j) -> p c j", p=2))

            if hostfed:
                feats = feat_store[t]
            else:
                feats = [sb_sm.tile([128, 4, 4], f32r, tag=f"featD{m}",
                                    name=f"featD_{t}_{m}") for m in range(2)]
                tabv_d = ccout[:].rearrange("c (r e) -> (c r) e", e=4)
                neighbor_block(t, lhs_hi, lhs_lo, rhs_hi, rhs_lo, own4,
                               tabv_d, False, feats)

            # featT -> xg rows 0..15
            ftp = ps_sm.tile([16, RC], f32r, tag="sm")
            for m in range(2):
                nc.tensor.transpose(ftp[:, 128 * m:128 * m + 128],
                                    feats[m][:].rearrange("p a b -> p (a b)"),
                                    ident_r[:])
            nc.scalar.copy(xg[0:16, :], ftp[:])
            gps = ps_sm.tile([POOL_OUT, RC], f32, tag="sm")
            nc.tensor.matmul(gps[:], wblk[:], xg[:], start=True, stop=True)
            nc.scalar.activation(gt_aug[0:32, :], gps[:], Act.Relu)

            # pool LSTM gates: [1024, RC] in 8 tiles; order i,f,g,o x 2
            pg = []
            for mt in range(8):
                sl = slice(128 * mt, 128 * mt + 128)
                pt = ps_med.tile([128, RC], f32, tag="pg")
                nc.tensor.matmul(pt[:], whhp[0][:, sl], hpT[0][:], start=True, stop=False)
                nc.tensor.matmul(pt[:], whhp[1][:, sl], hpT[1][:], start=False, stop=False)
                nc.tensor.matmul(pt[:], wihp[:, sl], gt_aug[:], start=False, stop=True)
                pg.append(pt)
            for ht in range(2):
                i_sb = sb_gate.tile([128, RC], f32, tag="i_sb")
                f_sb = sb_gate.tile([128, RC], f32, tag="f_sb")
                g_sb = sb_gate.tile([128, RC], f32, tag="g_sb")
                o_sb = sb_gate.tile([128, RC], f32, tag="o_sb")
                nc.scalar.activation(i_sb[:], pg[0 + ht][:], Act.Sigmoid)
                nc.scalar.activation(f_sb[:], pg[2 + ht][:], Act.Sigmoid)
                nc.scalar.activation(g_sb[:], pg[4 + ht][:], Act.Tanh)
                nc.scalar.activation(o_sb[:], pg[6 + ht][:], Act.Sigmoid)
                tmp = sb_gate.tile([128, RC], f32, tag="tmp")
                nc.vector.tensor_tensor(out=tmp[:], in0=i_sb[:], in1=g_sb[:], op=Alu.mult)
                nc.vector.tensor_tensor(out=cpT[ht][:], in0=f_sb[:], in1=cpT[ht][:], op=Alu.mult)
                nc.vector.tensor_tensor(out=cpT[ht][:], in0=cpT[ht][:], in1=tmp[:], op=Alu.add)
                th = sb_gate.tile([128, RC], f32, tag="th")
                nc.scalar.activation(th[:], cpT[ht][:], Act.Tanh)
                nc.vector.tensor_tensor(out=hpT[ht][:], in0=o_sb[:], in1=th[:], op=Alu.mult)

            # pooled -> xT rows 64..95 ; emb -> xT rows 0..61
            plp = ps_sm.tile([POOL_OUT, RC], f32, tag="sm")
            nc.tensor.matmul(plp[:], bh2p[:], ones_r[:], start=True, stop=False)
            nc.tensor.matmul(plp[:], wh2p[0][:], hpT[0][:], start=False, stop=False)
            nc.tensor.matmul(plp[:], wh2p[1][:], hpT[1][:], start=False, stop=True)
            nc.scalar.copy(xT[64:96, :], plp[:])
            ebp = ps_sm.tile([EMB - 2, RC], f32, tag="sm")
            nc.tensor.matmul(ebp[:], wie[:], (vrhs_store[t] if hostfed else vrhs)[:], start=True, stop=True)
            nc.scalar.activation(xT[0:62, :], ebp[:], Act.Relu)

            # main LSTM
            mg = []
            for mt in range(4):
                sl = slice(128 * mt, 128 * mt + 128)
                gtl = ps_med.tile([128, RC], f32, tag="pg")
                nc.tensor.matmul(gtl[:], whhm[widx][:, sl], hT[:], start=True, stop=False)
                nc.tensor.matmul(gtl[:], wiha[widx][:, sl], xT[:], start=False, stop=True)
                mg.append(gtl)
            i2 = sb_gate.tile([128, RC], f32, tag="i2")
            f2 = sb_gate.tile([128, RC], f32, tag="f2")
            g2 = sb_gate.tile([128, RC], f32, tag="g2")
            o2 = sb_gate.tile([128, RC], f32, tag="o2")
            nc.scalar.activation(i2[:], mg[0][:], Act.Sigmoid)
            nc.scalar.activation(f2[:], mg[1][:], Act.Sigmoid)
            nc.scalar.activation(g2[:], mg[2][:], Act.Tanh)
            nc.scalar.activation(o2[:], mg[3][:], Act.Sigmoid)
            tmp2 = sb_gate.tile([128, RC], f32, tag="tmp2")
            nc.vector.tensor_tensor(out=tmp2[:], in0=i2[:], in1=g2[:], op=Alu.mult)
            nc.vector.tensor_tensor(out=cT[:], in0=f2[:], in1=cT[:], op=Alu.mult)
            nc.vector.tensor_tensor(out=cT[:], in0=cT[:], in1=tmp2[:], op=Alu.add)
            th2 = sb_gate.tile([128, RC], f32, tag="th2")
            nc.scalar.activation(th2[:], cT[:], Act.Tanh)
            nc.vector.tensor_tensor(out=hT[:], in0=o2[:], in1=th2[:], op=Alu.mult)

            # normal = a*raw + b*sigmoid(raw) + c
            nrp = ps_sm.tile([5, RC], f32, tag="sm")
            nc.tensor.matmul(nrp[:], bhn[:], ones_r[:], start=True, stop=False)
            nc.tensor.matmul(nrp[:], whn[:], hT[:], start=False, stop=True)
            sgm = sb_sm.tile([5, RC], f32, tag="sgm")
            nc.scalar.activation(sgm[:], nrp[:], Act.Sigmoid)
            t1 = sb_sm.tile([5, RC], f32, tag="t1n")
            nc.vector.tensor_scalar(t1[:], nrp[:], scabc[:, 0:1], None, op0=Alu.mult)
            nrm = sb_sm.tile([5, RC], f32, tag="nrm")
            nc.vector.tensor_scalar(nrm[:], sgm[:], scabc[:, 1:2], scabc[:, 2:3],
                                    op0=Alu.mult, op1=Alu.add)
            nc.vector.tensor_tensor(out=nrm[:], in0=nrm[:], in1=t1[:], op=Alu.add)
            nc.sync.dma_start(out_nrm[t], nrm[:])

            if t >= NE:
                nc.vector.tensor_tensor(out=nxt[:], in0=prev1[:], in1=nrp[0:2, :],
                                        op=Alu.add)
                nc.sync.dma_start(out_pos[t - NE], nxt[:])
                if t < nt - 1:
                    velT = sb_sm.tile([2, RC], f32, tag="velT")
                    nc.vector.tensor_tensor(out=velT[:], in0=nxt[:], in1=prev1[:],
                                            op=Alu.subtract)
                    # next-step lhsT rows 0-1 = split(2*nxt)
                    l32 = sb_sm.tile([2, RC], f32, tag="l32")
                    nc.vector.tensor_scalar(l32[:], nxt[:], 2.0, None, op0=Alu.mult)
                    nc.vector.tensor_copy(lhs_hi[0:2, :], l32[:])
                    nc.vector.tensor_tensor(out=lhs_lo[0:2, :], in0=l32[:],
                                            in1=lhs_hi[0:2, :], op=Alu.subtract)
                    nc.vector.tensor_scalar(vrhs[0:2, :], velT[:], 4.0, None,
                                            op0=Alu.mult)
                    # payload pieces
                    sq32 = sb_sm.tile([2, RC], f32, tag="sq32")
                    nc.vector.tensor_tensor(out=sq32[:], in0=nxt[:], in1=nxt[:],
                                            op=Alu.mult)
                    xyhi = sb_sm.tile([2, RC], bf16, tag="xyhi")
                    xylo = sb_sm.tile([2, RC], bf16, tag="xylo")
                    sqhi = sb_sm.tile([2, RC], bf16, tag="sqhi")
                    sqlo = sb_sm.tile([2, RC], bf16, tag="sqlo")
                    nc.vector.tensor_copy(xyhi[:], nxt[:])
                    nc.vector.tensor_tensor(out=xylo[:], in0=nxt[:], in1=xyhi[:],
                                            op=Alu.subtract)
                    nc.vector.tensor_copy(sqhi[:], sq32[:])
                    nc.vector.tensor_tensor(out=sqlo[:], in0=sq32[:], in1=sqhi[:],
                                            op=Alu.subtract)
                    # payload bf16 blocks first (ready early; Sync queue is in-order)
                    bounce = dram.tile([1, 2048], f32, tag="bounce")
                    ccout = dram.tile([8, 2048], f32, tag="ccout")
                    bb = bounce[:].bitcast(bf16)  # [1, 4096]
                    nc.sync.dma_start(bb[:, 2048:2560].rearrange("o (p j) -> (o p) j", p=2), xyhi[:])
                    nc.sync.dma_start(bb[:, 2560:3072].rearrange("o (p j) -> (o p) j", p=2), xylo[:])
                    nc.sync.dma_start(bb[:, 3072:3584].rearrange("o (p j) -> (o p) j", p=2), sqhi[:])
                    nc.sync.dma_start(bb[:, 3584:4096].rearrange("o (p j) -> (o p) j", p=2), sqlo[:])
                    for m in range(2):
                        tp = ps_sm.tile([128, 2], f32, tag="sm")
                        nc.tensor.transpose(tp[:], nxt[:, 128 * m:128 * m + 128],
                                            ident_f[0:2, 0:2])
                        nc.scalar.copy(own4[m][:, 0:2], tp[:])
                        tv = ps_sm.tile([128, 2], f32, tag="sm")
                        nc.tensor.transpose(tv[:], velT[:, 128 * m:128 * m + 128],
                                            ident_f[0:2, 0:2])
                        nc.scalar.copy(own4[m][:, 2:4], tv[:])
                    for m in range(2):
                        nc.sync.dma_start(
                            bounce[:, 512 * m:512 * m + 512]
                            .rearrange("o (p e) -> o p e", p=128).squeeze(0),
                            own4[m][:])
                    nc.gpsimd.collective_compute(
                        "AllGather", Alu.bypass,
                        replica_groups=[list(range(NCORES))],
                        ins=[bounce.opt()], outs=[ccout.opt()])
                prev2, prev1, nxt = prev1, nxt, prev2

        ex.close()
    nc.compile()
    return nc


_CACHE = {}


def kernel(observed, goals, batch_split, n_predict,
           W_ie, b_ie, W_pe, b_pe,
           Wih_p, Whh_p, bih_p, bhh_p, W_h2p, b_h2p,
           Wih_e, Whh_e, bih_e, bhh_e,
           Wih_d, Whh_d, bih_d, bhh_d,
           W_hn, b_hn):
    import ml_dtypes

    _install_ntff_hook()
    from concourse.bass_utils import run_bass_kernel_spmd

    observed = np.asarray(observed, np.float32)
    nd = int(n_predict)
    nt = NE + nd

    if nd not in _CACHE:
        _CACHE[nd] = _build_module(nd)
    nc = _CACHE[nd]

    # ---- host-side input prep ----
    obs1 = observed[:-1]                    # [8, N, 2]
    obs2 = observed[1:]                     # [8, N, 2]
    # step t (t=0..7): (obs1[t], obs2[t]); step 8: (observed[-2], observed[-1])
    p_all = np.concatenate([obs2, observed[-1:None]], axis=0)       # [9, N, 2]
    v_all = np.concatenate([obs2 - obs1, (observed[-1] - observed[-2])[None]], axis=0)

    sq_all = p_all * p_all                                          # [9, N, 2]
    rhs = np.concatenate([p_all, sq_all], axis=2).transpose(0, 2, 1)  # [9, 4, N]
    rhs_hi, rhs_lo = _split_bf16(rhs.astype(np.float32))

    table = np.concatenate([p_all, v_all], axis=2).astype(np.float32)  # [9, N, 4]

    iota8 = np.broadcast_to(np.arange(8, dtype=np.uint32), (128, 8)).copy()
    scabc = np.array([[1, 0, 0], [1, 0, 0], [0, 0.2, 0.01], [0, 0.2, 0.01],
                      [0, 0.7, 0]], np.float32)
    ones_row = np.ones((1, RC), np.float32)
    zeros_rows = np.zeros((2, RC), np.float32)

    W_pe = np.asarray(W_pe, np.float32)
    wblk = np.zeros((17, POOL_OUT), np.float32)
    for k in range(4):
        wblk[4 * k:4 * k + 4, 8 * k:8 * k + 8] = W_pe
    wblk[16, :] = np.tile(np.asarray(b_pe, np.float32), 4)

    wihp = np.concatenate([np.asarray(Wih_p, np.float32),
                           (np.asarray(bih_p) + np.asarray(bhh_p)).astype(np.float32)[None]], axis=0)
    whhp = np.asarray(Whh_p, np.float32).reshape(2, 128, 4 * POOL_HID)
    wh2p = np.asarray(W_h2p, np.float32).reshape(2, 128, POOL_OUT)
    bh2p = np.asarray(b_h2p, np.float32)[None]
    wie = np.concatenate([np.asarray(W_ie, np.float32),
                          np.asarray(b_ie, np.float32)[None]], axis=0)  # [3, 62]
    wiha = np.stack([
        np.concatenate([np.asarray(Wih_e, np.float32),
                        (np.asarray(bih_e) + np.asarray(bhh_e)).astype(np.float32)[None]], axis=0),
        np.concatenate([np.asarray(Wih_d, np.float32),
                        (np.asarray(bih_d) + np.asarray(bhh_d)).astype(np.float32)[None]], axis=0)])
    whhm = np.stack([np.asarray(Whh_e, np.float32), np.asarray(Whh_d, np.float32)])
    whn = np.asarray(W_hn, np.float32)
    bhn = np.asarray(b_hn, np.float32)[None]

    in_maps = []
    for c in range(NCORES):
        sl = slice(RC * c, RC * c + RC)
        pm = p_all[:, sl]                       # [9, RC, 2]
        vm = v_all[:, sl]
        lhs = np.concatenate([2 * pm.transpose(0, 2, 1),
                              -np.ones((NE + 1, 2, RC), np.float32)], axis=1)  # [9,4,RC]
        lhs_hi, lhs_lo = _split_bf16(lhs.astype(np.float32))
        vrhs = np.concatenate([4 * vm.transpose(0, 2, 1),
                               np.ones((NE + 1, 1, RC), np.float32)], axis=1)
        own4 = table[:, sl].reshape(NE + 1, 2, 128, 4)
        initp = np.stack([observed[-2, sl].T, observed[-1, sl].T])  # [2, 2, RC]
        in_maps.append({
            "lhs_hi": lhs_hi, "lhs_lo": lhs_lo,
            "rhs_hi": rhs_hi, "rhs_lo": rhs_lo,
            "vrhs": vrhs.astype(np.float32), "own4": own4.astype(np.float32),
            "tab": table.reshape(-1, 4), "initp": initp.astype(np.float32),
            "iota8": iota8, "scabc": scabc, "onesr": ones_row, "zerosr": zeros_rows,
            "wblk": wblk, "wihp": wihp, "whhp": whhp, "wh2p": wh2p, "bh2p": bh2p,
            "wie": wie, "wiha": wiha, "whhm": whhm, "whn": whn, "bhn": bhn,
        })

    kernel.last_in_maps = in_maps
    res = run_bass_kernel_spmd(nc, in_maps=in_maps, core_ids=list(range(NCORES)))
    kernel.last_results = res

    nrm = np.stack([r["out_nrm"] for r in res.results])   # [8c, nt, 5, RC]
    dpos = np.stack([r["out_pos"] for r in res.results])  # [8c, nd, 2, RC]
    normals = nrm.transpose(1, 0, 3, 2).reshape(nt, N, 5)
    dec_pos = dpos.transpose(1, 0, 3, 2).reshape(nd, N, 2)
    enc_pos = observed[1:] + normals[:NE, :, 0:2]
    positions = np.concatenate([enc_pos, dec_pos], axis=0)
    return normals.astype(np.float32), positions.astype(np.float32)
